# revision 1
# baseline (speedup 1.0000x reference)
"""Multi-head causal self-attention with RoPE, tensor-parallel over heads
across 8 Trainium2 NeuronCores.

Strategy (Megatron-style TP over heads):
  - Each core owns 2 of the 16 heads: rows [c*256,(c+1)*256) of Wq/Wk/Wv
    and the matching columns of Wo.
  - On-core: qT/kT projections in transposed [d, s] layout (natural matmul
    output layout), RoPE via a signed-permutation matmul + elementwise ops,
    v in natural [s, d] layout, causal attention with scores computed
    transposed (S^T = K Q^T, softmax sum via a ones-matmul, no running max
    needed -- scores are O(10) so exp() cannot overflow), then a partial
    output projection against the core's Wo column-slice.
  - Host sums the 8 partial outputs (this replaces the TP all-reduce).

All matmuls run on float32r operands (full-rate fp32 on the PE). The BIR
verifier requires float32r matmul inputs to be produced as float32r, so
DRAM-sourced operands are declared float32r and engine-produced operands
(RoPE'd q/k, exp(scores), v, u) are written with float32r output dtype.
"""

import sys

import numpy as np

B, S, DIM = 2, 2048, 2048
NUM_HEADS = 16
HD = 128
N_CORES = 8
HPC = NUM_HEADS // N_CORES  # heads per core
DLOC = HPC * HD             # per-core slice of the model dim
ROPE_BASE = 10000.0
QCH = 512                   # attention q-chunk / phase-3 out-chunk
SC1 = 256                   # phase-1 s-chunk

_PROGRAM_CACHE = {}


def _rope_tables_T(seq_len, head_dim):
    # match reference float32 arithmetic: inv_freq over even indices,
    # emb = cat(freqs, freqs); returned transposed [head_dim, seq_len]
    inv_freq = (
        1.0
        / (np.float32(ROPE_BASE)
           ** (np.arange(0, head_dim, 2, dtype=np.float32) / np.float32(head_dim)))
    ).astype(np.float32)
    t = np.arange(seq_len, dtype=np.float32)
    freqs = np.outer(t, inv_freq).astype(np.float32)      # [S, D/2]
    emb = np.concatenate([freqs, freqs], axis=-1)         # [S, D]
    return (
        np.ascontiguousarray(np.cos(emb).astype(np.float32).T),
        np.ascontiguousarray(np.sin(emb).astype(np.float32).T),
    )


def _rot_matrix_T(head_dim):
    # rotated = cat(-x[1::2], x[::2]) = R @ x; return R.T [D, D]
    d2 = head_dim // 2
    R = np.zeros((head_dim, head_dim), dtype=np.float32)
    for dp in range(d2):
        R[dp, 2 * dp + 1] = -1.0
    for dp in range(d2, head_dim):
        R[dp, 2 * (dp - d2)] = 1.0
    return np.ascontiguousarray(R.T)


def _causal_masks(qch):
    # masks[i][kk, qq] = 0 if 128*i + kk <= qq else -1e9 (additive, applied
    # to raw scores before exp, for the 4 diagonal k-chunks of each q-chunk)
    m = np.zeros((4, 128, qch), dtype=np.float32)
    kk = np.arange(128)[:, None]
    qq = np.arange(qch)[None, :]
    for i in range(4):
        m[i] = np.where(128 * i + kk <= qq, 0.0, -1e9).astype(np.float32)
    return m


def build_program(b=B, s=S, dim=DIM):
    """Builds the per-core SPMD Bass program (identical on every core)."""
    if "/opt/trn_rl_repo" not in sys.path:
        sys.path.insert(0, "/opt/trn_rl_repo")
    import concourse.bacc as bacc
    import concourse.mybir as mybir
    import concourse.tile as tile

    f32 = mybir.dt.float32
    f32r = mybir.dt.float32r
    EXP = mybir.ActivationFunctionType.Exp

    bs = b * s
    n_din = dim // 128          # contraction chunks for projections
    n_s1 = bs // SC1            # phase-1 s-chunks
    n_qc = s // QCH             # attention q-chunks per batch
    n_sc3 = bs // 128           # phase-3 row chunks
    n_oc = dim // QCH           # phase-3 out-column chunks
    scale = float(HD) ** -0.5

    nc = bacc.Bacc("TRN2", target_bir_lowering=False, debug=False)

    xT_d = nc.dram_tensor("xT", [dim, bs], f32r, kind="ExternalInput")
    wqT_d = nc.dram_tensor("wqT", [dim, DLOC], f32r, kind="ExternalInput")
    wkT_d = nc.dram_tensor("wkT", [dim, DLOC], f32r, kind="ExternalInput")
    wvT_d = nc.dram_tensor("wvT", [dim, DLOC], f32r, kind="ExternalInput")
    woT_d = nc.dram_tensor("woT", [DLOC, dim], f32r, kind="ExternalInput")
    cosT_d = nc.dram_tensor("cosT", [HD, bs], f32, kind="ExternalInput")
    sinT_d = nc.dram_tensor("sinT", [HD, bs], f32, kind="ExternalInput")
    rT_d = nc.dram_tensor("rT", [HD, HD], f32r, kind="ExternalInput")
    ones_d = nc.dram_tensor("ones", [HD, HD], f32r, kind="ExternalInput")
    masks_d = nc.dram_tensor("masks", [4, HD, QCH], mybir.dt.bfloat16, kind="ExternalInput")
    out_d = nc.dram_tensor("out", [dim, bs], f32, kind="ExternalOutput")

    with tile.TileContext(nc) as tc:
        with tc.tile_pool(name="persist", bufs=1) as persist:
            # transposed roped projections [d, head, b*s]; v natural [s, chunk, d]
            qT = persist.tile([128, HPC, bs], f32r)
            kT = persist.tile([128, HPC, bs], f32r)
            vS = persist.tile([128, bs // 128, DLOC], f32r)
            rTs = persist.tile([HD, HD], f32r)
            nc.sync.dma_start(out=rTs, in_=rT_d[:])
            ones = persist.tile([128, 128], f32r)
            nc.sync.dma_start(out=ones, in_=ones_d[:])
            masks_s = persist.tile([128, 4, QCH], mybir.dt.bfloat16)
            nc.sync.dma_start(out=masks_s, in_=masks_d.rearrange("i p q -> p i q"))
            woT_s = persist.tile([128, HPC, dim], f32r)
            nc.sync.dma_start(
                out=woT_s, in_=woT_d.rearrange("(h p) n -> p h n", p=128)
            )

            # ---------------- phase 1: qkv projections + RoPE ----------------
            with (
                tc.tile_pool(name="p1w", bufs=1) as p1w,
                tc.tile_pool(name="p1x", bufs=2) as p1x,
                tc.tile_pool(name="p1t", bufs=2) as p1t,
                tc.tile_pool(name="ps_qk", bufs=4, space="PSUM") as ps_qk,
                tc.tile_pool(name="ps_rot", bufs=2, space="PSUM") as ps_rot,
                tc.tile_pool(name="ps_v", bufs=2, space="PSUM") as ps_v,
            ):
                wq_s = p1w.tile([128, n_din, DLOC], f32r)
                wk_s = p1w.tile([128, n_din, DLOC], f32r)
                wv_s = p1w.tile([128, n_din, DLOC], f32r)
                # split weight loads so the first matmuls start as soon as the
                # first contraction chunks land (DMA queues run in parallel)
                gw = max(1, n_din // 4)
                for g0 in range(0, n_din, gw):
                    for w_t, w_d in ((wq_s, wqT_d), (wk_s, wkT_d), (wv_s, wvT_d)):
                        nc.sync.dma_start(
                            out=w_t[:, g0 : g0 + gw, :],
                            in_=w_d.rearrange("(c p) m -> p c m", p=128)[
                                :, g0 : g0 + gw, :
                            ],
                        )

                for si in range(n_s1):
                    s0 = si * SC1
                    xt = p1x.tile([128, n_din, SC1], f32r, tag="xt")
                    xsrc = xT_d[:, s0 : s0 + SC1].rearrange("(c p) s -> p c s", p=128)
                    nh = n_din // 2
                    nc.sync.dma_start(out=xt[:, :nh, :], in_=xsrc[:, :nh, :])
                    nc.sync.dma_start(out=xt[:, nh:, :], in_=xsrc[:, nh:, :])
                    cost = p1x.tile([128, SC1], f32, tag="cost")
                    nc.sync.dma_start(out=cost, in_=cosT_d[:, s0 : s0 + SC1])
                    sint = p1x.tile([128, SC1], f32, tag="sint")
                    nc.sync.dma_start(out=sint, in_=sinT_d[:, s0 : s0 + SC1])

                    for w_s, store in ((wq_s, qT), (wk_s, kT)):
                        for h in range(HPC):
                            acc = ps_qk.tile([128, SC1], f32, tag="qk")
                            for c in range(n_din):
                                nc.tensor.matmul(
                                    acc,
                                    lhsT=w_s[:, c, h * HD : (h + 1) * HD],
                                    rhs=xt[:, c, :],
                                    start=(c == 0),
                                    stop=(c == n_din - 1),
                                )
                            raw = p1t.tile([128, SC1], f32r, tag="raw")
                            nc.scalar.copy(raw, acc)
                            rot = ps_rot.tile([128, SC1], f32, tag="rot")
                            nc.tensor.matmul(
                                rot, lhsT=rTs, rhs=raw, start=True, stop=True
                            )
                            t1 = p1t.tile([128, SC1], f32, tag="t1")
                            nc.vector.tensor_mul(t1, raw.bitcast(f32), cost)
                            t2 = p1t.tile([128, SC1], f32, tag="t2")
                            nc.vector.tensor_mul(t2, rot, sint)
                            nc.vector.tensor_add(store[:, h, s0 : s0 + SC1], t1, t2)

                    for sub in range(SC1 // 128):
                        vacc = ps_v.tile([128, DLOC], f32, tag="v")
                        for c in range(n_din):
                            nc.tensor.matmul(
                                vacc,
                                lhsT=xt[:, c, sub * 128 : (sub + 1) * 128],
                                rhs=wv_s[:, c, :],
                                start=(c == 0),
                                stop=(c == n_din - 1),
                            )
                        nc.scalar.copy(vS[:, s0 // 128 + sub, :], vacc)

            # ------------- phases 2+3: attention, then output projection -------------
            # pools for both phases coexist so phase-3 groups (per batch) can
            # start while later batches' attention is still running
            with (
                tc.tile_pool(name="persistB", bufs=1) as persistB,
                tc.tile_pool(name="p2", bufs=4) as p2,
                tc.tile_pool(name="p2l", bufs=3) as p2l,
                tc.tile_pool(name="p2r", bufs=2) as p2r,
                tc.tile_pool(name="p3", bufs=2) as p3,
                tc.tile_pool(name="ps_st", bufs=2, space="PSUM") as ps_st,
                tc.tile_pool(name="ps_o", bufs=2, space="PSUM") as ps_o,
                tc.tile_pool(name="ps3", bufs=4, space="PSUM") as ps3,
            ):
                uT = persistB.tile([128, HPC, bs], f32r)  # attn out, [d, h, b*s]

                SCG = min(2, s // QCH)
                n_scg_b = s // (SCG * QCH)  # phase-3 groups per batch

                def phase3_groups(bi):
                    # outT[dout, s] = woT.T @ uT for batch bi's s-range;
                    # emitted right after bi's attention so the PE queue
                    # pipelines projection bursts with attention tails
                    for oc in range(dim // 128):
                        o0 = oc * 128
                        for gl in range(n_scg_b):
                            g = bi * n_scg_b + gl
                            pos = [
                                ps3.tile([128, QCH], f32, tag="op", name=f"po{_j}")
                                for _j in range(SCG)
                            ]
                            for h in range(HPC):
                                for j in range(SCG):
                                    s0 = (g * SCG + j) * QCH
                                    nc.tensor.matmul(
                                        pos[j],
                                        lhsT=woT_s[:, h, o0 : o0 + 128],
                                        rhs=uT[:, h, s0 : s0 + QCH],
                                        start=(h == 0),
                                        stop=(h == HPC - 1),
                                    )
                            ot = p3.tile([128, SCG, QCH], f32, tag="ot")
                            for j in range(SCG):
                                if j % 2 == 0:
                                    nc.scalar.copy(ot[:, j, :], pos[j])
                                else:
                                    nc.vector.tensor_copy(ot[:, j, :], pos[j])
                            nc.sync.dma_start(
                                out=out_d[
                                    o0 : o0 + 128, g * SCG * QCH : (g + 1) * SCG * QCH
                                ],
                                in_=ot,
                            )

                for bi in range(b):
                    for h in range(HPC):
                        for qc in range(n_qc):
                            q0 = bi * s + qc * QCH
                            nkc = (qc + 1) * QCH // 128
                            outp = ps_o.tile([128, QCH], f32, tag="o")
                            lrep = ps_o.tile([128, QCH], f32, tag="o", name="lrep")
                            prev_pt = None
                            li = 0
                            for kc in range(nkc):
                                k0 = bi * s + kc * 128
                                st = ps_st.tile([128, QCH], f32, tag="st")
                                nc.tensor.matmul(
                                    st,
                                    lhsT=kT[:, h, k0 : k0 + 128],
                                    rhs=qT[:, h, q0 : q0 + QCH],
                                    start=True,
                                    stop=True,
                                )
                                di = kc - (nkc - 4)
                                if di >= 0:
                                    # additive -1e9 causal mask on raw scores
                                    nc.vector.tensor_add(st, st, masks_s[:, di, :])
                                pt = p2.tile([128, QCH], f32r, tag="pt")
                                nc.scalar.activation(pt, st, EXP, scale=scale)
                                nc.tensor.matmul(
                                    outp,
                                    lhsT=vS[
                                        :, bi * (s // 128) + kc, h * HD : (h + 1) * HD
                                    ],
                                    rhs=pt,
                                    start=(kc == 0),
                                    stop=(kc == nkc - 1),
                                )
                                if kc % 2 == 1:
                                    # softmax denominator: independent pair-sums
                                    # (DVE/GpSimd alternating), partition-reduced
                                    # by an interleaved ones-matmul accumulation
                                    lp = p2l.tile([128, QCH], f32r, tag="lp")
                                    eng = nc.vector if li % 2 == 0 else nc.gpsimd
                                    eng.tensor_add(lp, prev_pt, pt)
                                    nc.tensor.matmul(
                                        lrep,
                                        lhsT=ones,
                                        rhs=lp,
                                        start=(li == 0),
                                        stop=(li == nkc // 2 - 1),
                                    )
                                    li += 1
                                prev_pt = pt
                            rec = p2r.tile([128, QCH], f32, tag="rec")
                            nc.vector.reciprocal_approx_fast(rec, lrep)
                            nc.vector.tensor_mul(uT[:, h, q0 : q0 + QCH], outp, rec)
                    phase3_groups(bi)

    nc.compile()
    return nc


def make_in_maps(x, Wq, Wk, Wv, Wo, b=B, s=S, dim=DIM, n_cores=N_CORES):
    bs = b * s
    xT = np.ascontiguousarray(x.reshape(bs, dim).T.astype(np.float32))
    cosT1, sinT1 = _rope_tables_T(s, HD)
    cosT = np.ascontiguousarray(np.tile(cosT1, (1, b)))
    sinT = np.ascontiguousarray(np.tile(sinT1, (1, b)))
    rT = _rot_matrix_T(HD)
    ones = np.ones((HD, HD), dtype=np.float32)
    import ml_dtypes
    masks = _causal_masks(QCH).astype(ml_dtypes.bfloat16)
    in_maps = []
    for c in range(n_cores):
        sl = slice(c * DLOC, (c + 1) * DLOC)
        in_maps.append(
            {
                "xT": xT,
                "wqT": np.ascontiguousarray(Wq[sl, :].T.astype(np.float32)),
                "wkT": np.ascontiguousarray(Wk[sl, :].T.astype(np.float32)),
                "wvT": np.ascontiguousarray(Wv[sl, :].T.astype(np.float32)),
                "woT": np.ascontiguousarray(Wo[:, sl].T.astype(np.float32)),
                "cosT": cosT,
                "sinT": sinT,
                "rT": rT,
                "ones": ones,
                "masks": masks,
            }
        )
    return in_maps


def kernel(x, Wq, Wk, Wv, Wo, _trace=False):
    """Full-input / full-output entry point. Shards over 8 cores internally."""
    if "/opt/trn_rl_repo" not in sys.path:
        sys.path.insert(0, "/opt/trn_rl_repo")
    from concourse.bass_utils import run_bass_kernel_spmd

    x = np.asarray(x, dtype=np.float32)
    Wq, Wk, Wv, Wo = (np.asarray(w, dtype=np.float32) for w in (Wq, Wk, Wv, Wo))

    key = (B, S, DIM)
    if key not in _PROGRAM_CACHE:
        _PROGRAM_CACHE[key] = build_program(B, S, DIM)
    nc = _PROGRAM_CACHE[key]

    in_maps = make_in_maps(x, Wq, Wk, Wv, Wo)
    res = run_bass_kernel_spmd(
        nc, in_maps, core_ids=list(range(N_CORES)), trace=_trace
    )
    kernel.last_results = res
    acc = res.results[0]["out"].astype(np.float32)
    for c in range(1, N_CORES):
        acc = acc + res.results[c]["out"]
    return np.ascontiguousarray(acc.T).reshape(B, S, DIM)



# revision 2
# speedup vs baseline: 1.0258x; 1.0258x over previous
"""Multi-head causal self-attention with RoPE, tensor-parallel over heads
across 8 Trainium2 NeuronCores.

Strategy (Megatron-style TP over heads):
  - Each core owns 2 of the 16 heads: rows [c*256,(c+1)*256) of Wq/Wk/Wv
    and the matching columns of Wo.
  - On-core: qT/kT projections in transposed [d, s] layout (natural matmul
    output layout), RoPE via a signed-permutation matmul + elementwise ops,
    v in natural [s, d] layout, causal attention with scores computed
    transposed (S^T = K Q^T, softmax sum via a ones-matmul, no running max
    needed -- scores are O(10) so exp() cannot overflow), then a partial
    output projection against the core's Wo column-slice, streamed per
    512-position q-chunk so output DMA overlaps attention.
  - Host sums the 8 partial outputs (this replaces the TP all-reduce).

dtypes: all DMA'd tensors (x, weights, q/k/v/u, attention probs, output
partials) are bf16 -- matmul rate on the PE is identical to fp32r (1
cycle/row at free dim >= 256) but DMA bytes and SBUF footprint halve.
Accumulation stays fp32 in PSUM; the RoPE elementwise path and softmax
normalization stay fp32. rel-err budget is 2e-2; bf16 rounding lands
~100x under that.
"""

import sys

import numpy as np

B, S, DIM = 2, 2048, 2048
NUM_HEADS = 16
HD = 128
N_CORES = 8
HPC = NUM_HEADS // N_CORES  # heads per core
DLOC = HPC * HD             # per-core slice of the model dim
ROPE_BASE = 10000.0
QCH = 512                   # attention q-chunk / phase-3 out-chunk
SC1 = 256                   # phase-1 s-chunk

_PROGRAM_CACHE = {}


def _rope_tables_T(seq_len, head_dim):
    # match reference float32 arithmetic: inv_freq over even indices,
    # emb = cat(freqs, freqs); returned transposed [head_dim, seq_len]
    inv_freq = (
        1.0
        / (np.float32(ROPE_BASE)
           ** (np.arange(0, head_dim, 2, dtype=np.float32) / np.float32(head_dim)))
    ).astype(np.float32)
    t = np.arange(seq_len, dtype=np.float32)
    freqs = np.outer(t, inv_freq).astype(np.float32)      # [S, D/2]
    emb = np.concatenate([freqs, freqs], axis=-1)         # [S, D]
    return (
        np.ascontiguousarray(np.cos(emb).astype(np.float32).T),
        np.ascontiguousarray(np.sin(emb).astype(np.float32).T),
    )


def _rot_matrix_T(head_dim):
    # rotated = cat(-x[1::2], x[::2]) = R @ x; return R.T [D, D]
    d2 = head_dim // 2
    R = np.zeros((head_dim, head_dim), dtype=np.float32)
    for dp in range(d2):
        R[dp, 2 * dp + 1] = -1.0
    for dp in range(d2, head_dim):
        R[dp, 2 * (dp - d2)] = 1.0
    return np.ascontiguousarray(R.T)


def _causal_masks(qch):
    # masks[i][kk, qq] = 0 if 128*i + kk <= qq else -1e9 (additive, applied
    # to raw scores before exp, for the 4 diagonal k-chunks of each q-chunk)
    m = np.zeros((4, 128, qch), dtype=np.float32)
    kk = np.arange(128)[:, None]
    qq = np.arange(qch)[None, :]
    for i in range(4):
        m[i] = np.where(128 * i + kk <= qq, 0.0, -1e9).astype(np.float32)
    return m


def build_program(b=B, s=S, dim=DIM):
    """Builds the per-core SPMD Bass program (identical on every core)."""
    if "/opt/trn_rl_repo" not in sys.path:
        sys.path.insert(0, "/opt/trn_rl_repo")
    import concourse.bacc as bacc
    import concourse.mybir as mybir
    import concourse.tile as tile

    f32 = mybir.dt.float32
    f32r = mybir.dt.float32r
    bf16 = mybir.dt.bfloat16
    EXP = mybir.ActivationFunctionType.Exp

    bs = b * s
    n_din = dim // 128          # contraction chunks for projections
    n_s1 = bs // SC1            # phase-1 s-chunks
    n_qc = s // QCH             # attention q-chunks per batch
    scale = float(HD) ** -0.5

    nc = bacc.Bacc("TRN2", target_bir_lowering=False, debug=False)

    xT_d = nc.dram_tensor("xT", [dim, bs], bf16, kind="ExternalInput")
    wqT_d = nc.dram_tensor("wqT", [dim, DLOC], bf16, kind="ExternalInput")
    wkT_d = nc.dram_tensor("wkT", [dim, DLOC], bf16, kind="ExternalInput")
    wvT_d = nc.dram_tensor("wvT", [dim, DLOC], bf16, kind="ExternalInput")
    woT_d = nc.dram_tensor("woT", [DLOC, dim], bf16, kind="ExternalInput")
    cosT_d = nc.dram_tensor("cosT", [HD, bs], f32, kind="ExternalInput")
    sinT_d = nc.dram_tensor("sinT", [HD, bs], f32, kind="ExternalInput")
    rT_d = nc.dram_tensor("rT", [HD, HD], f32r, kind="ExternalInput")
    ones_d = nc.dram_tensor("ones", [HD, HD], bf16, kind="ExternalInput")
    masks_d = nc.dram_tensor("masks", [4, HD, QCH], bf16, kind="ExternalInput")
    out_d = nc.dram_tensor("out", [dim, bs], bf16, kind="ExternalOutput")

    with tile.TileContext(nc) as tc:
        with tc.tile_pool(name="persist", bufs=1) as persist:
            # transposed roped projections [d, head, b*s]; v natural [s, chunk, d]
            qT = persist.tile([128, HPC, bs], bf16)
            kT = persist.tile([128, HPC, bs], bf16)
            vS = persist.tile([128, bs // 128, DLOC], bf16)
            rTs = persist.tile([HD, HD], f32r)
            ones = persist.tile([128, 128], bf16)
            masks_s = persist.tile([128, 4, QCH], bf16)
            woT_s = persist.tile([128, HPC, dim], bf16)

            # ---------------- phase 1: qkv projections + RoPE ----------------
            with (
                tc.tile_pool(name="p1w", bufs=1) as p1w,
                tc.tile_pool(name="p1x", bufs=2) as p1x,
                tc.tile_pool(name="p1t", bufs=2) as p1t,
                tc.tile_pool(name="ps_qk", bufs=4, space="PSUM") as ps_qk,
                tc.tile_pool(name="ps_rot", bufs=2, space="PSUM") as ps_rot,
                tc.tile_pool(name="ps_v", bufs=2, space="PSUM") as ps_v,
            ):
                wq_s = p1w.tile([128, n_din, DLOC], bf16)
                wk_s = p1w.tile([128, n_din, DLOC], bf16)
                wv_s = p1w.tile([128, n_din, DLOC], bf16)
                # first contraction group of each weight goes first so the
                # first matmuls can start immediately; the rest stream behind
                gw = max(1, n_din // 4)
                for g0 in range(0, n_din, gw):
                    for w_t, w_d in ((wq_s, wqT_d), (wk_s, wkT_d), (wv_s, wvT_d)):
                        nc.sync.dma_start(
                            out=w_t[:, g0 : g0 + gw, :],
                            in_=w_d.rearrange("(c p) m -> p c m", p=128)[
                                :, g0 : g0 + gw, :
                            ],
                        )
                    if g0 == 0:
                        # small constants next; attention-only tensors (masks,
                        # woT) go on the gpsimd DGE queue, off the load path
                        nc.sync.dma_start(out=rTs, in_=rT_d[:])
                        nc.sync.dma_start(out=ones, in_=ones_d[:])
                        nc.gpsimd.dma_start(
                            out=masks_s, in_=masks_d.rearrange("i p q -> p i q")
                        )
                        nc.gpsimd.dma_start(
                            out=woT_s, in_=woT_d.rearrange("(h p) n -> p h n", p=128)
                        )

                for si in range(n_s1):
                    s0 = si * SC1
                    xt = p1x.tile([128, n_din, SC1], bf16, tag="xt")
                    xsrc = xT_d[:, s0 : s0 + SC1].rearrange("(c p) s -> p c s", p=128)
                    nh = n_din // 2
                    # alternate DGE queues so descriptor dispatch pipelines
                    xq = nc.sync if si % 2 == 0 else nc.gpsimd
                    xq.dma_start(out=xt[:, :nh, :], in_=xsrc[:, :nh, :])
                    xq.dma_start(out=xt[:, nh:, :], in_=xsrc[:, nh:, :])
                    cost = p1x.tile([128, SC1], f32, tag="cost")
                    xq.dma_start(out=cost, in_=cosT_d[:, s0 : s0 + SC1])
                    sint = p1x.tile([128, SC1], f32, tag="sint")
                    xq.dma_start(out=sint, in_=sinT_d[:, s0 : s0 + SC1])

                    for w_s, store in ((wq_s, qT), (wk_s, kT)):
                        for h in range(HPC):
                            acc = ps_qk.tile([128, SC1], f32, tag="qk")
                            for c in range(n_din):
                                nc.tensor.matmul(
                                    acc,
                                    lhsT=w_s[:, c, h * HD : (h + 1) * HD],
                                    rhs=xt[:, c, :],
                                    start=(c == 0),
                                    stop=(c == n_din - 1),
                                )
                            raw = p1t.tile([128, SC1], f32r, tag="raw")
                            nc.scalar.copy(raw, acc)
                            rot = ps_rot.tile([128, SC1], f32, tag="rot")
                            nc.tensor.matmul(
                                rot, lhsT=rTs, rhs=raw, start=True, stop=True
                            )
                            t1 = p1t.tile([128, SC1], f32, tag="t1")
                            nc.vector.tensor_mul(t1, raw.bitcast(f32), cost)
                            t2 = p1t.tile([128, SC1], f32, tag="t2")
                            nc.vector.tensor_mul(t2, rot, sint)
                            nc.vector.tensor_add(store[:, h, s0 : s0 + SC1], t1, t2)

                    for sub in range(SC1 // 128):
                        vacc = ps_v.tile([128, DLOC], f32, tag="v")
                        for c in range(n_din):
                            nc.tensor.matmul(
                                vacc,
                                lhsT=xt[:, c, sub * 128 : (sub + 1) * 128],
                                rhs=wv_s[:, c, :],
                                start=(c == 0),
                                stop=(c == n_din - 1),
                            )
                        nc.scalar.copy(vS[:, s0 // 128 + sub, :], vacc)

            # ------------- phases 2+3: attention + streamed output projection -------------
            with (
                tc.tile_pool(name="persistB", bufs=1) as persistB,
                tc.tile_pool(name="p2", bufs=4) as p2,
                tc.tile_pool(name="p2l", bufs=3) as p2l,
                tc.tile_pool(name="p2r", bufs=2) as p2r,
                tc.tile_pool(name="p3", bufs=3) as p3,
                tc.tile_pool(name="ps_st", bufs=2, space="PSUM") as ps_st,
                tc.tile_pool(name="ps_o", bufs=3, space="PSUM") as ps_o,
                tc.tile_pool(name="ps3", bufs=2, space="PSUM") as ps3,
            ):
                uT = persistB.tile([128, HPC, bs], bf16)  # attn out, [d, h, b*s]

                for bi in range(b):
                    for qc in range(n_qc):
                        q0 = bi * s + qc * QCH
                        nkc = (qc + 1) * QCH // 128
                        for h in range(HPC):
                            outp = ps_o.tile([128, QCH], f32, tag="o")
                            lrep = ps_o.tile([128, QCH], f32, tag="o", name="lrep")
                            prev_pt = None
                            li = 0
                            for kc in range(nkc):
                                k0 = bi * s + kc * 128
                                st = ps_st.tile([128, QCH], f32, tag="st")
                                nc.tensor.matmul(
                                    st,
                                    lhsT=kT[:, h, k0 : k0 + 128],
                                    rhs=qT[:, h, q0 : q0 + QCH],
                                    start=True,
                                    stop=True,
                                )
                                di = kc - (nkc - 4)
                                if di >= 0:
                                    # additive -1e9 causal mask on raw scores
                                    nc.vector.tensor_add(st, st, masks_s[:, di, :])
                                pt = p2.tile([128, QCH], bf16, tag="pt")
                                nc.scalar.activation(pt, st, EXP, scale=scale)
                                nc.tensor.matmul(
                                    outp,
                                    lhsT=vS[
                                        :, bi * (s // 128) + kc, h * HD : (h + 1) * HD
                                    ],
                                    rhs=pt,
                                    start=(kc == 0),
                                    stop=(kc == nkc - 1),
                                )
                                if kc % 2 == 1:
                                    # softmax denominator: DVE pair-sums,
                                    # partition-reduced by an interleaved
                                    # ones-matmul accumulation
                                    lp = p2l.tile([128, QCH], bf16, tag="lp")
                                    nc.vector.tensor_add(lp, prev_pt, pt)
                                    nc.tensor.matmul(
                                        lrep,
                                        lhsT=ones,
                                        rhs=lp,
                                        start=(li == 0),
                                        stop=(li == nkc // 2 - 1),
                                    )
                                    li += 1
                                prev_pt = pt
                            rec = p2r.tile([128, QCH], f32, tag="rec")
                            nc.vector.reciprocal_approx_fast(rec, lrep)
                            nc.vector.tensor_mul(uT[:, h, q0 : q0 + QCH], outp, rec)

                        # phase 3 for this 512-position q-range: stream the
                        # partial output projection + DMA while the next
                        # q-chunk's attention runs
                        for op in range(dim // 256):
                            o0 = op * 256
                            ot = p3.tile([128, 2, QCH], bf16, tag="ot")
                            for j in range(2):
                                pos = ps3.tile([128, QCH], f32, tag="op")
                                for h in range(HPC):
                                    nc.tensor.matmul(
                                        pos,
                                        lhsT=woT_s[
                                            :, h, o0 + j * 128 : o0 + (j + 1) * 128
                                        ],
                                        rhs=uT[:, h, q0 : q0 + QCH],
                                        start=(h == 0),
                                        stop=(h == HPC - 1),
                                    )
                                if j == 0:
                                    nc.scalar.copy(ot[:, j, :], pos)
                                else:
                                    nc.vector.tensor_copy(ot[:, j, :], pos)
                            nc.gpsimd.dma_start(
                                out=out_d[o0 : o0 + 256, q0 : q0 + QCH].rearrange(
                                    "(c p) s -> p c s", p=128
                                ),
                                in_=ot,
                            )

    nc.compile()
    return nc


def make_in_maps(x, Wq, Wk, Wv, Wo, b=B, s=S, dim=DIM, n_cores=N_CORES):
    import ml_dtypes

    bf16 = ml_dtypes.bfloat16
    bs = b * s
    xT = np.ascontiguousarray(x.reshape(bs, dim).T).astype(bf16)
    cosT1, sinT1 = _rope_tables_T(s, HD)
    cosT = np.ascontiguousarray(np.tile(cosT1, (1, b)))
    sinT = np.ascontiguousarray(np.tile(sinT1, (1, b)))
    rT = _rot_matrix_T(HD)
    ones = np.ones((HD, HD), dtype=bf16)
    masks = _causal_masks(QCH).astype(bf16)
    in_maps = []
    for c in range(n_cores):
        sl = slice(c * DLOC, (c + 1) * DLOC)
        in_maps.append(
            {
                "xT": xT,
                "wqT": np.ascontiguousarray(Wq[sl, :].T).astype(bf16),
                "wkT": np.ascontiguousarray(Wk[sl, :].T).astype(bf16),
                "wvT": np.ascontiguousarray(Wv[sl, :].T).astype(bf16),
                "woT": np.ascontiguousarray(Wo[:, sl].T).astype(bf16),
                "cosT": cosT,
                "sinT": sinT,
                "rT": rT,
                "ones": ones,
                "masks": masks,
            }
        )
    return in_maps


def kernel(x, Wq, Wk, Wv, Wo, _trace=False):
    """Full-input / full-output entry point. Shards over 8 cores internally."""
    if "/opt/trn_rl_repo" not in sys.path:
        sys.path.insert(0, "/opt/trn_rl_repo")
    from concourse.bass_utils import run_bass_kernel_spmd

    x = np.asarray(x, dtype=np.float32)
    Wq, Wk, Wv, Wo = (np.asarray(w, dtype=np.float32) for w in (Wq, Wk, Wv, Wo))

    key = (B, S, DIM)
    if key not in _PROGRAM_CACHE:
        _PROGRAM_CACHE[key] = build_program(B, S, DIM)
    nc = _PROGRAM_CACHE[key]

    in_maps = make_in_maps(x, Wq, Wk, Wv, Wo)
    res = run_bass_kernel_spmd(
        nc, in_maps, core_ids=list(range(N_CORES)), trace=_trace
    )
    kernel.last_results = res
    acc = res.results[0]["out"].astype(np.float32)
    for c in range(1, N_CORES):
        acc = acc + res.results[c]["out"].astype(np.float32)
    return np.ascontiguousarray(acc.T).reshape(B, S, DIM)


# revision 4
# speedup vs baseline: 1.2189x; 1.1883x over previous
"""Multi-head causal self-attention with RoPE, tensor-parallel over heads
across 8 Trainium2 NeuronCores.

Strategy (Megatron-style TP over heads):
  - Each core owns 2 of the 16 heads: rows [c*256,(c+1)*256) of Wq/Wk/Wv
    and the matching columns of Wo.
  - On-core: qT/kT projections in transposed [d, s] layout (natural matmul
    output layout), RoPE via a signed-permutation matmul + elementwise ops,
    v in natural [s, d] layout, causal attention with scores computed
    transposed (S^T = K Q^T, softmax sum via a ones-matmul, no running max
    needed -- scores are O(10) so exp() cannot overflow), then a partial
    output projection against the core's Wo column-slice, streamed per
    512-position q-chunk so output DMA overlaps attention.
  - Host sums the 8 partial outputs (this replaces the TP all-reduce).

Schedule notes (all tuned against perfetto traces):
  - Phase 1 uses 512-wide s-chunks and per-head accumulation passes so
    every projection matmul has free dim 512 and its (bf16, FWL) weight
    load hides under the previous matmul.
  - RoPE rot-matmuls for a half-chunk are emitted one half-chunk later so
    the PE never waits on the scalar-engine PSUM->SBUF raw copy.
  - Attention scores are built in 2-k-chunk PSUM groups: one exp() call
    per 1024 columns (halves ACT fixed overhead), softmax denominator via
    GpSimd pair-sum + ones-matmul accumulation, output projection per
    q-chunk right after its two heads finish.
  - dtypes: everything DMA'd or used as a matmul operand is bf16 (PE rate
    is identical to fp32r; DMA/SBUF halve); PSUM accumulation and the
    softmax/RoPE elementwise paths stay fp32.
"""

import sys

import numpy as np

B, S, DIM = 2, 2048, 2048
NUM_HEADS = 16
HD = 128
N_CORES = 8
HPC = NUM_HEADS // N_CORES  # heads per core
DLOC = HPC * HD             # per-core slice of the model dim
ROPE_BASE = 10000.0
QCH = 512                   # attention q-chunk / phase-3 out-chunk
SC1 = 512                   # phase-1 s-chunk

_PROGRAM_CACHE = {}


def _rope_tables_T(seq_len, head_dim):
    # match reference float32 arithmetic: inv_freq over even indices,
    # emb = cat(freqs, freqs); returned transposed [head_dim, seq_len]
    inv_freq = (
        1.0
        / (np.float32(ROPE_BASE)
           ** (np.arange(0, head_dim, 2, dtype=np.float32) / np.float32(head_dim)))
    ).astype(np.float32)
    t = np.arange(seq_len, dtype=np.float32)
    freqs = np.outer(t, inv_freq).astype(np.float32)      # [S, D/2]
    emb = np.concatenate([freqs, freqs], axis=-1)         # [S, D]
    return (
        np.ascontiguousarray(np.cos(emb).astype(np.float32).T),
        np.ascontiguousarray(np.sin(emb).astype(np.float32).T),
    )


def _rot_matrix_T(head_dim):
    # rotated = cat(-x[1::2], x[::2]) = R @ x; return R.T [D, D]
    d2 = head_dim // 2
    R = np.zeros((head_dim, head_dim), dtype=np.float32)
    for dp in range(d2):
        R[dp, 2 * dp + 1] = -1.0
    for dp in range(d2, head_dim):
        R[dp, 2 * (dp - d2)] = 1.0
    return np.ascontiguousarray(R.T)


def _causal_masks(qch):
    # masks[i][kk, qq] = 0 if 128*i + kk <= qq else -1e9 (additive, applied
    # to raw scores before exp, for the 4 diagonal k-chunks of each q-chunk)
    m = np.zeros((4, 128, qch), dtype=np.float32)
    kk = np.arange(128)[:, None]
    qq = np.arange(qch)[None, :]
    for i in range(4):
        m[i] = np.where(128 * i + kk <= qq, 0.0, -1e9).astype(np.float32)
    return m


def build_program(b=B, s=S, dim=DIM):
    """Builds the per-core SPMD Bass program (identical on every core)."""
    if "/opt/trn_rl_repo" not in sys.path:
        sys.path.insert(0, "/opt/trn_rl_repo")
    import concourse.bacc as bacc
    import concourse.mybir as mybir
    import concourse.tile as tile

    f32 = mybir.dt.float32
    f32r = mybir.dt.float32r
    bf16 = mybir.dt.bfloat16
    EXP = mybir.ActivationFunctionType.Exp

    bs = b * s
    n_din = dim // 128          # contraction chunks for projections
    n_s1 = bs // SC1            # phase-1 s-chunks
    n_qc = s // QCH             # attention q-chunks per batch
    n_sub = SC1 // 128
    scale = float(HD) ** -0.5

    nc = bacc.Bacc("TRN2", target_bir_lowering=False, debug=False)

    xT_d = nc.dram_tensor("xT", [dim, bs], bf16, kind="ExternalInput")
    wqT_d = nc.dram_tensor("wqT", [dim, DLOC], bf16, kind="ExternalInput")
    wkT_d = nc.dram_tensor("wkT", [dim, DLOC], bf16, kind="ExternalInput")
    wvT_d = nc.dram_tensor("wvT", [dim, DLOC], bf16, kind="ExternalInput")
    woT_d = nc.dram_tensor("woT", [DLOC, dim], bf16, kind="ExternalInput")
    cosT_d = nc.dram_tensor("cosT", [HD, bs], f32, kind="ExternalInput")
    sinT_d = nc.dram_tensor("sinT", [HD, bs], f32, kind="ExternalInput")
    rT_d = nc.dram_tensor("rT", [HD, HD], f32r, kind="ExternalInput")
    ones_d = nc.dram_tensor("ones", [HD, HD], bf16, kind="ExternalInput")
    masks_d = nc.dram_tensor("masks", [4, HD, QCH], bf16, kind="ExternalInput")
    out_d = nc.dram_tensor("out", [dim, bs], bf16, kind="ExternalOutput")

    with tile.TileContext(nc) as tc:
        with tc.tile_pool(name="persist", bufs=1) as persist:
            # transposed roped projections [d, head, b*s]; v natural [s, chunk, d]
            qT = persist.tile([128, HPC, bs], bf16)
            kT = persist.tile([128, HPC, bs], bf16)
            vS = persist.tile([128, bs // 128, DLOC], bf16)
            rTs = persist.tile([HD, HD], f32r)
            ones = persist.tile([128, 128], bf16)
            masks_s = persist.tile([128, 4, QCH], bf16)
            woT_s = persist.tile([128, HPC, dim], bf16)

            # ---------------- phase 1: qkv projections + RoPE ----------------
            with (
                tc.tile_pool(name="p1w", bufs=1) as p1w,
                tc.tile_pool(name="p1x", bufs=2) as p1x,
                tc.tile_pool(name="p1t", bufs=2) as p1t,
                tc.tile_pool(name="ps_qk", bufs=2, space="PSUM") as ps_qk,
                tc.tile_pool(name="ps_rot", bufs=2, space="PSUM") as ps_rot,
                tc.tile_pool(name="ps_v", bufs=1, space="PSUM") as ps_v,
            ):
                wq_s = p1w.tile([128, n_din, DLOC], bf16)
                wk_s = p1w.tile([128, n_din, DLOC], bf16)
                wv_s = p1w.tile([128, n_din, DLOC], bf16)
                # first contraction group of each weight goes first so the
                # first matmuls start immediately; the rest stream behind;
                # attention-only tensors (masks, woT) ride the gpsimd queue
                gw = max(1, n_din // 4)
                for g0 in range(0, n_din, gw):
                    for w_t, w_d in ((wq_s, wqT_d), (wk_s, wkT_d), (wv_s, wvT_d)):
                        nc.sync.dma_start(
                            out=w_t[:, g0 : g0 + gw, :],
                            in_=w_d.rearrange("(c p) m -> p c m", p=128)[
                                :, g0 : g0 + gw, :
                            ],
                        )
                    if g0 == 0:
                        nc.sync.dma_start(out=rTs, in_=rT_d[:])
                        nc.sync.dma_start(out=ones, in_=ones_d[:])
                        nc.gpsimd.dma_start(
                            out=masks_s, in_=masks_d.rearrange("i p q -> p i q")
                        )
                        nc.gpsimd.dma_start(
                            out=woT_s, in_=woT_d.rearrange("(h p) n -> p h n", p=128)
                        )

                # RoPE for a finished half-chunk is emitted one half-chunk
                # later so the rot-matmul never stalls the PE on the scalar
                # engine's PSUM->SBUF copy of its input
                pend = []

                def emit_ropes():
                    while pend:
                        raw, cs, sn, dst = pend.pop(0)
                        rot = ps_rot.tile([128, SC1], f32, tag="rot")
                        nc.tensor.matmul(
                            rot, lhsT=rTs, rhs=raw, start=True, stop=True
                        )
                        t1 = p1t.tile([128, SC1], f32, tag="t1")
                        nc.vector.tensor_mul(t1, raw.bitcast(f32), cs)
                        t2 = p1t.tile([128, SC1], f32, tag="t2")
                        nc.vector.tensor_mul(t2, rot, sn)
                        nc.vector.tensor_add(dst, t1, t2)

                for si in range(n_s1):
                    s0 = si * SC1
                    xt = p1x.tile([128, n_din, SC1], bf16, tag="xt")
                    xsrc = xT_d[:, s0 : s0 + SC1].rearrange("(c p) s -> p c s", p=128)
                    nh = n_din // 2
                    xq = nc.sync if si % 2 == 0 else nc.gpsimd
                    xq.dma_start(out=xt[:, :nh, :], in_=xsrc[:, :nh, :])
                    xq.dma_start(out=xt[:, nh:, :], in_=xsrc[:, nh:, :])
                    cost = p1x.tile([128, SC1], f32, tag="cost")
                    xq.dma_start(out=cost, in_=cosT_d[:, s0 : s0 + SC1])
                    sint = p1x.tile([128, SC1], f32, tag="sint")
                    xq.dma_start(out=sint, in_=sinT_d[:, s0 : s0 + SC1])

                    for h in range(HPC):
                        qacc = ps_qk.tile([128, SC1], f32, tag="qa")
                        kacc = ps_qk.tile([128, SC1], f32, tag="ka")
                        if h == 0:
                            vacc = ps_v.tile([128, n_sub, DLOC], f32, tag="va")
                        for c in range(n_din):
                            nc.tensor.matmul(
                                qacc,
                                lhsT=wq_s[:, c, h * HD : (h + 1) * HD],
                                rhs=xt[:, c, :],
                                start=(c == 0),
                                stop=(c == n_din - 1),
                            )
                            nc.tensor.matmul(
                                kacc,
                                lhsT=wk_s[:, c, h * HD : (h + 1) * HD],
                                rhs=xt[:, c, :],
                                start=(c == 0),
                                stop=(c == n_din - 1),
                            )
                            if h == 0:
                                # v interleaved: its x-stationary weight loads
                                # hide under the wider q/k matmuls. Two subs
                                # share each 2KB PSUM bank, so the
                                # accumulation group (start clears the WHOLE
                                # bank's has_written bits) must open on the
                                # first sub of the bank and close on the last.
                                for sub in range(n_sub):
                                    nc.tensor.matmul(
                                        vacc[:, sub, :],
                                        lhsT=xt[:, c, sub * 128 : (sub + 1) * 128],
                                        rhs=wv_s[:, c, :],
                                        start=(c == 0 and sub % 2 == 0),
                                        stop=(c == n_din - 1 and sub % 2 == 1),
                                    )
                        emit_ropes()
                        rawq = p1t.tile([128, SC1], f32r, tag=f"rawq{h}")
                        nc.scalar.copy(rawq, qacc)
                        rawk = p1t.tile([128, SC1], f32r, tag=f"rawk{h}")
                        nc.scalar.copy(rawk, kacc)
                        if h == 0:
                            nc.scalar.copy(
                                vS[:, si * n_sub : (si + 1) * n_sub, :], vacc
                            )
                        pend.append((rawq, cost, sint, qT[:, h, s0 : s0 + SC1]))
                        pend.append((rawk, cost, sint, kT[:, h, s0 : s0 + SC1]))
                emit_ropes()

            # ------------- phases 2+3: attention + streamed output projection -------------
            with (
                tc.tile_pool(name="persistB", bufs=1) as persistB,
                tc.tile_pool(name="p2", bufs=4) as p2,
                tc.tile_pool(name="p2l", bufs=3) as p2l,
                tc.tile_pool(name="p2r", bufs=2) as p2r,
                tc.tile_pool(name="p3", bufs=3) as p3,
                tc.tile_pool(name="ps_st", bufs=2, space="PSUM") as ps_st,
                tc.tile_pool(name="ps_o", bufs=2, space="PSUM") as ps_o,
                tc.tile_pool(name="ps3", bufs=2, space="PSUM") as ps3,
            ):
                uT = persistB.tile([128, HPC, bs], bf16)  # attn out, [d, h, b*s]
                ncopy = 0

                for bi in range(b):
                    for qc in range(n_qc):
                        q0 = bi * s + qc * QCH
                        nkc = (qc + 1) * QCH // 128
                        ng = nkc // 2
                        for h in range(HPC):
                            outp = ps_o.tile([128, QCH], f32, tag="o")
                            lrep = ps_o.tile([128, QCH], f32, tag="o", name="lrep")
                            for gi in range(ng):
                                # scores for 2 k-chunks land in one 2-bank
                                # PSUM group -> a single exp() per 1024 cols
                                stg = ps_st.tile([128, 2, QCH], f32, tag="st")
                                for jj in range(2):
                                    kc = 2 * gi + jj
                                    k0 = bi * s + kc * 128
                                    nc.tensor.matmul(
                                        stg[:, jj, :],
                                        lhsT=kT[:, h, k0 : k0 + 128],
                                        rhs=qT[:, h, q0 : q0 + QCH],
                                        start=True,
                                        stop=True,
                                    )
                                if gi >= ng - 2:
                                    # additive -1e9 causal masks, one fused
                                    # add for both diagonal k-chunks
                                    mi = 2 * (gi - (ng - 2))
                                    nc.vector.tensor_add(
                                        stg, stg, masks_s[:, mi : mi + 2, :]
                                    )
                                pt = p2.tile([128, 2, QCH], bf16, tag="pt")
                                nc.scalar.activation(pt, stg, EXP, scale=scale)
                                for jj in range(2):
                                    kc = 2 * gi + jj
                                    nc.tensor.matmul(
                                        outp,
                                        lhsT=vS[
                                            :,
                                            bi * (s // 128) + kc,
                                            h * HD : (h + 1) * HD,
                                        ],
                                        rhs=pt[:, jj, :],
                                        start=(kc == 0),
                                        stop=(kc == nkc - 1),
                                    )
                                # softmax denominator: pair-sum on the (idle)
                                # gpsimd engine, partition-reduced by an
                                # interleaved ones-matmul accumulation
                                lp = p2l.tile([128, QCH], bf16, tag="lp")
                                nc.gpsimd.tensor_add(lp, pt[:, 0, :], pt[:, 1, :])
                                nc.tensor.matmul(
                                    lrep,
                                    lhsT=ones,
                                    rhs=lp,
                                    start=(gi == 0),
                                    stop=(gi == ng - 1),
                                )
                            rec = p2r.tile([128, QCH], f32, tag="rec")
                            nc.vector.reciprocal_approx_fast(rec, lrep)
                            nc.vector.tensor_mul(uT[:, h, q0 : q0 + QCH], outp, rec)

                        # phase 3 for this 512-position q-range: stream the
                        # partial output projection + DMA while the next
                        # q-chunk's attention runs
                        for op in range(dim // 256):
                            o0 = op * 256
                            ot = p3.tile([128, 2, QCH], bf16, tag="ot")
                            for j in range(2):
                                pos = ps3.tile([128, QCH], f32, tag="op")
                                for h in range(HPC):
                                    nc.tensor.matmul(
                                        pos,
                                        lhsT=woT_s[
                                            :, h, o0 + j * 128 : o0 + (j + 1) * 128
                                        ],
                                        rhs=uT[:, h, q0 : q0 + QCH],
                                        start=(h == 0),
                                        stop=(h == HPC - 1),
                                    )
                                if ncopy % 2 == 0:
                                    nc.scalar.copy(ot[:, j, :], pos)
                                else:
                                    nc.vector.tensor_copy(ot[:, j, :], pos)
                                ncopy += 1
                            nc.sync.dma_start(
                                out=out_d[o0 : o0 + 256, q0 : q0 + QCH].rearrange(
                                    "(c p) s -> p c s", p=128
                                ),
                                in_=ot,
                            )

    nc.compile()
    return nc


def make_in_maps(x, Wq, Wk, Wv, Wo, b=B, s=S, dim=DIM, n_cores=N_CORES):
    import ml_dtypes

    bf16 = ml_dtypes.bfloat16
    bs = b * s
    xT = np.ascontiguousarray(x.reshape(bs, dim).T).astype(bf16)
    cosT1, sinT1 = _rope_tables_T(s, HD)
    cosT = np.ascontiguousarray(np.tile(cosT1, (1, b)))
    sinT = np.ascontiguousarray(np.tile(sinT1, (1, b)))
    rT = _rot_matrix_T(HD)
    ones = np.ones((HD, HD), dtype=bf16)
    masks = _causal_masks(QCH).astype(bf16)
    in_maps = []
    for c in range(n_cores):
        sl = slice(c * DLOC, (c + 1) * DLOC)
        in_maps.append(
            {
                "xT": xT,
                "wqT": np.ascontiguousarray(Wq[sl, :].T).astype(bf16),
                "wkT": np.ascontiguousarray(Wk[sl, :].T).astype(bf16),
                "wvT": np.ascontiguousarray(Wv[sl, :].T).astype(bf16),
                "woT": np.ascontiguousarray(Wo[:, sl].T).astype(bf16),
                "cosT": cosT,
                "sinT": sinT,
                "rT": rT,
                "ones": ones,
                "masks": masks,
            }
        )
    return in_maps


def kernel(x, Wq, Wk, Wv, Wo, _trace=False):
    """Full-input / full-output entry point. Shards over 8 cores internally."""
    if "/opt/trn_rl_repo" not in sys.path:
        sys.path.insert(0, "/opt/trn_rl_repo")
    from concourse.bass_utils import run_bass_kernel_spmd

    x = np.asarray(x, dtype=np.float32)
    Wq, Wk, Wv, Wo = (np.asarray(w, dtype=np.float32) for w in (Wq, Wk, Wv, Wo))

    key = (B, S, DIM)
    if key not in _PROGRAM_CACHE:
        _PROGRAM_CACHE[key] = build_program(B, S, DIM)
    nc = _PROGRAM_CACHE[key]

    in_maps = make_in_maps(x, Wq, Wk, Wv, Wo)
    res = run_bass_kernel_spmd(
        nc, in_maps, core_ids=list(range(N_CORES)), trace=_trace
    )
    kernel.last_results = res
    acc = res.results[0]["out"].astype(np.float32)
    for c in range(1, N_CORES):
        acc = acc + res.results[c]["out"].astype(np.float32)
    return np.ascontiguousarray(acc.T).reshape(B, S, DIM)


# revision 7
# speedup vs baseline: 1.2214x; 1.0020x over previous
"""Multi-head causal self-attention with RoPE, tensor-parallel over heads
across 8 Trainium2 NeuronCores.

Strategy (Megatron-style TP over heads):
  - Each core owns 2 of the 16 heads: rows [c*256,(c+1)*256) of Wq/Wk/Wv
    and the matching columns of Wo.
  - On-core: qT/kT projections in transposed [d, s] layout (natural matmul
    output layout), RoPE via a signed-permutation matmul + elementwise ops,
    v in natural [s, d] layout, causal attention with scores computed
    transposed (S^T = K Q^T, softmax sum via a ones-matmul, no running max
    needed -- scores are O(10) so exp() cannot overflow), then a partial
    output projection against the core's Wo column-slice, streamed per
    512-position q-chunk so output DMA overlaps attention.
  - Host sums the 8 partial outputs (this replaces the TP all-reduce).

Schedule notes (all tuned against perfetto traces):
  - All DRAM tensors use host-pre-shuffled layouts so every DMA reads/writes
    long contiguous per-partition segments (the naive [dim, n] rearrange
    pattern shredded weight loads into 512B packets and cost ~30us of
    startup).
  - A burst of warm-up matmuls on the `ones` tile runs during the initial
    DMA wait so the PE's HAM clock-gate opens before real work arrives.
  - Phase 1 uses 512-wide s-chunks and per-head accumulation passes; each
    pass carries half the v sub-chunks so the PE stream is uniform. RoPE
    rot-matmuls for a half-chunk are emitted one half-chunk later so the PE
    never waits on the scalar-engine PSUM->SBUF raw copy.
  - Attention scores are built in 2-k-chunk PSUM groups: one exp() call per
    1024 columns (halves ACT fixed overhead). The softmax denominator is a
    per-k-chunk ones-matmul accumulation -- slightly more PE work than
    pair-summing first, but no cross-engine latency in the in-order PE
    queue. The per-q-chunk output projection is deferred into the next
    q-chunk's attention stream to cover the recip/divide latency.
  - dtypes: everything DMA'd or used as a matmul operand is bf16 (PE rate
    is identical to fp32r; DMA/SBUF halve); PSUM accumulation and the
    softmax/RoPE elementwise paths stay fp32.
"""

import sys

import numpy as np

B, S, DIM = 2, 2048, 2048
NUM_HEADS = 16
HD = 128
N_CORES = 8
HPC = NUM_HEADS // N_CORES  # heads per core
DLOC = HPC * HD             # per-core slice of the model dim
ROPE_BASE = 10000.0
QCH = 512                   # attention q-chunk / phase-3 out-chunk
SC1 = 512                   # phase-1 s-chunk
N_WARM = 30                 # PE warm-up matmuls during startup DMA

_PROGRAM_CACHE = {}


def _rope_tables_T(seq_len, head_dim):
    # match reference float32 arithmetic: inv_freq over even indices,
    # emb = cat(freqs, freqs); returned transposed [head_dim, seq_len]
    inv_freq = (
        1.0
        / (np.float32(ROPE_BASE)
           ** (np.arange(0, head_dim, 2, dtype=np.float32) / np.float32(head_dim)))
    ).astype(np.float32)
    t = np.arange(seq_len, dtype=np.float32)
    freqs = np.outer(t, inv_freq).astype(np.float32)      # [S, D/2]
    emb = np.concatenate([freqs, freqs], axis=-1)         # [S, D]
    return (
        np.ascontiguousarray(np.cos(emb).astype(np.float32).T),
        np.ascontiguousarray(np.sin(emb).astype(np.float32).T),
    )


def _rot_matrix_T(head_dim):
    # rotated = cat(-x[1::2], x[::2]) = R @ x; return R.T [D, D]
    d2 = head_dim // 2
    R = np.zeros((head_dim, head_dim), dtype=np.float32)
    for dp in range(d2):
        R[dp, 2 * dp + 1] = -1.0
    for dp in range(d2, head_dim):
        R[dp, 2 * (dp - d2)] = 1.0
    return np.ascontiguousarray(R.T)


def _causal_masks(qch):
    # masks[i][kk, qq] = 0 if 128*i + kk <= qq else -1e9 (additive, applied
    # to raw scores before exp, for the 4 diagonal k-chunks of each q-chunk)
    m = np.zeros((4, 128, qch), dtype=np.float32)
    kk = np.arange(128)[:, None]
    qq = np.arange(qch)[None, :]
    for i in range(4):
        m[i] = np.where(128 * i + kk <= qq, 0.0, -1e9).astype(np.float32)
    return m


def build_program(b=B, s=S, dim=DIM):
    """Builds the per-core SPMD Bass program (identical on every core)."""
    if "/opt/trn_rl_repo" not in sys.path:
        sys.path.insert(0, "/opt/trn_rl_repo")
    import concourse.bacc as bacc
    import concourse.mybir as mybir
    import concourse.tile as tile

    f32 = mybir.dt.float32
    f32r = mybir.dt.float32r
    bf16 = mybir.dt.bfloat16
    EXP = mybir.ActivationFunctionType.Exp

    bs = b * s
    n_din = dim // 128          # contraction chunks for projections
    n_s1 = bs // SC1            # phase-1 s-chunks
    n_qc = s // QCH             # attention q-chunks per batch
    n_sub = SC1 // 128
    n_og = dim // 256           # phase-3 256-row output groups
    scale = float(HD) ** -0.5

    nc = bacc.Bacc("TRN2", target_bir_lowering=False, debug=False)

    # host-pre-shuffled layouts: every DMA slice is contiguous per partition
    xP_d = nc.dram_tensor("xP", [128, n_s1, n_din, SC1], bf16, kind="ExternalInput")
    wA_d = nc.dram_tensor("wA", [128, n_din, 3, DLOC], bf16, kind="ExternalInput")
    woP_d = nc.dram_tensor("woP", [128, HPC, dim], bf16, kind="ExternalInput")
    cosT_d = nc.dram_tensor("cosT", [HD, bs], f32, kind="ExternalInput")
    sinT_d = nc.dram_tensor("sinT", [HD, bs], f32, kind="ExternalInput")
    rT_d = nc.dram_tensor("rT", [HD, HD], f32r, kind="ExternalInput")
    ones_d = nc.dram_tensor("ones", [HD, HD], bf16, kind="ExternalInput")
    masks_d = nc.dram_tensor("masks", [128, 4, QCH], bf16, kind="ExternalInput")
    outP_d = nc.dram_tensor(
        "outP", [128, n_og, b * n_qc, 2, QCH], bf16, kind="ExternalOutput"
    )

    with tile.TileContext(nc) as tc:
        with tc.tile_pool(name="persist", bufs=1) as persist:
            # transposed roped projections [d, head, b*s]; v natural [s, chunk, d]
            qT = persist.tile([128, HPC, bs], bf16)
            kT = persist.tile([128, HPC, bs], bf16)
            vS = persist.tile([128, bs // 128, DLOC], bf16)
            rTs = persist.tile([HD, HD], f32r)
            ones = persist.tile([128, 128], bf16)
            masks_s = persist.tile([128, 4, QCH], bf16)
            woT_s = persist.tile([128, HPC, dim], bf16)

            # ---------------- phase 1: qkv projections + RoPE ----------------
            with (
                tc.tile_pool(name="p1w", bufs=1) as p1w,
                tc.tile_pool(name="p1x", bufs=2) as p1x,
                tc.tile_pool(name="p1t", bufs=2) as p1t,
                tc.tile_pool(name="ps_qk", bufs=2, space="PSUM") as ps_qk,
                tc.tile_pool(name="ps_rot", bufs=2, space="PSUM") as ps_rot,
                tc.tile_pool(name="ps_v", bufs=2, space="PSUM") as ps_v,
            ):
                wA_s = p1w.tile([128, n_din, 3, DLOC], bf16)
                gw = max(1, n_din // 4)

                # startup order: ones (for PE warm-up) -> first weight group
                # -> first x chunk + rope tables -> remaining weight groups.
                # masks/woT (attention-only) ride the gpsimd DGE queue.
                nc.sync.dma_start(out=ones, in_=ones_d[:])
                nc.sync.dma_start(out=wA_s[:, 0:gw, :, :], in_=wA_d[:, 0:gw, :, :])
                nc.gpsimd.dma_start(out=masks_s, in_=masks_d[:])
                nc.gpsimd.dma_start(out=woT_s, in_=woP_d[:])

                def issue_x(si):
                    s0 = si * SC1
                    xt = p1x.tile([128, n_din, SC1], bf16, tag="xt")
                    xq = nc.sync if si % 2 == 0 else nc.gpsimd
                    nh = n_din // 2
                    xq.dma_start(out=xt[:, :nh, :], in_=xP_d[:, si, :nh, :])
                    xq.dma_start(out=xt[:, nh:, :], in_=xP_d[:, si, nh:, :])
                    cost = p1x.tile([128, SC1], f32, tag="cost")
                    xq.dma_start(out=cost, in_=cosT_d[:, s0 : s0 + SC1])
                    sint = p1x.tile([128, SC1], f32, tag="sint")
                    xq.dma_start(out=sint, in_=sinT_d[:, s0 : s0 + SC1])
                    return xt, cost, sint

                x_pre = issue_x(0)
                nc.sync.dma_start(out=rTs, in_=rT_d[:])
                for g0 in range(gw, n_din, gw):
                    nc.sync.dma_start(
                        out=wA_s[:, g0 : g0 + gw, :, :], in_=wA_d[:, g0 : g0 + gw, :, :]
                    )

                # PE warm-up: open the HAM clock gate during the DMA wait
                warm = ps_rot.tile([128, SC1], f32, tag="rot")
                for _ in range(N_WARM):
                    nc.tensor.matmul(
                        warm[:, :128], lhsT=ones, rhs=ones, start=True, stop=True
                    )

                # RoPE for a finished half-chunk is emitted one half-chunk
                # later so the rot-matmul never stalls the PE on the scalar
                # engine's PSUM->SBUF copy of its input
                pend = []

                def emit_ropes():
                    while pend:
                        raw, cs, sn, dst = pend.pop(0)
                        rot = ps_rot.tile([128, SC1], f32, tag="rot")
                        nc.tensor.matmul(
                            rot, lhsT=rTs, rhs=raw, start=True, stop=True
                        )
                        t1 = p1t.tile([128, SC1], f32, tag="t1")
                        nc.vector.tensor_mul(t1, raw.bitcast(f32), cs)
                        t2 = p1t.tile([128, SC1], f32, tag="t2")
                        nc.vector.tensor_mul(t2, rot, sn)
                        nc.vector.tensor_add(dst, t1, t2)

                for si in range(n_s1):
                    s0 = si * SC1
                    xt, cost, sint = x_pre if si == 0 else issue_x(si)
                    for h in range(HPC):
                        qacc = ps_qk.tile([128, SC1], f32, tag="qa")
                        kacc = ps_qk.tile([128, SC1], f32, tag="ka")
                        # each h-pass carries half the v sub-chunks; two subs
                        # share this 2KB PSUM bank so the accumulation group
                        # (start clears the whole bank's has_written bits)
                        # opens on the first sub and closes on the last
                        vacc = ps_v.tile([128, 2, DLOC], f32, tag="va")
                        for c in range(n_din):
                            nc.tensor.matmul(
                                qacc,
                                lhsT=wA_s[:, c, 0, h * HD : (h + 1) * HD],
                                rhs=xt[:, c, :],
                                start=(c == 0),
                                stop=(c == n_din - 1),
                            )
                            nc.tensor.matmul(
                                kacc,
                                lhsT=wA_s[:, c, 1, h * HD : (h + 1) * HD],
                                rhs=xt[:, c, :],
                                start=(c == 0),
                                stop=(c == n_din - 1),
                            )
                            for jsub in range(2):
                                sub = 2 * h + jsub
                                nc.tensor.matmul(
                                    vacc[:, jsub, :],
                                    lhsT=xt[:, c, sub * 128 : (sub + 1) * 128],
                                    rhs=wA_s[:, c, 2, :],
                                    start=(c == 0 and jsub == 0),
                                    stop=(c == n_din - 1 and jsub == 1),
                                )
                        emit_ropes()
                        rawq = p1t.tile([128, SC1], f32r, tag=f"rawq{h}")
                        nc.scalar.copy(rawq, qacc)
                        rawk = p1t.tile([128, SC1], f32r, tag=f"rawk{h}")
                        nc.scalar.copy(rawk, kacc)
                        nc.scalar.copy(
                            vS[:, si * n_sub + 2 * h : si * n_sub + 2 * h + 2, :],
                            vacc,
                        )
                        pend.append((rawq, cost, sint, qT[:, h, s0 : s0 + SC1]))
                        pend.append((rawk, cost, sint, kT[:, h, s0 : s0 + SC1]))
                emit_ropes()

            # ------------- phases 2+3: attention + streamed output projection -------------
            with (
                tc.tile_pool(name="persistB", bufs=1) as persistB,
                tc.tile_pool(name="p2", bufs=4) as p2,
                tc.tile_pool(name="p2r", bufs=2) as p2r,
                tc.tile_pool(name="p3", bufs=3) as p3,
                tc.tile_pool(name="ps_st", bufs=2, space="PSUM") as ps_st,
                tc.tile_pool(name="ps_o", bufs=2, space="PSUM") as ps_o,
                tc.tile_pool(name="ps3", bufs=2, space="PSUM") as ps3,
            ):
                uT = persistB.tile([128, HPC, bs], bf16)  # attn out, [d, h, b*s]
                ncopy = 0
                pend_p3 = [None]

                def make_p3(q0, qg):
                    def emit():
                        nonlocal ncopy
                        for og in range(n_og):
                            o0 = og * 256
                            ot = p3.tile([128, 2, QCH], bf16, tag="ot")
                            for j in range(2):
                                pos = ps3.tile([128, QCH], f32, tag="op")
                                for h in range(HPC):
                                    nc.tensor.matmul(
                                        pos,
                                        lhsT=woT_s[
                                            :, h, o0 + j * 128 : o0 + (j + 1) * 128
                                        ],
                                        rhs=uT[:, h, q0 : q0 + QCH],
                                        start=(h == 0),
                                        stop=(h == HPC - 1),
                                    )
                                if ncopy % 2 == 0:
                                    nc.scalar.copy(ot[:, j, :], pos)
                                else:
                                    nc.vector.tensor_copy(ot[:, j, :], pos)
                                ncopy += 1
                            nc.sync.dma_start(out=outP_d[:, og, qg, :, :], in_=ot)

                    return emit

                for bi in range(b):
                    for qc in range(n_qc):
                        q0 = bi * s + qc * QCH
                        qg = bi * n_qc + qc
                        nkc = (qc + 1) * QCH // 128
                        ng = nkc // 2
                        for h in range(HPC):
                            outp = ps_o.tile([128, QCH], f32, tag="o")
                            lrep = ps_o.tile([128, QCH], f32, tag="o", name="lrep")
                            for gi in range(ng):
                                # scores for 2 k-chunks land in one 2-bank
                                # PSUM group -> a single exp() per 1024 cols
                                stg = ps_st.tile([128, 2, QCH], f32, tag="st")
                                for jj in range(2):
                                    kc = 2 * gi + jj
                                    k0 = bi * s + kc * 128
                                    nc.tensor.matmul(
                                        stg[:, jj, :],
                                        lhsT=kT[:, h, k0 : k0 + 128],
                                        rhs=qT[:, h, q0 : q0 + QCH],
                                        start=True,
                                        stop=True,
                                    )
                                if gi >= ng - 2:
                                    # additive -1e9 causal masks, one fused
                                    # add for both diagonal k-chunks
                                    mi = 2 * (gi - (ng - 2))
                                    nc.vector.tensor_add(
                                        stg, stg, masks_s[:, mi : mi + 2, :]
                                    )
                                pt = p2.tile([128, 2, QCH], bf16, tag="pt")
                                nc.scalar.activation(pt, stg, EXP, scale=scale)
                                for jj in range(2):
                                    kc = 2 * gi + jj
                                    nc.tensor.matmul(
                                        outp,
                                        lhsT=vS[
                                            :,
                                            bi * (s // 128) + kc,
                                            h * HD : (h + 1) * HD,
                                        ],
                                        rhs=pt[:, jj, :],
                                        start=(kc == 0),
                                        stop=(kc == nkc - 1),
                                    )
                                    # softmax denominator rides the PE: no
                                    # cross-engine latency in the in-order
                                    # matmul queue
                                    nc.tensor.matmul(
                                        lrep,
                                        lhsT=ones,
                                        rhs=pt[:, jj, :],
                                        start=(kc == 0),
                                        stop=(kc == nkc - 1),
                                    )
                                if h == 0 and gi == 0 and pend_p3[0] is not None:
                                    # previous q-chunk's output projection,
                                    # deferred here so its uT divide latency
                                    # is covered by this group's matmuls
                                    pend_p3[0]()
                                    pend_p3[0] = None
                            rec = p2r.tile([128, QCH], f32, tag="rec")
                            nc.vector.reciprocal_approx_fast(rec, lrep)
                            nc.vector.tensor_mul(uT[:, h, q0 : q0 + QCH], outp, rec)
                        pend_p3[0] = make_p3(q0, qg)
                pend_p3[0]()

    nc.compile()
    return nc


def make_in_maps(x, Wq, Wk, Wv, Wo, b=B, s=S, dim=DIM, n_cores=N_CORES):
    import ml_dtypes

    bf16 = ml_dtypes.bfloat16
    bs = b * s
    n_din = dim // 128
    n_s1 = bs // SC1
    # x pre-shuffled so each [128, c, s-chunk] tile DMA is one contiguous
    # per-partition segment: xP[p, si, c, s'] = x[si*SC1+s', c*128+p]
    xP = np.ascontiguousarray(
        x.reshape(bs, dim).reshape(n_s1, SC1, n_din, 128).transpose(3, 0, 2, 1)
    ).astype(bf16)
    cosT1, sinT1 = _rope_tables_T(s, HD)
    cosT = np.ascontiguousarray(np.tile(cosT1, (1, b)))
    sinT = np.ascontiguousarray(np.tile(sinT1, (1, b)))
    rT = _rot_matrix_T(HD)
    ones = np.ones((HD, HD), dtype=bf16)
    masks = np.ascontiguousarray(_causal_masks(QCH).transpose(1, 0, 2)).astype(bf16)
    in_maps = []
    for c in range(n_cores):
        sl = slice(c * DLOC, (c + 1) * DLOC)
        # packed q/k/v weights: wA[p, c, iw, m] = W_iw.T[c*128+p, m]
        wA = np.ascontiguousarray(
            np.stack([Wq[sl].T, Wk[sl].T, Wv[sl].T], axis=0)
            .reshape(3, n_din, 128, DLOC)
            .transpose(2, 1, 0, 3)
        ).astype(bf16)
        woP = np.ascontiguousarray(
            Wo[:, sl].T.reshape(HPC, 128, dim).transpose(1, 0, 2)
        ).astype(bf16)
        in_maps.append(
            {
                "xP": xP,
                "wA": wA,
                "woP": woP,
                "cosT": cosT,
                "sinT": sinT,
                "rT": rT,
                "ones": ones,
                "masks": masks,
            }
        )
    return in_maps


def kernel(x, Wq, Wk, Wv, Wo, _trace=False):
    """Full-input / full-output entry point. Shards over 8 cores internally."""
    if "/opt/trn_rl_repo" not in sys.path:
        sys.path.insert(0, "/opt/trn_rl_repo")
    from concourse.bass_utils import run_bass_kernel_spmd

    x = np.asarray(x, dtype=np.float32)
    Wq, Wk, Wv, Wo = (np.asarray(w, dtype=np.float32) for w in (Wq, Wk, Wv, Wo))

    key = (B, S, DIM)
    if key not in _PROGRAM_CACHE:
        _PROGRAM_CACHE[key] = build_program(B, S, DIM)
    nc = _PROGRAM_CACHE[key]

    in_maps = make_in_maps(x, Wq, Wk, Wv, Wo)
    res = run_bass_kernel_spmd(
        nc, in_maps, core_ids=list(range(N_CORES)), trace=_trace
    )
    kernel.last_results = res

    n_qc = S // QCH
    n_og = DIM // 256
    acc = None
    for c in range(N_CORES):
        # outP[p, og, qg, j, s'] = out[og*256 + j*128 + p, qg*QCH + s']
        o = res.results[c]["outP"].astype(np.float32)
        o = o.transpose(1, 3, 0, 2, 4).reshape(DIM, B * S)
        acc = o if acc is None else acc + o
    return np.ascontiguousarray(acc.T).reshape(B, S, DIM)


# revision 13
# speedup vs baseline: 1.3224x; 1.0827x over previous
"""Multi-head causal self-attention with RoPE, tensor-parallel over heads
across 8 Trainium2 NeuronCores.

Strategy (Megatron-style TP over heads):
  - Each core owns 2 of the 16 heads: rows [c*256,(c+1)*256) of Wq/Wk/Wv
    and the matching columns of Wo.
  - On-core: qT/kT projections in transposed [d, s] layout (natural matmul
    output layout), RoPE via a signed-permutation matmul + elementwise ops,
    v in natural [s, d] layout, causal attention with scores computed
    transposed (S^T = K Q^T, softmax sum via a ones-matmul, no running max
    needed -- scores are O(10) so exp() cannot overflow), then a partial
    output projection against the core's Wo column-slice, streamed per
    512-position q-chunk so output DMA overlaps attention.
  - Host sums the 8 partial outputs (this replaces the TP all-reduce).

Schedule notes (all tuned against perfetto traces):
  - All DRAM tensors use host-pre-shuffled layouts so every DMA moves long
    contiguous per-partition segments (naive rearrange patterns shredded
    weight loads into 512B packets and cost ~30us of startup).
  - Warm-up matmuls on the `ones` tile run during the initial DMA wait so
    the PE's HAM clock-gate opens before real work arrives.
  - q/k/v results live in per-512-chunk tiles, not monolithic tensors, so
    phase 2's first score matmuls do not serialize against the LAST RoPE
    writes (tile-granular dependency tracking); the PE flows from phase 1
    into attention without going idle (and without a HAM re-throttle).
  - Attention is k-chunk-granular: score matmul -> (mask) -> exp, with the
    p@v and denominator ones-matmuls DEFERRED three k-chunks behind via a
    job queue that also drains across h/q-chunk boundaries. This keeps the
    in-order PE queue from ever waiting on the scalar engine's exp.
  - The per-q-chunk output projection is split into 8 one-OG bursts popped
    one per k-chunk of the NEXT q-chunk, so its PSUM->SBUF copies never
    flood the ACT/DVE queues ahead of exp/mask work.
  - dtypes: everything DMA'd or used as a matmul operand is bf16 (PE rate
    is identical to fp32r; DMA/SBUF halve); PSUM accumulation and the
    softmax/RoPE elementwise paths stay fp32.
"""

import sys
from collections import deque

import numpy as np

B, S, DIM = 2, 2048, 2048
NUM_HEADS = 16
HD = 128
N_CORES = 8
HPC = NUM_HEADS // N_CORES  # heads per core
DLOC = HPC * HD             # per-core slice of the model dim
ROPE_BASE = 10000.0
QCH = 512                   # attention q-chunk / phase-3 out-chunk
SC1 = 512                   # phase-1 s-chunk
N_WARM = 55                 # PE warm-up matmuls during startup DMA
DEFER = 3                   # k-chunks of pv/ones deferral behind exp

_PROGRAM_CACHE = {}


def _rope_tables_T(seq_len, head_dim):
    # match reference float32 arithmetic: inv_freq over even indices,
    # emb = cat(freqs, freqs); returned transposed [head_dim, seq_len]
    inv_freq = (
        1.0
        / (np.float32(ROPE_BASE)
           ** (np.arange(0, head_dim, 2, dtype=np.float32) / np.float32(head_dim)))
    ).astype(np.float32)
    t = np.arange(seq_len, dtype=np.float32)
    freqs = np.outer(t, inv_freq).astype(np.float32)      # [S, D/2]
    emb = np.concatenate([freqs, freqs], axis=-1)         # [S, D]
    return (
        np.ascontiguousarray(np.cos(emb).astype(np.float32).T),
        np.ascontiguousarray(np.sin(emb).astype(np.float32).T),
    )


def _rot_matrix_T(head_dim):
    # rotated = cat(-x[1::2], x[::2]) = R @ x; return R.T [D, D]
    d2 = head_dim // 2
    R = np.zeros((head_dim, head_dim), dtype=np.float32)
    for dp in range(d2):
        R[dp, 2 * dp + 1] = -1.0
    for dp in range(d2, head_dim):
        R[dp, 2 * (dp - d2)] = 1.0
    return np.ascontiguousarray(R.T)


def _causal_masks(qch):
    # masks[i][kk, qq] = 0 if 128*i + kk <= qq else -1e9 (additive, applied
    # to raw scores before exp, for the 4 diagonal k-chunks of each q-chunk)
    m = np.zeros((4, 128, qch), dtype=np.float32)
    kk = np.arange(128)[:, None]
    qq = np.arange(qch)[None, :]
    for i in range(4):
        m[i] = np.where(128 * i + kk <= qq, 0.0, -1e9).astype(np.float32)
    return m


def build_program(b=B, s=S, dim=DIM):
    """Builds the per-core SPMD Bass program (identical on every core)."""
    if "/opt/trn_rl_repo" not in sys.path:
        sys.path.insert(0, "/opt/trn_rl_repo")
    import concourse.bacc as bacc
    import concourse.mybir as mybir
    import concourse.tile as tile

    f32 = mybir.dt.float32
    f32r = mybir.dt.float32r
    bf16 = mybir.dt.bfloat16
    EXP = mybir.ActivationFunctionType.Exp

    bs = b * s
    n_din = dim // 128          # contraction chunks for projections
    n_s1 = bs // SC1            # phase-1 s-chunks
    n_qc = s // QCH             # attention q-chunks per batch
    n_sub = SC1 // 128
    n_og = dim // 256           # phase-3 256-row output groups
    scale = float(HD) ** -0.5

    nc = bacc.Bacc("TRN2", target_bir_lowering=False, debug=False)

    # host-pre-shuffled layouts: every DMA slice is contiguous per partition
    xP_d = nc.dram_tensor("xP", [128, n_s1, n_din, SC1], bf16, kind="ExternalInput")
    wA_d = nc.dram_tensor("wA", [128, n_din, 3, DLOC], bf16, kind="ExternalInput")
    woP_d = nc.dram_tensor("woP", [128, HPC, dim], bf16, kind="ExternalInput")
    cosT_d = nc.dram_tensor("cosT", [HD, bs], f32, kind="ExternalInput")
    sinT_d = nc.dram_tensor("sinT", [HD, bs], f32, kind="ExternalInput")
    rT_d = nc.dram_tensor("rT", [HD, HD], f32r, kind="ExternalInput")
    ones_d = nc.dram_tensor("ones", [HD, HD], bf16, kind="ExternalInput")
    masks_d = nc.dram_tensor("masks", [128, 4, QCH], bf16, kind="ExternalInput")
    outP_d = nc.dram_tensor(
        "outP", [128, n_og, b * n_qc, 2, QCH], bf16, kind="ExternalOutput"
    )

    with tile.TileContext(nc) as tc:
        with tc.tile_pool(name="persist", bufs=1) as persist:
            # per-512-chunk projection tiles (fine-grained deps; see header)
            qTs = [
                persist.tile([128, HPC, SC1], bf16, name=f"qT{i}")
                for i in range(n_s1)
            ]
            kTs = [
                persist.tile([128, HPC, SC1], bf16, name=f"kT{i}")
                for i in range(n_s1)
            ]
            vSs = [
                persist.tile([128, n_sub, DLOC], bf16, name=f"vS{i}")
                for i in range(n_s1)
            ]
            rTs = persist.tile([HD, HD], f32r)
            ones = persist.tile([128, 128], bf16)
            masks_s = persist.tile([128, 4, QCH], bf16)
            woT_s = persist.tile([128, HPC, dim], bf16)

            # ---------------- phase 1: qkv projections + RoPE ----------------
            with (
                tc.tile_pool(name="p1w", bufs=1) as p1w,
                tc.tile_pool(name="p1x", bufs=2) as p1x,
                tc.tile_pool(name="p1t", bufs=2) as p1t,
                tc.tile_pool(name="ps_qk", bufs=2, space="PSUM") as ps_qk,
                tc.tile_pool(name="ps_rot", bufs=2, space="PSUM") as ps_rot,
                tc.tile_pool(name="ps_v", bufs=2, space="PSUM") as ps_v,
            ):
                wA_s = p1w.tile([128, n_din, 3, DLOC], bf16)
                gw = max(1, n_din // 4)

                # startup order: ones (for warm-up) -> first weight group ->
                # first x chunk in quarters -> remaining weights -> tables.
                # masks/woT (attention-only) ride the gpsimd queue at si==1.
                nc.sync.dma_start(out=ones, in_=ones_d[:])
                nc.sync.dma_start(out=wA_s[:, 0:gw, :, :], in_=wA_d[:, 0:gw, :, :])
                xt0 = p1x.tile([128, n_din, SC1], bf16, tag="xt")
                for qq in range(4):
                    nc.sync.dma_start(
                        out=xt0[:, qq * 4 : (qq + 1) * 4, :],
                        in_=xP_d[:, 0, qq * 4 : (qq + 1) * 4, :],
                    )
                for g0 in range(gw, n_din, gw):
                    nc.sync.dma_start(
                        out=wA_s[:, g0 : g0 + gw, :, :], in_=wA_d[:, g0 : g0 + gw, :, :]
                    )
                nc.sync.dma_start(out=rTs, in_=rT_d[:])
                cost0 = p1x.tile([128, SC1], f32, tag="cost")
                nc.sync.dma_start(out=cost0, in_=cosT_d[:, 0:SC1])
                sint0 = p1x.tile([128, SC1], f32, tag="sint")
                nc.sync.dma_start(out=sint0, in_=sinT_d[:, 0:SC1])

                # PE warm-up: open the HAM clock gate during the DMA wait
                warm = ps_rot.tile([128, SC1], f32, tag="rot")
                for _ in range(N_WARM):
                    nc.tensor.matmul(
                        warm[:, :128], lhsT=ones, rhs=ones, start=True, stop=True
                    )

                def issue_x(si):
                    s0 = si * SC1
                    xt = p1x.tile([128, n_din, SC1], bf16, tag="xt")
                    xq = nc.sync if si % 2 == 0 else nc.gpsimd
                    nh = n_din // 2
                    xq.dma_start(out=xt[:, :nh, :], in_=xP_d[:, si, :nh, :])
                    xq.dma_start(out=xt[:, nh:, :], in_=xP_d[:, si, nh:, :])
                    cost = p1x.tile([128, SC1], f32, tag="cost")
                    xq.dma_start(out=cost, in_=cosT_d[:, s0 : s0 + SC1])
                    sint = p1x.tile([128, SC1], f32, tag="sint")
                    xq.dma_start(out=sint, in_=sinT_d[:, s0 : s0 + SC1])
                    if si == 1:
                        nc.gpsimd.dma_start(out=masks_s, in_=masks_d[:])
                        nc.gpsimd.dma_start(out=woT_s, in_=woP_d[:])
                    return xt, cost, sint

                # RoPE for a finished half-chunk is emitted one half-chunk
                # later so the rot-matmul never stalls the PE on the scalar
                # engine's PSUM->SBUF copy of its input
                pend = []

                def emit_ropes():
                    while pend:
                        raw, cs, sn, dst = pend.pop(0)
                        rot = ps_rot.tile([128, SC1], f32, tag="rot")
                        nc.tensor.matmul(
                            rot, lhsT=rTs, rhs=raw, start=True, stop=True
                        )
                        t1 = p1t.tile([128, SC1], f32, tag="t1")
                        nc.vector.tensor_mul(t1, raw.bitcast(f32), cs)
                        t2 = p1t.tile([128, SC1], f32, tag="t2")
                        nc.vector.tensor_mul(t2, rot, sn)
                        nc.vector.tensor_add(dst, t1, t2)

                for si in range(n_s1):
                    xt, cost, sint = (
                        (xt0, cost0, sint0) if si == 0 else issue_x(si)
                    )
                    for h in range(HPC):
                        qacc = ps_qk.tile([128, SC1], f32, tag="qa")
                        kacc = ps_qk.tile([128, SC1], f32, tag="ka")
                        # each h-pass carries half the v sub-chunks; two subs
                        # share this 2KB PSUM bank so the accumulation group
                        # (start clears the whole bank's has_written bits)
                        # opens on the first sub and closes on the last
                        vacc = ps_v.tile([128, 2, DLOC], f32, tag="va")
                        for c in range(n_din):
                            nc.tensor.matmul(
                                qacc,
                                lhsT=wA_s[:, c, 0, h * HD : (h + 1) * HD],
                                rhs=xt[:, c, :],
                                start=(c == 0),
                                stop=(c == n_din - 1),
                            )
                            nc.tensor.matmul(
                                kacc,
                                lhsT=wA_s[:, c, 1, h * HD : (h + 1) * HD],
                                rhs=xt[:, c, :],
                                start=(c == 0),
                                stop=(c == n_din - 1),
                            )
                            for jsub in range(2):
                                sub = 2 * h + jsub
                                nc.tensor.matmul(
                                    vacc[:, jsub, :],
                                    lhsT=xt[:, c, sub * 128 : (sub + 1) * 128],
                                    rhs=wA_s[:, c, 2, :],
                                    start=(c == 0 and jsub == 0),
                                    stop=(c == n_din - 1 and jsub == 1),
                                )
                        emit_ropes()
                        rawq = p1t.tile([128, SC1], f32r, tag=f"rawq{h}")
                        nc.scalar.copy(rawq, qacc)
                        rawk = p1t.tile([128, SC1], f32r, tag=f"rawk{h}")
                        nc.scalar.copy(rawk, kacc)
                        nc.scalar.copy(vSs[si][:, 2 * h : 2 * h + 2, :], vacc)
                        pend.append((rawq, cost, sint, qTs[si][:, h, :]))
                        pend.append((rawk, cost, sint, kTs[si][:, h, :]))
                emit_ropes()

            # ------------- phases 2+3: attention + streamed output projection -------------
            with (
                tc.tile_pool(name="persistB", bufs=1) as persistB,
                tc.tile_pool(name="p2", bufs=6) as p2,
                tc.tile_pool(name="p2r", bufs=2) as p2r,
                tc.tile_pool(name="p3", bufs=3) as p3,
                tc.tile_pool(name="ps_st", bufs=3, space="PSUM") as ps_st,
                tc.tile_pool(name="ps_o", bufs=3, space="PSUM") as ps_o,
                tc.tile_pool(name="ps3", bufs=2, space="PSUM") as ps3,
            ):
                uT = persistB.tile([128, HPC, bs], bf16)  # attn out, [d, h, b*s]
                jobs = deque()          # deferred pv/ones emissions
                p3q = deque()           # deferred output-projection OG bursts

                def drain(keep):
                    while len(jobs) > keep:
                        jobs.popleft()()

                def make_job(hctx, kc, nkc, pt, bi, h, q0):
                    def emit():
                        if "outp" not in hctx:
                            # lrep first: with bufs=3 the next h's outp then
                            # lands on this h's lrep slot, which frees at
                            # recip() -- earlier than outp's uT-divide
                            hctx["lrep"] = ps_o.tile(
                                [128, QCH], f32, tag="o", name="lrep"
                            )
                            hctx["outp"] = ps_o.tile(
                                [128, QCH], f32, tag="o", name="outp"
                            )
                        outp, lrep = hctx["outp"], hctx["lrep"]
                        gk = bi * (s // 128) + kc
                        nc.tensor.matmul(
                            outp,
                            lhsT=vSs[gk // n_sub][
                                :, gk % n_sub, h * HD : (h + 1) * HD
                            ],
                            rhs=pt,
                            start=(kc == 0),
                            stop=(kc == nkc - 1),
                        )
                        nc.tensor.matmul(
                            lrep,
                            lhsT=ones,
                            rhs=pt,
                            start=(kc == 0),
                            stop=(kc == nkc - 1),
                        )
                        if kc == nkc - 1:
                            rec = p2r.tile([128, QCH], f32, tag="rec")
                            nc.vector.reciprocal_approx_fast(rec, lrep)
                            nc.vector.tensor_mul(
                                uT[:, h, q0 : q0 + QCH], outp, rec
                            )
                    return emit

                def make_og(q0, qg, og):
                    def emit():
                        o0 = og * 256
                        ot = p3.tile([128, 2, QCH], bf16, tag="ot")
                        for j in range(2):
                            pos = ps3.tile([128, QCH], f32, tag="op")
                            for h in range(HPC):
                                nc.tensor.matmul(
                                    pos,
                                    lhsT=woT_s[
                                        :, h, o0 + j * 128 : o0 + (j + 1) * 128
                                    ],
                                    rhs=uT[:, h, q0 : q0 + QCH],
                                    start=(h == 0),
                                    stop=(h == HPC - 1),
                                )
                            if j == 0:
                                nc.scalar.copy(ot[:, j, :], pos)
                            else:
                                nc.vector.tensor_copy(ot[:, j, :], pos)
                        nc.sync.dma_start(out=outP_d[:, og, qg, :, :], in_=ot)
                    return emit

                for bi in range(b):
                    for qc in range(n_qc):
                        q0 = bi * s + qc * QCH
                        qg = bi * n_qc + qc
                        nkc = (qc + 1) * QCH // 128
                        for h in range(HPC):
                            hctx = {}
                            for kc in range(nkc):
                                gk = bi * (s // 128) + kc
                                st = ps_st.tile([128, QCH], f32, tag="st")
                                nc.tensor.matmul(
                                    st,
                                    lhsT=kTs[gk // n_sub][
                                        :,
                                        h,
                                        (gk % n_sub) * 128 : (gk % n_sub + 1) * 128,
                                    ],
                                    rhs=qTs[qg][:, h, :],
                                    start=True,
                                    stop=True,
                                )
                                di = kc - (nkc - 4)
                                if di >= 0:
                                    # additive -1e9 causal mask on raw scores
                                    nc.vector.tensor_add(
                                        st, st, masks_s[:, di, :]
                                    )
                                pt = p2.tile([128, QCH], bf16, tag="pt")
                                nc.scalar.activation(pt, st, EXP, scale=scale)
                                jobs.append(
                                    make_job(hctx, kc, nkc, pt, bi, h, q0)
                                )
                                drain(DEFER)
                                if kc >= 3 and p3q:
                                    # one output-projection burst of the
                                    # previous q-chunk per k-chunk step
                                    p3q.popleft()()
                        for og in range(n_og):
                            p3q.append(make_og(q0, qg, og))
                drain(0)
                while p3q:
                    p3q.popleft()()

    nc.compile()
    return nc


def make_in_maps(x, Wq, Wk, Wv, Wo, b=B, s=S, dim=DIM, n_cores=N_CORES):
    import ml_dtypes

    bf16 = ml_dtypes.bfloat16
    bs = b * s
    n_din = dim // 128
    n_s1 = bs // SC1
    # x pre-shuffled so each [128, c, s-chunk] tile DMA is one contiguous
    # per-partition segment: xP[p, si, c, s'] = x[si*SC1+s', c*128+p]
    xP = np.ascontiguousarray(
        x.reshape(bs, dim).reshape(n_s1, SC1, n_din, 128).transpose(3, 0, 2, 1)
    ).astype(bf16)
    cosT1, sinT1 = _rope_tables_T(s, HD)
    cosT = np.ascontiguousarray(np.tile(cosT1, (1, b)))
    sinT = np.ascontiguousarray(np.tile(sinT1, (1, b)))
    rT = _rot_matrix_T(HD)
    ones = np.ones((HD, HD), dtype=bf16)
    masks = np.ascontiguousarray(_causal_masks(QCH).transpose(1, 0, 2)).astype(bf16)
    in_maps = []
    for c in range(n_cores):
        sl = slice(c * DLOC, (c + 1) * DLOC)
        # packed q/k/v weights: wA[p, c, iw, m] = W_iw.T[c*128+p, m]
        wA = np.ascontiguousarray(
            np.stack([Wq[sl].T, Wk[sl].T, Wv[sl].T], axis=0)
            .reshape(3, n_din, 128, DLOC)
            .transpose(2, 1, 0, 3)
        ).astype(bf16)
        woP = np.ascontiguousarray(
            Wo[:, sl].T.reshape(HPC, 128, dim).transpose(1, 0, 2)
        ).astype(bf16)
        in_maps.append(
            {
                "xP": xP,
                "wA": wA,
                "woP": woP,
                "cosT": cosT,
                "sinT": sinT,
                "rT": rT,
                "ones": ones,
                "masks": masks,
            }
        )
    return in_maps


def kernel(x, Wq, Wk, Wv, Wo, _trace=False):
    """Full-input / full-output entry point. Shards over 8 cores internally."""
    if "/opt/trn_rl_repo" not in sys.path:
        sys.path.insert(0, "/opt/trn_rl_repo")
    from concourse.bass_utils import run_bass_kernel_spmd

    x = np.asarray(x, dtype=np.float32)
    Wq, Wk, Wv, Wo = (np.asarray(w, dtype=np.float32) for w in (Wq, Wk, Wv, Wo))

    key = (B, S, DIM)
    if key not in _PROGRAM_CACHE:
        _PROGRAM_CACHE[key] = build_program(B, S, DIM)
    nc = _PROGRAM_CACHE[key]

    in_maps = make_in_maps(x, Wq, Wk, Wv, Wo)
    res = run_bass_kernel_spmd(
        nc, in_maps, core_ids=list(range(N_CORES)), trace=_trace
    )
    kernel.last_results = res

    acc = None
    for c in range(N_CORES):
        # outP[p, og, qg, j, s'] = out[og*256 + j*128 + p, qg*QCH + s']
        o = res.results[c]["outP"].astype(np.float32)
        o = o.transpose(1, 3, 0, 2, 4).reshape(DIM, B * S)
        acc = o if acc is None else acc + o
    return np.ascontiguousarray(acc.T).reshape(B, S, DIM)


# revision 19
# speedup vs baseline: 1.3347x; 1.0092x over previous
"""Multi-head causal self-attention with RoPE, tensor-parallel over heads
across 8 Trainium2 NeuronCores.

Strategy (Megatron-style TP over heads):
  - Each core owns 2 of the 16 heads: rows [c*256,(c+1)*256) of Wq/Wk/Wv
    and the matching columns of Wo.
  - On-core: qT/kT projections in transposed [d, s] layout (natural matmul
    output layout), RoPE via a signed-permutation matmul + elementwise ops,
    v in natural [s, d] layout, causal attention with scores computed
    transposed (S^T = K Q^T, softmax sum via a ones-matmul, no running max
    needed -- scores are O(10) so exp() cannot overflow), then a partial
    output projection against the core's Wo column-slice, streamed per
    512-position q-chunk so output DMA overlaps attention.
  - Host sums the 8 partial outputs (this replaces the TP all-reduce).

Schedule notes (all tuned against perfetto traces):
  - All DRAM tensors use host-pre-shuffled layouts so every DMA moves long
    contiguous per-partition segments (naive rearrange patterns shredded
    weight loads into 512B packets and cost ~30us of startup).
  - Warm-up matmuls on the `ones` tile run during the initial DMA wait so
    the PE's HAM clock-gate opens before real work arrives.
  - q/k/v results live in per-512-chunk tiles, not monolithic tensors, so
    phase 2's first score matmuls do not serialize against the LAST RoPE
    writes (tile-granular dependency tracking); the PE flows from phase 1
    into attention without going idle (and without a HAM re-throttle).
  - Attention is k-chunk-granular: score matmul -> (mask) -> exp, with the
    p@v and denominator ones-matmuls DEFERRED three k-chunks behind via a
    job queue that also drains across h/q-chunk boundaries. This keeps the
    in-order PE queue from ever waiting on the scalar engine's exp.
  - The per-q-chunk output projection is split into 8 one-OG bursts popped
    one per k-chunk of the NEXT q-chunk, so its PSUM->SBUF copies never
    flood the ACT/DVE queues ahead of exp/mask work.
  - dtypes: everything DMA'd or used as a matmul operand is bf16 (PE rate
    is identical to fp32r; DMA/SBUF halve); PSUM accumulation and the
    softmax/RoPE elementwise paths stay fp32.
"""

import sys
from collections import deque

import numpy as np

B, S, DIM = 2, 2048, 2048
NUM_HEADS = 16
HD = 128
N_CORES = 8
HPC = NUM_HEADS // N_CORES  # heads per core
DLOC = HPC * HD             # per-core slice of the model dim
ROPE_BASE = 10000.0
QCH = 512                   # attention q-chunk / phase-3 out-chunk
SC1 = 512                   # phase-1 s-chunk
N_WARM = 55                 # PE warm-up matmuls during startup DMA
DEFER = 3                   # k-chunks of pv/ones deferral behind exp

_PROGRAM_CACHE = {}


def _rope_tables_T(seq_len, head_dim):
    # match reference float32 arithmetic: inv_freq over even indices,
    # emb = cat(freqs, freqs); returned transposed [head_dim, seq_len]
    inv_freq = (
        1.0
        / (np.float32(ROPE_BASE)
           ** (np.arange(0, head_dim, 2, dtype=np.float32) / np.float32(head_dim)))
    ).astype(np.float32)
    t = np.arange(seq_len, dtype=np.float32)
    freqs = np.outer(t, inv_freq).astype(np.float32)      # [S, D/2]
    emb = np.concatenate([freqs, freqs], axis=-1)         # [S, D]
    return (
        np.ascontiguousarray(np.cos(emb).astype(np.float32).T),
        np.ascontiguousarray(np.sin(emb).astype(np.float32).T),
    )


def _rot_matrix_T(head_dim):
    # rotated = cat(-x[1::2], x[::2]) = R @ x; return R.T [D, D]
    d2 = head_dim // 2
    R = np.zeros((head_dim, head_dim), dtype=np.float32)
    for dp in range(d2):
        R[dp, 2 * dp + 1] = -1.0
    for dp in range(d2, head_dim):
        R[dp, 2 * (dp - d2)] = 1.0
    return np.ascontiguousarray(R.T)


def _causal_masks(qch):
    # masks[i][kk, qq] = 0 if 128*i + kk <= qq else -1e9 (additive, applied
    # to raw scores before exp, for the 4 diagonal k-chunks of each q-chunk)
    m = np.zeros((4, 128, qch), dtype=np.float32)
    kk = np.arange(128)[:, None]
    qq = np.arange(qch)[None, :]
    for i in range(4):
        m[i] = np.where(128 * i + kk <= qq, 0.0, -1e9).astype(np.float32)
    return m


def build_program(b=B, s=S, dim=DIM):
    """Builds the per-core SPMD Bass program (identical on every core)."""
    if "/opt/trn_rl_repo" not in sys.path:
        sys.path.insert(0, "/opt/trn_rl_repo")
    import concourse.bacc as bacc
    import concourse.mybir as mybir
    import concourse.tile as tile

    f32 = mybir.dt.float32
    f32r = mybir.dt.float32r
    bf16 = mybir.dt.bfloat16
    EXP = mybir.ActivationFunctionType.Exp

    bs = b * s
    n_din = dim // 128          # contraction chunks for projections
    n_s1 = bs // SC1            # phase-1 s-chunks
    n_qc = s // QCH             # attention q-chunks per batch
    n_sub = SC1 // 128
    n_og = dim // 256           # phase-3 256-row output groups
    scale = float(HD) ** -0.5

    nc = bacc.Bacc("TRN2", target_bir_lowering=False, debug=False)

    # host-pre-shuffled layouts: every DMA slice is contiguous per partition
    xP_d = nc.dram_tensor("xP", [128, n_s1, n_din, SC1], bf16, kind="ExternalInput")
    wA_d = nc.dram_tensor("wA", [128, n_din, 3, DLOC], bf16, kind="ExternalInput")
    woP_d = nc.dram_tensor("woP", [128, HPC, dim], bf16, kind="ExternalInput")
    cosT_d = nc.dram_tensor("cosT", [HD, bs], f32, kind="ExternalInput")
    sinT_d = nc.dram_tensor("sinT", [HD, bs], f32, kind="ExternalInput")
    rT_d = nc.dram_tensor("rT", [HD, HD], f32r, kind="ExternalInput")
    ones_d = nc.dram_tensor("ones", [HD, HD], bf16, kind="ExternalInput")
    masks_d = nc.dram_tensor("masks", [128, 4, QCH], bf16, kind="ExternalInput")
    outP_d = nc.dram_tensor(
        "outP", [128, n_og, b * n_qc, 2, QCH], bf16, kind="ExternalOutput"
    )

    with tile.TileContext(nc) as tc:
        with tc.tile_pool(name="persist", bufs=1) as persist:
            # per-512-chunk projection tiles (fine-grained deps; see header)
            qTs = [
                persist.tile([128, HPC, SC1], bf16, name=f"qT{i}")
                for i in range(n_s1)
            ]
            kTs = [
                persist.tile([128, HPC, SC1], bf16, name=f"kT{i}")
                for i in range(n_s1)
            ]
            vSs = [
                persist.tile([128, n_sub, DLOC], bf16, name=f"vS{i}")
                for i in range(n_s1)
            ]
            rTs = persist.tile([HD, HD], f32r)
            ones = persist.tile([128, 128], bf16)
            masks_s = persist.tile([128, 4, QCH], bf16)
            woT_s = persist.tile([128, HPC, dim], bf16)

            # ---------------- phase 1: qkv projections + RoPE ----------------
            with (
                tc.tile_pool(name="p1w", bufs=1) as p1w,
                tc.tile_pool(name="p1x", bufs=2) as p1x,
                tc.tile_pool(name="p1t", bufs=2) as p1t,
                tc.tile_pool(name="ps_qk", bufs=2, space="PSUM") as ps_qk,
                tc.tile_pool(name="ps_rot", bufs=2, space="PSUM") as ps_rot,
                tc.tile_pool(name="ps_v", bufs=2, space="PSUM") as ps_v,
            ):
                wA_s = p1w.tile([128, n_din, 3, DLOC], bf16)
                gw = max(1, n_din // 4)

                # startup order: ones (for warm-up) -> first weight group ->
                # first x chunk in quarters -> remaining weights -> tables.
                # masks/woT (attention-only) ride the gpsimd queue at si==1.
                nc.sync.dma_start(out=ones, in_=ones_d[:])
                xt0 = p1x.tile([128, n_din, SC1], bf16, tag="xt")
                nc.sync.dma_start(out=xt0[:, 0:4, :], in_=xP_d[:, 0, 0:4, :])
                nc.sync.dma_start(out=wA_s[:, 0:gw, :, :], in_=wA_d[:, 0:gw, :, :])
                for qq in range(1, 4):
                    nc.sync.dma_start(
                        out=xt0[:, qq * 4 : (qq + 1) * 4, :],
                        in_=xP_d[:, 0, qq * 4 : (qq + 1) * 4, :],
                    )
                for g0 in range(gw, n_din, gw):
                    nc.sync.dma_start(
                        out=wA_s[:, g0 : g0 + gw, :, :], in_=wA_d[:, g0 : g0 + gw, :, :]
                    )
                nc.sync.dma_start(out=rTs, in_=rT_d[:])
                cost0 = p1x.tile([128, SC1], f32, tag="cost")
                nc.sync.dma_start(out=cost0, in_=cosT_d[:, 0:SC1])
                sint0 = p1x.tile([128, SC1], f32, tag="sint")
                nc.sync.dma_start(out=sint0, in_=sinT_d[:, 0:SC1])

                # PE warm-up: open the HAM clock gate during the DMA wait
                warm = ps_rot.tile([128, SC1], f32, tag="rot")
                for _ in range(N_WARM):
                    nc.tensor.matmul(
                        warm[:, :128], lhsT=ones, rhs=ones, start=True, stop=True
                    )

                def issue_x(si):
                    s0 = si * SC1
                    xt = p1x.tile([128, n_din, SC1], bf16, tag="xt")
                    xq = nc.sync if si % 2 == 0 else nc.gpsimd
                    nh = n_din // 2
                    xq.dma_start(out=xt[:, :nh, :], in_=xP_d[:, si, :nh, :])
                    xq.dma_start(out=xt[:, nh:, :], in_=xP_d[:, si, nh:, :])
                    cost = p1x.tile([128, SC1], f32, tag="cost")
                    xq.dma_start(out=cost, in_=cosT_d[:, s0 : s0 + SC1])
                    sint = p1x.tile([128, SC1], f32, tag="sint")
                    xq.dma_start(out=sint, in_=sinT_d[:, s0 : s0 + SC1])
                    if si == 1:
                        nc.gpsimd.dma_start(out=masks_s, in_=masks_d[:])
                        nc.gpsimd.dma_start(out=woT_s, in_=woP_d[:])
                    return xt, cost, sint

                # RoPE for a finished half-chunk is emitted one half-chunk
                # later so the rot-matmul never stalls the PE on the scalar
                # engine's PSUM->SBUF copy of its input
                pend = []

                def emit_ropes():
                    # two-pass: the t2 muls (the only PSUM readers) run first
                    # so the rot banks release as early as possible
                    work = []
                    while pend:
                        raw, cs, sn, dst = pend.pop(0)
                        rot = ps_rot.tile([128, SC1], f32, tag="rot")
                        nc.tensor.matmul(
                            rot, lhsT=rTs, rhs=raw, start=True, stop=True
                        )
                        work.append((raw, cs, sn, dst, rot))
                    t2s = []
                    for raw, cs, sn, dst, rot in work:
                        t2 = p1t.tile([128, SC1], f32, tag="t2")
                        nc.vector.tensor_mul(t2, rot, sn)
                        t2s.append(t2)
                    for (raw, cs, sn, dst, rot), t2 in zip(work, t2s):
                        t1 = p1t.tile([128, SC1], f32, tag="t1")
                        nc.vector.tensor_mul(t1, raw.bitcast(f32), cs)
                        nc.vector.tensor_add(dst, t1, t2)

                for si in range(n_s1):
                    xt, cost, sint = (
                        (xt0, cost0, sint0) if si == 0 else issue_x(si)
                    )
                    for h in range(HPC):
                        qacc = ps_qk.tile([128, SC1], f32, tag="qa")
                        kacc = ps_qk.tile([128, SC1], f32, tag="ka")
                        # each h-pass carries half the v sub-chunks; two subs
                        # share this 2KB PSUM bank so the accumulation group
                        # (start clears the whole bank's has_written bits)
                        # opens on the first sub and closes on the last
                        vacc = ps_v.tile([128, 2, DLOC], f32, tag="va")
                        for c in range(n_din):
                            nc.tensor.matmul(
                                qacc,
                                lhsT=wA_s[:, c, 0, h * HD : (h + 1) * HD],
                                rhs=xt[:, c, :],
                                start=(c == 0),
                                stop=(c == n_din - 1),
                            )
                            nc.tensor.matmul(
                                kacc,
                                lhsT=wA_s[:, c, 1, h * HD : (h + 1) * HD],
                                rhs=xt[:, c, :],
                                start=(c == 0),
                                stop=(c == n_din - 1),
                            )
                            for jsub in range(2):
                                sub = 2 * h + jsub
                                nc.tensor.matmul(
                                    vacc[:, jsub, :],
                                    lhsT=xt[:, c, sub * 128 : (sub + 1) * 128],
                                    rhs=wA_s[:, c, 2, :],
                                    start=(c == 0 and jsub == 0),
                                    stop=(c == n_din - 1 and jsub == 1),
                                )
                        emit_ropes()
                        rawq = p1t.tile([128, SC1], f32r, tag=f"rawq{h}")
                        nc.scalar.copy(rawq, qacc)
                        rawk = p1t.tile([128, SC1], f32r, tag=f"rawk{h}")
                        nc.scalar.copy(rawk, kacc)
                        nc.scalar.copy(vSs[si][:, 2 * h : 2 * h + 2, :], vacc)
                        pend.append((rawq, cost, sint, qTs[si][:, h, :]))
                        pend.append((rawk, cost, sint, kTs[si][:, h, :]))
                emit_ropes()

            # ------------- phases 2+3: attention + streamed output projection -------------
            with (
                tc.tile_pool(name="persistB", bufs=1) as persistB,
                tc.tile_pool(name="p2", bufs=6) as p2,
                tc.tile_pool(name="p2l", bufs=3) as p2l,
                tc.tile_pool(name="p2r", bufs=2) as p2r,
                tc.tile_pool(name="p3", bufs=3) as p3,
                # creation order fixes PSUM bank assignment: these banks
                # collide with phase-1 pools whose last readers finish at
                # different times -- put ps_st last so its slots land on the
                # banks the phase-1 tail frees earliest
                tc.tile_pool(name="ps_o", bufs=3, space="PSUM") as ps_o,
                tc.tile_pool(name="ps3", bufs=2, space="PSUM") as ps3,
                tc.tile_pool(name="ps_st", bufs=3, space="PSUM") as ps_st,
            ):
                uT = persistB.tile([128, HPC, bs], bf16)  # attn out, [d, h, b*s]
                jobs = deque()          # deferred pv/ones emissions
                p3q = deque()           # deferred output-projection OG bursts

                # bridge warm-up: keep the PE's HAM clock-gate open while the
                # phase-1 DVE/ACT tail drains (the first attention matmuls
                # wait on PSUM-bank anti-deps from that tail)
                wscr = ps3.tile([128, QCH], f32, tag="op")
                for _ in range(20):
                    nc.tensor.matmul(
                        wscr[:, :128], lhsT=ones, rhs=ones, start=True, stop=True
                    )

                def drain(keep):
                    while len(jobs) > keep:
                        jobs.popleft()()

                def make_job(hctx, kc, nkc, pt, lp, bi, h, q0):
                    def emit():
                        if "outp" not in hctx:
                            # lrep first: with bufs=3 the next h's outp then
                            # lands on this h's lrep slot, which frees at
                            # recip() -- earlier than outp's uT-divide
                            hctx["lrep"] = ps_o.tile(
                                [128, QCH], f32, tag="o", name="lrep"
                            )
                            hctx["outp"] = ps_o.tile(
                                [128, QCH], f32, tag="o", name="outp"
                            )
                        outp, lrep = hctx["outp"], hctx["lrep"]
                        gk = bi * (s // 128) + kc
                        nc.tensor.matmul(
                            outp,
                            lhsT=vSs[gk // n_sub][
                                :, gk % n_sub, h * HD : (h + 1) * HD
                            ],
                            rhs=pt,
                            start=(kc == 0),
                            stop=(kc == nkc - 1),
                        )
                        if lp is not None:
                            # softmax denominator: gpsimd pair-sum halves the
                            # ones-matmul count; its latency hides behind the
                            # job deferral window
                            nc.tensor.matmul(
                                lrep,
                                lhsT=ones,
                                rhs=lp,
                                start=(kc == 1),
                                stop=(kc == nkc - 1),
                            )
                        if kc == nkc - 1:
                            rec = p2r.tile([128, QCH], f32, tag="rec")
                            nc.vector.reciprocal_approx_fast(rec, lrep)
                            nc.vector.tensor_mul(
                                uT[:, h, q0 : q0 + QCH], outp, rec
                            )
                    return emit

                def make_og(q0, qg, og):
                    def emit():
                        o0 = og * 256
                        ot = p3.tile([128, 2, QCH], bf16, tag="ot")
                        for j in range(2):
                            pos = ps3.tile([128, QCH], f32, tag="op")
                            for h in range(HPC):
                                nc.tensor.matmul(
                                    pos,
                                    lhsT=woT_s[
                                        :, h, o0 + j * 128 : o0 + (j + 1) * 128
                                    ],
                                    rhs=uT[:, h, q0 : q0 + QCH],
                                    start=(h == 0),
                                    stop=(h == HPC - 1),
                                )
                            if j == 0:
                                nc.scalar.copy(ot[:, j, :], pos)
                            else:
                                nc.vector.tensor_copy(ot[:, j, :], pos)
                        nc.sync.dma_start(out=outP_d[:, og, qg, :, :], in_=ot)
                    return emit

                for bi in range(b):
                    for qc in range(n_qc):
                        q0 = bi * s + qc * QCH
                        qg = bi * n_qc + qc
                        nkc = (qc + 1) * QCH // 128
                        for h in range(HPC):
                            hctx = {}
                            prev_pt = None
                            for kc in range(nkc):
                                gk = bi * (s // 128) + kc
                                st = ps_st.tile([128, QCH], f32, tag="st")
                                nc.tensor.matmul(
                                    st,
                                    lhsT=kTs[gk // n_sub][
                                        :,
                                        h,
                                        (gk % n_sub) * 128 : (gk % n_sub + 1) * 128,
                                    ],
                                    rhs=qTs[qg][:, h, :],
                                    start=True,
                                    stop=True,
                                )
                                di = kc - (nkc - 4)
                                if di >= 0:
                                    # additive -1e9 causal mask on raw scores
                                    nc.vector.tensor_add(
                                        st, st, masks_s[:, di, :]
                                    )
                                pt = p2.tile([128, QCH], bf16, tag="pt")
                                nc.scalar.activation(pt, st, EXP, scale=scale)
                                lp = None
                                if kc % 2 == 1:
                                    lp = p2l.tile([128, QCH], bf16, tag="lp")
                                    nc.gpsimd.tensor_add(lp, prev_pt, pt)
                                prev_pt = pt
                                jobs.append(
                                    make_job(hctx, kc, nkc, pt, lp, bi, h, q0)
                                )
                                drain(DEFER)
                                if kc >= 3 and p3q:
                                    # one output-projection burst of the
                                    # previous q-chunk per k-chunk step
                                    p3q.popleft()()
                        for og in range(n_og):
                            p3q.append(make_og(q0, qg, og))
                drain(0)
                while p3q:
                    p3q.popleft()()

    nc.compile()
    return nc


def make_in_maps(x, Wq, Wk, Wv, Wo, b=B, s=S, dim=DIM, n_cores=N_CORES):
    import ml_dtypes

    bf16 = ml_dtypes.bfloat16
    bs = b * s
    n_din = dim // 128
    n_s1 = bs // SC1
    # x pre-shuffled so each [128, c, s-chunk] tile DMA is one contiguous
    # per-partition segment: xP[p, si, c, s'] = x[si*SC1+s', c*128+p]
    xP = np.ascontiguousarray(
        x.reshape(bs, dim).reshape(n_s1, SC1, n_din, 128).transpose(3, 0, 2, 1)
    ).astype(bf16)
    cosT1, sinT1 = _rope_tables_T(s, HD)
    cosT = np.ascontiguousarray(np.tile(cosT1, (1, b)))
    sinT = np.ascontiguousarray(np.tile(sinT1, (1, b)))
    rT = _rot_matrix_T(HD)
    ones = np.ones((HD, HD), dtype=bf16)
    masks = np.ascontiguousarray(_causal_masks(QCH).transpose(1, 0, 2)).astype(bf16)
    in_maps = []
    for c in range(n_cores):
        sl = slice(c * DLOC, (c + 1) * DLOC)
        # packed q/k/v weights: wA[p, c, iw, m] = W_iw.T[c*128+p, m]
        wA = np.ascontiguousarray(
            np.stack([Wq[sl].T, Wk[sl].T, Wv[sl].T], axis=0)
            .reshape(3, n_din, 128, DLOC)
            .transpose(2, 1, 0, 3)
        ).astype(bf16)
        woP = np.ascontiguousarray(
            Wo[:, sl].T.reshape(HPC, 128, dim).transpose(1, 0, 2)
        ).astype(bf16)
        in_maps.append(
            {
                "xP": xP,
                "wA": wA,
                "woP": woP,
                "cosT": cosT,
                "sinT": sinT,
                "rT": rT,
                "ones": ones,
                "masks": masks,
            }
        )
    return in_maps


def kernel(x, Wq, Wk, Wv, Wo, _trace=False):
    """Full-input / full-output entry point. Shards over 8 cores internally."""
    if "/opt/trn_rl_repo" not in sys.path:
        sys.path.insert(0, "/opt/trn_rl_repo")
    from concourse.bass_utils import run_bass_kernel_spmd

    x = np.asarray(x, dtype=np.float32)
    Wq, Wk, Wv, Wo = (np.asarray(w, dtype=np.float32) for w in (Wq, Wk, Wv, Wo))

    key = (B, S, DIM)
    if key not in _PROGRAM_CACHE:
        _PROGRAM_CACHE[key] = build_program(B, S, DIM)
    nc = _PROGRAM_CACHE[key]

    in_maps = make_in_maps(x, Wq, Wk, Wv, Wo)
    res = run_bass_kernel_spmd(
        nc, in_maps, core_ids=list(range(N_CORES)), trace=_trace
    )
    kernel.last_results = res

    acc = None
    for c in range(N_CORES):
        # outP[p, og, qg, j, s'] = out[og*256 + j*128 + p, qg*QCH + s']
        o = res.results[c]["outP"].astype(np.float32)
        o = o.transpose(1, 3, 0, 2, 4).reshape(DIM, B * S)
        acc = o if acc is None else acc + o
    return np.ascontiguousarray(acc.T).reshape(B, S, DIM)


# revision 22
# speedup vs baseline: 1.3422x; 1.0057x over previous
"""Multi-head causal self-attention with RoPE, tensor-parallel over heads
across 8 Trainium2 NeuronCores.

Strategy (Megatron-style TP over heads):
  - Each core owns 2 of the 16 heads: rows [c*256,(c+1)*256) of Wq/Wk/Wv
    and the matching columns of Wo.
  - On-core: qT/kT projections in transposed [d, s] layout (natural matmul
    output layout), RoPE via a signed-permutation matmul + elementwise ops,
    v in natural [s, d] layout, causal attention with scores computed
    transposed (S^T = K Q^T, softmax sum via a ones-matmul, no running max
    needed -- scores are O(10) so exp() cannot overflow), then a partial
    output projection against the core's Wo column-slice, streamed per
    512-position q-chunk so output DMA overlaps attention.
  - Host sums the 8 partial outputs (this replaces the TP all-reduce).

Schedule notes (all tuned against perfetto traces):
  - All DRAM tensors use host-pre-shuffled layouts so every DMA moves long
    contiguous per-partition segments (naive rearrange patterns shredded
    weight loads into 512B packets and cost ~30us of startup).
  - Warm-up matmuls on the `ones` tile run during the initial DMA wait so
    the PE's HAM clock-gate opens before real work arrives.
  - q/k/v results live in per-512-chunk tiles, not monolithic tensors, so
    phase 2's first score matmuls do not serialize against the LAST RoPE
    writes (tile-granular dependency tracking); the PE flows from phase 1
    into attention without going idle (and without a HAM re-throttle).
  - Attention is k-chunk-granular: score matmul -> (mask) -> exp, with the
    p@v and denominator ones-matmuls DEFERRED three k-chunks behind via a
    job queue that also drains across h/q-chunk boundaries. This keeps the
    in-order PE queue from ever waiting on the scalar engine's exp.
  - The per-q-chunk output projection is split into 8 one-OG bursts popped
    one per k-chunk of the NEXT q-chunk, so its PSUM->SBUF copies never
    flood the ACT/DVE queues ahead of exp/mask work.
  - dtypes: everything DMA'd or used as a matmul operand is bf16 (PE rate
    is identical to fp32r; DMA/SBUF halve); PSUM accumulation and the
    softmax/RoPE elementwise paths stay fp32.
"""

import sys
from collections import deque

import numpy as np

B, S, DIM = 2, 2048, 2048
NUM_HEADS = 16
HD = 128
N_CORES = 8
HPC = NUM_HEADS // N_CORES  # heads per core
DLOC = HPC * HD             # per-core slice of the model dim
ROPE_BASE = 10000.0
QCH = 512                   # attention q-chunk / phase-3 out-chunk
SC1 = 512                   # phase-1 s-chunk
N_WARM = 55                 # PE warm-up matmuls during startup DMA
DEFER = 3                   # k-chunks of pv/ones deferral behind exp

_PROGRAM_CACHE = {}


def _rope_tables_T(seq_len, head_dim):
    # match reference float32 arithmetic: inv_freq over even indices,
    # emb = cat(freqs, freqs); returned transposed [head_dim, seq_len]
    inv_freq = (
        1.0
        / (np.float32(ROPE_BASE)
           ** (np.arange(0, head_dim, 2, dtype=np.float32) / np.float32(head_dim)))
    ).astype(np.float32)
    t = np.arange(seq_len, dtype=np.float32)
    freqs = np.outer(t, inv_freq).astype(np.float32)      # [S, D/2]
    emb = np.concatenate([freqs, freqs], axis=-1)         # [S, D]
    return (
        np.ascontiguousarray(np.cos(emb).astype(np.float32).T),
        np.ascontiguousarray(np.sin(emb).astype(np.float32).T),
    )


def _rot_matrix_T(head_dim):
    # rotated = cat(-x[1::2], x[::2]) = R @ x; return R.T [D, D]
    d2 = head_dim // 2
    R = np.zeros((head_dim, head_dim), dtype=np.float32)
    for dp in range(d2):
        R[dp, 2 * dp + 1] = -1.0
    for dp in range(d2, head_dim):
        R[dp, 2 * (dp - d2)] = 1.0
    return np.ascontiguousarray(R.T)


def _causal_masks(qch):
    # masks[i][kk, qq] = 0 if 128*i + kk <= qq else -1e9 (additive, applied
    # to raw scores before exp, for the 4 diagonal k-chunks of each q-chunk)
    m = np.zeros((4, 128, qch), dtype=np.float32)
    kk = np.arange(128)[:, None]
    qq = np.arange(qch)[None, :]
    for i in range(4):
        m[i] = np.where(128 * i + kk <= qq, 0.0, -1e9).astype(np.float32)
    return m


def build_program(b=B, s=S, dim=DIM):
    """Builds the per-core SPMD Bass program (identical on every core)."""
    if "/opt/trn_rl_repo" not in sys.path:
        sys.path.insert(0, "/opt/trn_rl_repo")
    import concourse.bacc as bacc
    import concourse.mybir as mybir
    import concourse.tile as tile

    f32 = mybir.dt.float32
    f32r = mybir.dt.float32r
    bf16 = mybir.dt.bfloat16
    EXP = mybir.ActivationFunctionType.Exp

    bs = b * s
    n_din = dim // 128          # contraction chunks for projections
    n_s1 = bs // SC1            # phase-1 s-chunks
    n_qc = s // QCH             # attention q-chunks per batch
    n_sub = SC1 // 128
    n_og = dim // 256           # phase-3 256-row output groups
    scale = float(HD) ** -0.5

    nc = bacc.Bacc("TRN2", target_bir_lowering=False, debug=False)

    # host-pre-shuffled layouts: every DMA slice is contiguous per partition
    xP_d = nc.dram_tensor("xP", [128, n_s1, n_din, SC1], bf16, kind="ExternalInput")
    wA_d = nc.dram_tensor("wA", [128, n_din, 3, DLOC], bf16, kind="ExternalInput")
    woP_d = nc.dram_tensor("woP", [128, HPC, dim], bf16, kind="ExternalInput")
    cosT_d = nc.dram_tensor("cosT", [HD, bs], f32, kind="ExternalInput")
    sinT_d = nc.dram_tensor("sinT", [HD, bs], f32, kind="ExternalInput")
    rT_d = nc.dram_tensor("rT", [HD, HD], f32r, kind="ExternalInput")
    ones_d = nc.dram_tensor("ones", [HD, HD], bf16, kind="ExternalInput")
    masks_d = nc.dram_tensor("masks", [128, 4, QCH], bf16, kind="ExternalInput")
    outP_d = nc.dram_tensor(
        "outP", [128, n_og, b * n_qc, 2, QCH], bf16, kind="ExternalOutput"
    )

    with tile.TileContext(nc) as tc:
        with tc.tile_pool(name="persist", bufs=1) as persist:
            # per-512-chunk projection tiles (fine-grained deps; see header)
            qTs = [
                persist.tile([128, HPC, SC1], bf16, name=f"qT{i}")
                for i in range(n_s1)
            ]
            kTs = [
                persist.tile([128, HPC, SC1], bf16, name=f"kT{i}")
                for i in range(n_s1)
            ]
            vSs = [
                persist.tile([128, n_sub, DLOC], bf16, name=f"vS{i}")
                for i in range(n_s1)
            ]
            rTs = persist.tile([HD, HD], f32r)
            ones = persist.tile([128, 128], bf16)
            masks_s = persist.tile([128, 4, QCH], bf16)
            woT_s = persist.tile([128, HPC, dim], bf16)

            # ---------------- phase 1: qkv projections + RoPE ----------------
            with (
                tc.tile_pool(name="p1w", bufs=1) as p1w,
                tc.tile_pool(name="p1x", bufs=2) as p1x,
                tc.tile_pool(name="p1t", bufs=2) as p1t,
                tc.tile_pool(name="ps_qk", bufs=2, space="PSUM") as ps_qk,
                tc.tile_pool(name="ps_rot", bufs=2, space="PSUM") as ps_rot,
                tc.tile_pool(name="ps_v", bufs=2, space="PSUM") as ps_v,
            ):
                wA_s = p1w.tile([128, n_din, 3, DLOC], bf16)
                gw = max(1, n_din // 4)

                # startup order: ones (for warm-up) -> first weight group ->
                # first x chunk in quarters -> remaining weights -> tables.
                # masks/woT (attention-only) ride the gpsimd queue at si==1.
                nc.sync.dma_start(out=ones, in_=ones_d[:])
                xt0 = p1x.tile([128, n_din, SC1], bf16, tag="xt")
                nc.sync.dma_start(out=xt0[:, 0:4, :], in_=xP_d[:, 0, 0:4, :])
                nc.sync.dma_start(out=wA_s[:, 0:gw, :, :], in_=wA_d[:, 0:gw, :, :])
                for qq in range(1, 4):
                    nc.sync.dma_start(
                        out=xt0[:, qq * 4 : (qq + 1) * 4, :],
                        in_=xP_d[:, 0, qq * 4 : (qq + 1) * 4, :],
                    )
                for g0 in range(gw, n_din, gw):
                    nc.sync.dma_start(
                        out=wA_s[:, g0 : g0 + gw, :, :], in_=wA_d[:, g0 : g0 + gw, :, :]
                    )
                nc.sync.dma_start(out=rTs, in_=rT_d[:])
                cost0 = p1x.tile([128, SC1], f32, tag="cost")
                nc.sync.dma_start(out=cost0, in_=cosT_d[:, 0:SC1])
                sint0 = p1x.tile([128, SC1], f32, tag="sint")
                nc.sync.dma_start(out=sint0, in_=sinT_d[:, 0:SC1])

                # PE warm-up: open the HAM clock gate during the DMA wait
                warm = ps_rot.tile([128, SC1], f32, tag="rot")
                for _ in range(N_WARM):
                    nc.tensor.matmul(
                        warm[:, :128], lhsT=ones, rhs=ones, start=True, stop=True
                    )

                def issue_x(si):
                    # all x on the sync queue: the gpsimd DGE would otherwise
                    # compete for HBM bandwidth during the critical startup
                    s0 = si * SC1
                    xt = p1x.tile([128, n_din, SC1], bf16, tag="xt")
                    nh = n_din // 2
                    nc.sync.dma_start(out=xt[:, :nh, :], in_=xP_d[:, si, :nh, :])
                    nc.sync.dma_start(out=xt[:, nh:, :], in_=xP_d[:, si, nh:, :])
                    cost = p1x.tile([128, SC1], f32, tag="cost")
                    nc.sync.dma_start(out=cost, in_=cosT_d[:, s0 : s0 + SC1])
                    sint = p1x.tile([128, SC1], f32, tag="sint")
                    nc.sync.dma_start(out=sint, in_=sinT_d[:, s0 : s0 + SC1])
                    if si == 2:
                        # attention-only tensors, needed ~150us later
                        nc.gpsimd.dma_start(out=masks_s, in_=masks_d[:])
                        nc.gpsimd.dma_start(out=woT_s, in_=woP_d[:])
                    return xt, cost, sint

                # RoPE for a finished half-chunk is emitted one half-chunk
                # later so the rot-matmul never stalls the PE on the scalar
                # engine's PSUM->SBUF copy of its input
                pend = []

                def emit_ropes():
                    # two-pass: the t2 muls (the only PSUM readers) run first
                    # so the rot banks release as early as possible
                    work = []
                    while pend:
                        raw, cs, sn, dst = pend.pop(0)
                        rot = ps_rot.tile([128, SC1], f32, tag="rot")
                        nc.tensor.matmul(
                            rot, lhsT=rTs, rhs=raw, start=True, stop=True
                        )
                        work.append((raw, cs, sn, dst, rot))
                    t2s = []
                    for raw, cs, sn, dst, rot in work:
                        t2 = p1t.tile([128, SC1], f32, tag="t2")
                        nc.vector.tensor_mul(t2, rot, sn)
                        t2s.append(t2)
                    for (raw, cs, sn, dst, rot), t2 in zip(work, t2s):
                        t1 = p1t.tile([128, SC1], f32, tag="t1")
                        nc.vector.tensor_mul(t1, raw.bitcast(f32), cs)
                        nc.vector.tensor_add(dst, t1, t2)

                for si in range(n_s1):
                    xt, cost, sint = (
                        (xt0, cost0, sint0) if si == 0 else issue_x(si)
                    )
                    for h in range(HPC):
                        qacc = ps_qk.tile([128, SC1], f32, tag="qa")
                        kacc = ps_qk.tile([128, SC1], f32, tag="ka")
                        # each h-pass carries half the v sub-chunks; two subs
                        # share this 2KB PSUM bank so the accumulation group
                        # (start clears the whole bank's has_written bits)
                        # opens on the first sub and closes on the last
                        vacc = ps_v.tile([128, 2, DLOC], f32, tag="va")
                        for c in range(n_din):
                            nc.tensor.matmul(
                                qacc,
                                lhsT=wA_s[:, c, 0, h * HD : (h + 1) * HD],
                                rhs=xt[:, c, :],
                                start=(c == 0),
                                stop=(c == n_din - 1),
                            )
                            nc.tensor.matmul(
                                kacc,
                                lhsT=wA_s[:, c, 1, h * HD : (h + 1) * HD],
                                rhs=xt[:, c, :],
                                start=(c == 0),
                                stop=(c == n_din - 1),
                            )
                            for jsub in range(2):
                                sub = 2 * h + jsub
                                nc.tensor.matmul(
                                    vacc[:, jsub, :],
                                    lhsT=xt[:, c, sub * 128 : (sub + 1) * 128],
                                    rhs=wA_s[:, c, 2, :],
                                    start=(c == 0 and jsub == 0),
                                    stop=(c == n_din - 1 and jsub == 1),
                                )
                        emit_ropes()
                        rawq = p1t.tile([128, SC1], f32r, tag=f"rawq{h}")
                        nc.scalar.copy(rawq, qacc)
                        rawk = p1t.tile([128, SC1], f32r, tag=f"rawk{h}")
                        nc.scalar.copy(rawk, kacc)
                        nc.scalar.copy(vSs[si][:, 2 * h : 2 * h + 2, :], vacc)
                        pend.append((rawq, cost, sint, qTs[si][:, h, :]))
                        pend.append((rawk, cost, sint, kTs[si][:, h, :]))
                emit_ropes()

            # ------------- phases 2+3: attention + streamed output projection -------------
            with (
                tc.tile_pool(name="persistB", bufs=1) as persistB,
                tc.tile_pool(name="p2", bufs=6) as p2,
                tc.tile_pool(name="p2l", bufs=3) as p2l,
                tc.tile_pool(name="p2r", bufs=2) as p2r,
                tc.tile_pool(name="p3", bufs=3) as p3,
                # creation order fixes PSUM bank assignment: these banks
                # collide with phase-1 pools whose last readers finish at
                # different times -- put ps_st last so its slots land on the
                # banks the phase-1 tail frees earliest
                tc.tile_pool(name="ps_o", bufs=3, space="PSUM") as ps_o,
                tc.tile_pool(name="ps3", bufs=2, space="PSUM") as ps3,
                tc.tile_pool(name="ps_st", bufs=3, space="PSUM") as ps_st,
            ):
                uT = persistB.tile([128, HPC, bs], bf16)  # attn out, [d, h, b*s]
                jobs = deque()          # deferred pv/ones emissions
                p3q = deque()           # deferred output-projection OG bursts

                # bridge warm-up: keep the PE's HAM clock-gate open while the
                # phase-1 DVE/ACT tail drains (the first attention matmuls
                # wait on PSUM-bank anti-deps from that tail)
                wscr = ps3.tile([128, QCH], f32, tag="op")
                for _ in range(20):
                    nc.tensor.matmul(
                        wscr[:, :128], lhsT=ones, rhs=ones, start=True, stop=True
                    )

                def drain(keep):
                    while len(jobs) > keep:
                        jobs.popleft()()

                def make_job(hctx, kc, nkc, pt, lp, bi, h, q0, first, last):
                    def emit():
                        if "outp" not in hctx:
                            # lrep first: with bufs=3 the next h's outp then
                            # lands on this h's lrep slot, which frees at
                            # recip() -- earlier than outp's uT-divide
                            hctx["lrep"] = ps_o.tile(
                                [128, QCH], f32, tag="o", name="lrep"
                            )
                            hctx["outp"] = ps_o.tile(
                                [128, QCH], f32, tag="o", name="outp"
                            )
                        outp, lrep = hctx["outp"], hctx["lrep"]
                        gk = bi * (s // 128) + kc
                        nc.tensor.matmul(
                            outp,
                            lhsT=vSs[gk // n_sub][
                                :, gk % n_sub, h * HD : (h + 1) * HD
                            ],
                            rhs=pt,
                            start=first,
                            stop=last,
                        )
                        if lp is not None:
                            # softmax denominator: gpsimd pair-sum halves the
                            # ones-matmul count; its latency hides behind the
                            # job deferral window
                            nc.tensor.matmul(
                                lrep,
                                lhsT=ones,
                                rhs=lp,
                                start=(first or kc == nkc - 2),
                                stop=last,
                            )
                        if last:
                            rec = p2r.tile([128, QCH], f32, tag="rec")
                            nc.vector.reciprocal_approx_fast(rec, lrep)
                            nc.vector.tensor_mul(
                                uT[:, h, q0 : q0 + QCH], outp, rec
                            )
                    return emit

                def make_og(q0, qg, og):
                    def emit():
                        o0 = og * 256
                        ot = p3.tile([128, 2, QCH], bf16, tag="ot")
                        for j in range(2):
                            pos = ps3.tile([128, QCH], f32, tag="op")
                            for h in range(HPC):
                                nc.tensor.matmul(
                                    pos,
                                    lhsT=woT_s[
                                        :, h, o0 + j * 128 : o0 + (j + 1) * 128
                                    ],
                                    rhs=uT[:, h, q0 : q0 + QCH],
                                    start=(h == 0),
                                    stop=(h == HPC - 1),
                                )
                            if j == 0:
                                nc.scalar.copy(ot[:, j, :], pos)
                            else:
                                nc.vector.tensor_copy(ot[:, j, :], pos)
                        nc.sync.dma_start(out=outP_d[:, og, qg, :, :], in_=ot)
                    return emit

                for bi in range(b):
                    for qc in range(n_qc):
                        q0 = bi * s + qc * QCH
                        qg = bi * n_qc + qc
                        nkc = (qc + 1) * QCH // 128
                        for h in range(HPC):
                            hctx = {}
                            prev_pt = None
                            # k-chunks DESCENDING: the masked diagonal chunks
                            # (whose mask->exp->pair-sum chains are longest)
                            # run first, when their deferred pv/ones pops are
                            # still far away; the final chunk's exp is
                            # mask-free so the closing recip chain is short
                            for ki in range(nkc):
                                kc = nkc - 1 - ki
                                gk = bi * (s // 128) + kc
                                st = ps_st.tile([128, QCH], f32, tag="st")
                                nc.tensor.matmul(
                                    st,
                                    lhsT=kTs[gk // n_sub][
                                        :,
                                        h,
                                        (gk % n_sub) * 128 : (gk % n_sub + 1) * 128,
                                    ],
                                    rhs=qTs[qg][:, h, :],
                                    start=True,
                                    stop=True,
                                )
                                di = kc - (nkc - 4)
                                if di >= 0:
                                    # additive -1e9 causal mask on raw scores
                                    nc.vector.tensor_add(
                                        st, st, masks_s[:, di, :]
                                    )
                                pt = p2.tile([128, QCH], bf16, tag="pt")
                                nc.scalar.activation(pt, st, EXP, scale=scale)
                                lp = None
                                if ki % 2 == 1:
                                    lp = p2l.tile([128, QCH], bf16, tag="lp")
                                    nc.gpsimd.tensor_add(lp, prev_pt, pt)
                                prev_pt = pt
                                jobs.append(
                                    make_job(
                                        hctx, kc, nkc, pt, lp, bi, h, q0,
                                        first=(ki == 0), last=(ki == nkc - 1),
                                    )
                                )
                                drain(DEFER)
                                if ki >= 3 and p3q:
                                    # one output-projection burst of the
                                    # previous q-chunk per k-chunk step
                                    p3q.popleft()()
                        for og in range(n_og):
                            p3q.append(make_og(q0, qg, og))
                drain(0)
                while p3q:
                    p3q.popleft()()

    nc.compile()
    return nc


def make_in_maps(x, Wq, Wk, Wv, Wo, b=B, s=S, dim=DIM, n_cores=N_CORES):
    import ml_dtypes

    bf16 = ml_dtypes.bfloat16
    bs = b * s
    n_din = dim // 128
    n_s1 = bs // SC1
    # x pre-shuffled so each [128, c, s-chunk] tile DMA is one contiguous
    # per-partition segment: xP[p, si, c, s'] = x[si*SC1+s', c*128+p]
    xP = np.ascontiguousarray(
        x.reshape(bs, dim).reshape(n_s1, SC1, n_din, 128).transpose(3, 0, 2, 1)
    ).astype(bf16)
    cosT1, sinT1 = _rope_tables_T(s, HD)
    cosT = np.ascontiguousarray(np.tile(cosT1, (1, b)))
    sinT = np.ascontiguousarray(np.tile(sinT1, (1, b)))
    rT = _rot_matrix_T(HD)
    ones = np.ones((HD, HD), dtype=bf16)
    masks = np.ascontiguousarray(_causal_masks(QCH).transpose(1, 0, 2)).astype(bf16)
    in_maps = []
    for c in range(n_cores):
        sl = slice(c * DLOC, (c + 1) * DLOC)
        # packed q/k/v weights: wA[p, c, iw, m] = W_iw.T[c*128+p, m]
        wA = np.ascontiguousarray(
            np.stack([Wq[sl].T, Wk[sl].T, Wv[sl].T], axis=0)
            .reshape(3, n_din, 128, DLOC)
            .transpose(2, 1, 0, 3)
        ).astype(bf16)
        woP = np.ascontiguousarray(
            Wo[:, sl].T.reshape(HPC, 128, dim).transpose(1, 0, 2)
        ).astype(bf16)
        in_maps.append(
            {
                "xP": xP,
                "wA": wA,
                "woP": woP,
                "cosT": cosT,
                "sinT": sinT,
                "rT": rT,
                "ones": ones,
                "masks": masks,
            }
        )
    return in_maps


def kernel(x, Wq, Wk, Wv, Wo, _trace=False):
    """Full-input / full-output entry point. Shards over 8 cores internally."""
    if "/opt/trn_rl_repo" not in sys.path:
        sys.path.insert(0, "/opt/trn_rl_repo")
    from concourse.bass_utils import run_bass_kernel_spmd

    x = np.asarray(x, dtype=np.float32)
    Wq, Wk, Wv, Wo = (np.asarray(w, dtype=np.float32) for w in (Wq, Wk, Wv, Wo))

    key = (B, S, DIM)
    if key not in _PROGRAM_CACHE:
        _PROGRAM_CACHE[key] = build_program(B, S, DIM)
    nc = _PROGRAM_CACHE[key]

    in_maps = make_in_maps(x, Wq, Wk, Wv, Wo)
    res = run_bass_kernel_spmd(
        nc, in_maps, core_ids=list(range(N_CORES)), trace=_trace
    )
    kernel.last_results = res

    acc = None
    for c in range(N_CORES):
        # outP[p, og, qg, j, s'] = out[og*256 + j*128 + p, qg*QCH + s']
        o = res.results[c]["outP"].astype(np.float32)
        o = o.transpose(1, 3, 0, 2, 4).reshape(DIM, B * S)
        acc = o if acc is None else acc + o
    return np.ascontiguousarray(acc.T).reshape(B, S, DIM)


# revision 28
# speedup vs baseline: 1.3615x; 1.0144x over previous
"""Multi-head causal self-attention with RoPE, tensor-parallel over heads
across 8 Trainium2 NeuronCores.

Strategy (Megatron-style TP over heads):
  - Each core owns 2 of the 16 heads: rows [c*256,(c+1)*256) of Wq/Wk/Wv
    and the matching columns of Wo.
  - On-core: qT/kT projections in transposed [d, s] layout (natural matmul
    output layout), RoPE via a signed-permutation matmul + elementwise ops,
    v in natural [s, d] layout, causal attention with scores computed
    transposed (S^T = K Q^T, softmax sum via a ones-matmul, no running max
    needed -- scores are O(10) so exp() cannot overflow), then a partial
    output projection against the core's Wo column-slice, streamed per
    512-position q-chunk so output DMA overlaps attention.
  - Host sums the 8 partial outputs (this replaces the TP all-reduce).

Schedule notes (all tuned against perfetto traces):
  - All DRAM tensors use host-pre-shuffled layouts so every DMA moves long
    contiguous per-partition segments (naive rearrange patterns shredded
    weight loads into 512B packets and cost ~30us of startup).
  - Warm-up matmuls on the `ones` tile run during the initial DMA wait so
    the PE's HAM clock-gate opens before real work arrives.
  - q/k/v results live in per-512-chunk tiles, not monolithic tensors, so
    phase 2's first score matmuls do not serialize against the LAST RoPE
    writes (tile-granular dependency tracking); the PE flows from phase 1
    into attention without going idle (and without a HAM re-throttle).
  - Attention is k-chunk-granular: score matmul -> (mask) -> exp, with the
    p@v and denominator ones-matmuls DEFERRED three k-chunks behind via a
    job queue that also drains across h/q-chunk boundaries. This keeps the
    in-order PE queue from ever waiting on the scalar engine's exp.
  - The per-q-chunk output projection is split into 8 one-OG bursts popped
    one per k-chunk of the NEXT q-chunk, so its PSUM->SBUF copies never
    flood the ACT/DVE queues ahead of exp/mask work.
  - dtypes: everything DMA'd or used as a matmul operand is bf16 (PE rate
    is identical to fp32r; DMA/SBUF halve); PSUM accumulation and the
    softmax/RoPE elementwise paths stay fp32.
"""

import sys
from collections import deque

import numpy as np

B, S, DIM = 2, 2048, 2048
NUM_HEADS = 16
HD = 128
N_CORES = 8
HPC = NUM_HEADS // N_CORES  # heads per core
DLOC = HPC * HD             # per-core slice of the model dim
ROPE_BASE = 10000.0
QCH = 512                   # attention q-chunk / phase-3 out-chunk
SC1 = 512                   # phase-1 s-chunk
N_WARM = 90                 # PE warm-up matmuls during startup DMA
DEFER = 3                   # k-chunks of pv/ones deferral behind exp

_PROGRAM_CACHE = {}


def _rope_tables_T(seq_len, head_dim):
    # match reference float32 arithmetic: inv_freq over even indices,
    # emb = cat(freqs, freqs); returned transposed [head_dim, seq_len]
    inv_freq = (
        1.0
        / (np.float32(ROPE_BASE)
           ** (np.arange(0, head_dim, 2, dtype=np.float32) / np.float32(head_dim)))
    ).astype(np.float32)
    t = np.arange(seq_len, dtype=np.float32)
    freqs = np.outer(t, inv_freq).astype(np.float32)      # [S, D/2]
    emb = np.concatenate([freqs, freqs], axis=-1)         # [S, D]
    return (
        np.ascontiguousarray(np.cos(emb).astype(np.float32).T),
        np.ascontiguousarray(np.sin(emb).astype(np.float32).T),
    )


def _rot_matrix_T(head_dim):
    # rotated = cat(-x[1::2], x[::2]) = R @ x; return R.T [D, D]
    d2 = head_dim // 2
    R = np.zeros((head_dim, head_dim), dtype=np.float32)
    for dp in range(d2):
        R[dp, 2 * dp + 1] = -1.0
    for dp in range(d2, head_dim):
        R[dp, 2 * (dp - d2)] = 1.0
    return np.ascontiguousarray(R.T)


def _causal_masks(qch):
    # masks[i][kk, qq] = 0 if 128*i + kk <= qq else -1e9 (additive, applied
    # to raw scores before exp, for the 4 diagonal k-chunks of each q-chunk)
    m = np.zeros((4, 128, qch), dtype=np.float32)
    kk = np.arange(128)[:, None]
    qq = np.arange(qch)[None, :]
    for i in range(4):
        m[i] = np.where(128 * i + kk <= qq, 0.0, -1e9).astype(np.float32)
    return m


def build_program(b=B, s=S, dim=DIM):
    """Builds the per-core SPMD Bass program (identical on every core)."""
    if "/opt/trn_rl_repo" not in sys.path:
        sys.path.insert(0, "/opt/trn_rl_repo")
    import concourse.bacc as bacc
    import concourse.mybir as mybir
    import concourse.tile as tile

    f32 = mybir.dt.float32
    f32r = mybir.dt.float32r
    bf16 = mybir.dt.bfloat16
    EXP = mybir.ActivationFunctionType.Exp

    bs = b * s
    n_din = dim // 128          # contraction chunks for projections
    n_s1 = bs // SC1            # phase-1 s-chunks
    n_qc = s // QCH             # attention q-chunks per batch
    n_sub = SC1 // 128
    n_og = dim // 256           # phase-3 256-row output groups
    scale = float(HD) ** -0.5

    nc = bacc.Bacc("TRN2", target_bir_lowering=False, debug=False)

    # host-pre-shuffled layouts: every DMA slice is contiguous per partition
    xP_d = nc.dram_tensor("xP", [128, n_s1, n_din, SC1], bf16, kind="ExternalInput")
    wA_d = nc.dram_tensor("wA", [128, n_din, 3, DLOC], bf16, kind="ExternalInput")
    woP_d = nc.dram_tensor("woP", [128, HPC, dim], bf16, kind="ExternalInput")
    cosT_d = nc.dram_tensor("cosT", [HD, bs], f32, kind="ExternalInput")
    sinT_d = nc.dram_tensor("sinT", [HD, bs], f32, kind="ExternalInput")
    rT_d = nc.dram_tensor("rT", [HD, HD], f32r, kind="ExternalInput")
    ones_d = nc.dram_tensor("ones", [HD, HD], bf16, kind="ExternalInput")
    masks_d = nc.dram_tensor("masks", [128, 4, QCH], bf16, kind="ExternalInput")
    outP_d = nc.dram_tensor(
        "outP", [128, n_og, b * n_qc, 2, QCH], bf16, kind="ExternalOutput"
    )

    with tile.TileContext(nc) as tc:
        with tc.tile_pool(name="persist", bufs=1) as persist:
            # per-512-chunk projection tiles (fine-grained deps; see header)
            qTs = [
                persist.tile([128, HPC, SC1], bf16, name=f"qT{i}")
                for i in range(n_s1)
            ]
            kTs = [
                persist.tile([128, HPC, SC1], bf16, name=f"kT{i}")
                for i in range(n_s1)
            ]
            vSs = [
                persist.tile([128, n_sub, DLOC], bf16, name=f"vS{i}")
                for i in range(n_s1)
            ]
            rTs = persist.tile([HD, HD], f32r)
            ones = persist.tile([128, 128], bf16)
            masks_s = persist.tile([128, 4, QCH], bf16)
            woT_s = persist.tile([128, HPC, dim], bf16)

            # ---------------- phase 1: qkv projections + RoPE ----------------
            with (
                tc.tile_pool(name="p1w", bufs=1) as p1w,
                tc.tile_pool(name="p1x", bufs=2) as p1x,
                tc.tile_pool(name="p1t", bufs=2) as p1t,
                tc.tile_pool(name="ps_qk", bufs=2, space="PSUM") as ps_qk,
                tc.tile_pool(name="ps_rot", bufs=2, space="PSUM") as ps_rot,
                tc.tile_pool(name="ps_v", bufs=2, space="PSUM") as ps_v,
            ):
                wA_s = p1w.tile([128, n_din, 3, DLOC], bf16)
                gw = max(1, n_din // 4)

                # startup order: ones (for warm-up) -> first weight group ->
                # first x chunk in quarters -> remaining weights -> tables.
                # masks/woT (attention-only) ride the gpsimd queue at si==1.
                nc.sync.dma_start(out=ones, in_=ones_d[:])
                xt0 = p1x.tile([128, n_din, SC1], bf16, tag="xt")
                nc.sync.dma_start(out=xt0[:, 0:4, :], in_=xP_d[:, 0, 0:4, :])
                nc.sync.dma_start(out=wA_s[:, 0:gw, :, :], in_=wA_d[:, 0:gw, :, :])
                for qq in range(1, 4):
                    nc.sync.dma_start(
                        out=xt0[:, qq * 4 : (qq + 1) * 4, :],
                        in_=xP_d[:, 0, qq * 4 : (qq + 1) * 4, :],
                    )
                for g0 in range(gw, n_din, gw):
                    nc.sync.dma_start(
                        out=wA_s[:, g0 : g0 + gw, :, :], in_=wA_d[:, g0 : g0 + gw, :, :]
                    )
                nc.sync.dma_start(out=rTs, in_=rT_d[:])
                cost0 = p1x.tile([128, SC1], f32, tag="cost")
                nc.sync.dma_start(out=cost0, in_=cosT_d[:, 0:SC1])
                sint0 = p1x.tile([128, SC1], f32, tag="sint")
                nc.sync.dma_start(out=sint0, in_=sinT_d[:, 0:SC1])

                # PE warm-up: open the HAM clock gate during the DMA wait
                warm = ps_rot.tile([128, SC1], f32, tag="rot")
                for _ in range(N_WARM):
                    nc.tensor.matmul(
                        warm[:, :128], lhsT=ones, rhs=ones, start=True, stop=True
                    )

                def issue_x(si):
                    # all x on the sync queue: the gpsimd DGE would otherwise
                    # compete for HBM bandwidth during the critical startup
                    s0 = si * SC1
                    xt = p1x.tile([128, n_din, SC1], bf16, tag="xt")
                    nh = n_din // 2
                    nc.sync.dma_start(out=xt[:, :nh, :], in_=xP_d[:, si, :nh, :])
                    nc.sync.dma_start(out=xt[:, nh:, :], in_=xP_d[:, si, nh:, :])
                    cost = p1x.tile([128, SC1], f32, tag="cost")
                    nc.sync.dma_start(out=cost, in_=cosT_d[:, s0 : s0 + SC1])
                    sint = p1x.tile([128, SC1], f32, tag="sint")
                    nc.sync.dma_start(out=sint, in_=sinT_d[:, s0 : s0 + SC1])
                    if si == 2:
                        # attention-only tensors, needed ~150us later
                        nc.gpsimd.dma_start(out=masks_s, in_=masks_d[:])
                        nc.gpsimd.dma_start(out=woT_s, in_=woP_d[:])
                    return xt, cost, sint

                # RoPE for a finished half-chunk is emitted one half-chunk
                # later so the rot-matmul never stalls the PE on the scalar
                # engine's PSUM->SBUF copy of its input
                pend = []

                def emit_ropes():
                    # two-pass: the t2 muls (the only PSUM readers) run first
                    # so the rot banks release as early as possible
                    work = []
                    while pend:
                        raw, cs, sn, dst = pend.pop(0)
                        rot = ps_rot.tile([128, SC1], f32, tag="rot")
                        nc.tensor.matmul(
                            rot, lhsT=rTs, rhs=raw, start=True, stop=True
                        )
                        work.append((raw, cs, sn, dst, rot))
                    t2s = []
                    for raw, cs, sn, dst, rot in work:
                        t2 = p1t.tile([128, SC1], f32, tag="t2")
                        nc.vector.tensor_mul(t2, rot, sn)
                        t2s.append(t2)
                    for (raw, cs, sn, dst, rot), t2 in zip(work, t2s):
                        t1 = p1t.tile([128, SC1], f32, tag="t1")
                        nc.vector.tensor_mul(t1, raw.bitcast(f32), cs)
                        nc.vector.tensor_add(dst, t1, t2)

                for si in range(n_s1):
                    xt, cost, sint = (
                        (xt0, cost0, sint0) if si == 0 else issue_x(si)
                    )
                    for h in range(HPC):
                        qacc = ps_qk.tile([128, SC1], f32, tag="qa")
                        kacc = ps_qk.tile([128, SC1], f32, tag="ka")
                        # each h-pass carries half the v sub-chunks; two subs
                        # share this 2KB PSUM bank so the accumulation group
                        # (start clears the whole bank's has_written bits)
                        # opens on the first sub and closes on the last
                        vacc = ps_v.tile([128, 2, DLOC], f32, tag="va")
                        for c in range(n_din):
                            nc.tensor.matmul(
                                qacc,
                                lhsT=wA_s[:, c, 0, h * HD : (h + 1) * HD],
                                rhs=xt[:, c, :],
                                start=(c == 0),
                                stop=(c == n_din - 1),
                            )
                            nc.tensor.matmul(
                                kacc,
                                lhsT=wA_s[:, c, 1, h * HD : (h + 1) * HD],
                                rhs=xt[:, c, :],
                                start=(c == 0),
                                stop=(c == n_din - 1),
                            )
                            for jsub in range(2):
                                sub = 2 * h + jsub
                                nc.tensor.matmul(
                                    vacc[:, jsub, :],
                                    lhsT=xt[:, c, sub * 128 : (sub + 1) * 128],
                                    rhs=wA_s[:, c, 2, :],
                                    start=(c == 0 and jsub == 0),
                                    stop=(c == n_din - 1 and jsub == 1),
                                )
                        emit_ropes()
                        rawq = p1t.tile([128, SC1], f32r, tag=f"rawq{h}")
                        nc.scalar.copy(rawq, qacc)
                        rawk = p1t.tile([128, SC1], f32r, tag=f"rawk{h}")
                        nc.scalar.copy(rawk, kacc)
                        nc.scalar.copy(vSs[si][:, 2 * h : 2 * h + 2, :], vacc)
                        pend.append((rawq, cost, sint, qTs[si][:, h, :]))
                        pend.append((rawk, cost, sint, kTs[si][:, h, :]))
                emit_ropes()

            # ------------- phases 2+3: attention + streamed output projection -------------
            with (
                tc.tile_pool(name="persistB", bufs=1) as persistB,
                tc.tile_pool(name="p2", bufs=6) as p2,
                tc.tile_pool(name="p2l", bufs=3) as p2l,
                tc.tile_pool(name="p2r", bufs=2) as p2r,
                tc.tile_pool(name="p3", bufs=3) as p3,
                # creation order fixes PSUM bank assignment: these banks
                # collide with phase-1 pools whose last readers finish at
                # different times -- put ps_st last so its slots land on the
                # banks the phase-1 tail frees earliest
                tc.tile_pool(name="ps_o", bufs=3, space="PSUM") as ps_o,
                tc.tile_pool(name="ps3", bufs=2, space="PSUM") as ps3,
                tc.tile_pool(name="ps_st", bufs=3, space="PSUM") as ps_st,
            ):
                uT = persistB.tile([128, HPC, bs], bf16)  # attn out, [d, h, b*s]
                jobs = deque()          # deferred pv/ones emissions
                p3q = deque()           # deferred output-projection OG bursts

                # bridge warm-up: keep the PE's HAM clock-gate open while the
                # phase-1 DVE/ACT tail drains (the first attention matmuls
                # wait on PSUM-bank anti-deps from that tail). Allocated from
                # ps_o so it lands on bank 0 = phase-1's qa slot 0, whose
                # last reader (the raw-q copy of the final s-chunk's first
                # head) finishes earliest of all banks.
                wscr = ps_o.tile([128, QCH], f32, tag="o", name="wscr")
                for _ in range(30):
                    nc.tensor.matmul(
                        wscr[:, :128], lhsT=ones, rhs=ones, start=True, stop=True
                    )

                def drain(keep):
                    while len(jobs) > keep:
                        jobs.popleft()()

                def make_job(hctx, kc, nkc, pt, bi, h, q0, first, last):
                    def emit():
                        if "outp" not in hctx:
                            # lrep first: with bufs=3 the next h's outp then
                            # lands on this h's lrep slot, which frees at
                            # recip() -- earlier than outp's uT-divide
                            hctx["lrep"] = ps_o.tile(
                                [128, QCH], f32, tag="o", name="lrep"
                            )
                            hctx["outp"] = ps_o.tile(
                                [128, QCH], f32, tag="o", name="outp"
                            )
                        outp, lrep = hctx["outp"], hctx["lrep"]
                        gk = bi * (s // 128) + kc
                        nc.tensor.matmul(
                            outp,
                            lhsT=vSs[gk // n_sub][
                                :, gk % n_sub, h * HD : (h + 1) * HD
                            ],
                            rhs=pt,
                            start=first,
                            stop=last,
                        )
                        # softmax denominator rides the PE per k-chunk: no
                        # cross-engine latency in the in-order matmul queue
                        nc.tensor.matmul(
                            lrep,
                            lhsT=ones,
                            rhs=pt,
                            start=first,
                            stop=last,
                        )
                        if last:
                            rec = p2r.tile([128, QCH], f32, tag="rec")
                            nc.vector.reciprocal_approx_fast(rec, lrep)
                            nc.vector.tensor_mul(
                                uT[:, h, q0 : q0 + QCH], outp, rec
                            )
                    return emit

                def make_og(q0, qg, og):
                    def emit():
                        o0 = og * 256
                        ot = p3.tile([128, 2, QCH], bf16, tag="ot")
                        for j in range(2):
                            pos = ps3.tile([128, QCH], f32, tag="op")
                            for h in range(HPC):
                                nc.tensor.matmul(
                                    pos,
                                    lhsT=woT_s[
                                        :, h, o0 + j * 128 : o0 + (j + 1) * 128
                                    ],
                                    rhs=uT[:, h, q0 : q0 + QCH],
                                    start=(h == 0),
                                    stop=(h == HPC - 1),
                                )
                            if j == 0:
                                nc.scalar.copy(ot[:, j, :], pos)
                            else:
                                nc.vector.tensor_copy(ot[:, j, :], pos)
                        nc.sync.dma_start(out=outP_d[:, og, qg, :, :], in_=ot)
                    return emit

                for bi in range(b):
                    for qc in range(n_qc):
                        q0 = bi * s + qc * QCH
                        qg = bi * n_qc + qc
                        nkc = (qc + 1) * QCH // 128
                        for h in range(HPC):
                            hctx = {}
                            # k-chunks DESCENDING: the masked diagonal chunks
                            # (whose mask->exp->pair-sum chains are longest)
                            # run first, when their deferred pv/ones pops are
                            # still far away; the final chunk's exp is
                            # mask-free so the closing recip chain is short
                            for ki in range(nkc):
                                kc = nkc - 1 - ki
                                gk = bi * (s // 128) + kc
                                st = ps_st.tile([128, QCH], f32, tag="st")
                                nc.tensor.matmul(
                                    st,
                                    lhsT=kTs[gk // n_sub][
                                        :,
                                        h,
                                        (gk % n_sub) * 128 : (gk % n_sub + 1) * 128,
                                    ],
                                    rhs=qTs[qg][:, h, :],
                                    start=True,
                                    stop=True,
                                )
                                di = kc - (nkc - 4)
                                if di >= 0:
                                    # additive -1e9 causal mask on raw scores
                                    nc.vector.tensor_add(
                                        st, st, masks_s[:, di, :]
                                    )
                                pt = p2.tile([128, QCH], bf16, tag="pt")
                                nc.scalar.activation(pt, st, EXP, scale=scale)
                                jobs.append(
                                    make_job(
                                        hctx, kc, nkc, pt, bi, h, q0,
                                        first=(ki == 0), last=(ki == nkc - 1),
                                    )
                                )
                                drain(DEFER)
                                if ki >= 3 and p3q:
                                    # one output-projection burst of the
                                    # previous q-chunk per k-chunk step
                                    p3q.popleft()()
                        for og in range(n_og):
                            p3q.append(make_og(q0, qg, og))
                drain(0)
                while p3q:
                    p3q.popleft()()

    nc.compile()
    return nc


def make_in_maps(x, Wq, Wk, Wv, Wo, b=B, s=S, dim=DIM, n_cores=N_CORES):
    import ml_dtypes

    bf16 = ml_dtypes.bfloat16
    bs = b * s
    n_din = dim // 128
    n_s1 = bs // SC1
    # x pre-shuffled so each [128, c, s-chunk] tile DMA is one contiguous
    # per-partition segment: xP[p, si, c, s'] = x[si*SC1+s', c*128+p]
    xP = np.ascontiguousarray(
        x.reshape(bs, dim).reshape(n_s1, SC1, n_din, 128).transpose(3, 0, 2, 1)
    ).astype(bf16)
    cosT1, sinT1 = _rope_tables_T(s, HD)
    cosT = np.ascontiguousarray(np.tile(cosT1, (1, b)))
    sinT = np.ascontiguousarray(np.tile(sinT1, (1, b)))
    rT = _rot_matrix_T(HD)
    ones = np.ones((HD, HD), dtype=bf16)
    masks = np.ascontiguousarray(_causal_masks(QCH).transpose(1, 0, 2)).astype(bf16)
    in_maps = []
    for c in range(n_cores):
        sl = slice(c * DLOC, (c + 1) * DLOC)
        # packed q/k/v weights: wA[p, c, iw, m] = W_iw.T[c*128+p, m]
        wA = np.ascontiguousarray(
            np.stack([Wq[sl].T, Wk[sl].T, Wv[sl].T], axis=0)
            .reshape(3, n_din, 128, DLOC)
            .transpose(2, 1, 0, 3)
        ).astype(bf16)
        woP = np.ascontiguousarray(
            Wo[:, sl].T.reshape(HPC, 128, dim).transpose(1, 0, 2)
        ).astype(bf16)
        in_maps.append(
            {
                "xP": xP,
                "wA": wA,
                "woP": woP,
                "cosT": cosT,
                "sinT": sinT,
                "rT": rT,
                "ones": ones,
                "masks": masks,
            }
        )
    return in_maps


def kernel(x, Wq, Wk, Wv, Wo, _trace=False):
    """Full-input / full-output entry point. Shards over 8 cores internally."""
    if "/opt/trn_rl_repo" not in sys.path:
        sys.path.insert(0, "/opt/trn_rl_repo")
    from concourse.bass_utils import run_bass_kernel_spmd

    x = np.asarray(x, dtype=np.float32)
    Wq, Wk, Wv, Wo = (np.asarray(w, dtype=np.float32) for w in (Wq, Wk, Wv, Wo))

    key = (B, S, DIM)
    if key not in _PROGRAM_CACHE:
        _PROGRAM_CACHE[key] = build_program(B, S, DIM)
    nc = _PROGRAM_CACHE[key]

    in_maps = make_in_maps(x, Wq, Wk, Wv, Wo)
    res = run_bass_kernel_spmd(
        nc, in_maps, core_ids=list(range(N_CORES)), trace=_trace
    )
    kernel.last_results = res

    acc = None
    for c in range(N_CORES):
        # outP[p, og, qg, j, s'] = out[og*256 + j*128 + p, qg*QCH + s']
        o = res.results[c]["outP"].astype(np.float32)
        o = o.transpose(1, 3, 0, 2, 4).reshape(DIM, B * S)
        acc = o if acc is None else acc + o
    return np.ascontiguousarray(acc.T).reshape(B, S, DIM)


# revision 29
# speedup vs baseline: 1.3661x; 1.0034x over previous
"""Multi-head causal self-attention with RoPE, tensor-parallel over heads
across 8 Trainium2 NeuronCores.

Strategy (Megatron-style TP over heads):
  - Each core owns 2 of the 16 heads: rows [c*256,(c+1)*256) of Wq/Wk/Wv
    and the matching columns of Wo.
  - On-core: qT/kT projections in transposed [d, s] layout (natural matmul
    output layout), RoPE via a signed-permutation matmul + elementwise ops,
    v in natural [s, d] layout, causal attention with scores computed
    transposed (S^T = K Q^T, softmax sum via a ones-matmul, no running max
    needed -- scores are O(10) so exp() cannot overflow), then a partial
    output projection against the core's Wo column-slice, streamed per
    512-position q-chunk so output DMA overlaps attention.
  - Host sums the 8 partial outputs (this replaces the TP all-reduce).

Schedule notes (all tuned against perfetto traces):
  - All DRAM tensors use host-pre-shuffled layouts so every DMA moves long
    contiguous per-partition segments (naive rearrange patterns shredded
    weight loads into 512B packets and cost ~30us of startup).
  - Warm-up matmuls on the `ones` tile run during the initial DMA wait so
    the PE's HAM clock-gate opens before real work arrives.
  - q/k/v results live in per-512-chunk tiles, not monolithic tensors, so
    phase 2's first score matmuls do not serialize against the LAST RoPE
    writes (tile-granular dependency tracking); the PE flows from phase 1
    into attention without going idle (and without a HAM re-throttle).
  - Attention is k-chunk-granular: score matmul -> (mask) -> exp, with the
    p@v and denominator ones-matmuls DEFERRED three k-chunks behind via a
    job queue that also drains across h/q-chunk boundaries. This keeps the
    in-order PE queue from ever waiting on the scalar engine's exp.
  - The per-q-chunk output projection is split into 8 one-OG bursts popped
    one per k-chunk of the NEXT q-chunk, so its PSUM->SBUF copies never
    flood the ACT/DVE queues ahead of exp/mask work.
  - dtypes: everything DMA'd or used as a matmul operand is bf16 (PE rate
    is identical to fp32r; DMA/SBUF halve); PSUM accumulation and the
    softmax/RoPE elementwise paths stay fp32.
"""

import sys
from collections import deque

import numpy as np

B, S, DIM = 2, 2048, 2048
NUM_HEADS = 16
HD = 128
N_CORES = 8
HPC = NUM_HEADS // N_CORES  # heads per core
DLOC = HPC * HD             # per-core slice of the model dim
ROPE_BASE = 10000.0
QCH = 512                   # attention q-chunk / phase-3 out-chunk
SC1 = 512                   # phase-1 s-chunk
N_WARM = 90                 # PE warm-up matmuls during startup DMA
DEFER = 4                   # k-chunks of pv/ones deferral behind exp

_PROGRAM_CACHE = {}


def _rope_tables_T(seq_len, head_dim):
    # match reference float32 arithmetic: inv_freq over even indices,
    # emb = cat(freqs, freqs); returned transposed [head_dim, seq_len]
    inv_freq = (
        1.0
        / (np.float32(ROPE_BASE)
           ** (np.arange(0, head_dim, 2, dtype=np.float32) / np.float32(head_dim)))
    ).astype(np.float32)
    t = np.arange(seq_len, dtype=np.float32)
    freqs = np.outer(t, inv_freq).astype(np.float32)      # [S, D/2]
    emb = np.concatenate([freqs, freqs], axis=-1)         # [S, D]
    return (
        np.ascontiguousarray(np.cos(emb).astype(np.float32).T),
        np.ascontiguousarray(np.sin(emb).astype(np.float32).T),
    )


def _rot_matrix_T(head_dim):
    # rotated = cat(-x[1::2], x[::2]) = R @ x; return R.T [D, D]
    d2 = head_dim // 2
    R = np.zeros((head_dim, head_dim), dtype=np.float32)
    for dp in range(d2):
        R[dp, 2 * dp + 1] = -1.0
    for dp in range(d2, head_dim):
        R[dp, 2 * (dp - d2)] = 1.0
    return np.ascontiguousarray(R.T)


def _causal_masks(qch):
    # masks[i][kk, qq] = 0 if 128*i + kk <= qq else -1e9 (additive, applied
    # to raw scores before exp, for the 4 diagonal k-chunks of each q-chunk)
    m = np.zeros((4, 128, qch), dtype=np.float32)
    kk = np.arange(128)[:, None]
    qq = np.arange(qch)[None, :]
    for i in range(4):
        m[i] = np.where(128 * i + kk <= qq, 0.0, -1e9).astype(np.float32)
    return m


def build_program(b=B, s=S, dim=DIM):
    """Builds the per-core SPMD Bass program (identical on every core)."""
    if "/opt/trn_rl_repo" not in sys.path:
        sys.path.insert(0, "/opt/trn_rl_repo")
    import concourse.bacc as bacc
    import concourse.mybir as mybir
    import concourse.tile as tile

    f32 = mybir.dt.float32
    f32r = mybir.dt.float32r
    bf16 = mybir.dt.bfloat16
    EXP = mybir.ActivationFunctionType.Exp

    bs = b * s
    n_din = dim // 128          # contraction chunks for projections
    n_s1 = bs // SC1            # phase-1 s-chunks
    n_qc = s // QCH             # attention q-chunks per batch
    n_sub = SC1 // 128
    n_og = dim // 256           # phase-3 256-row output groups
    scale = float(HD) ** -0.5

    nc = bacc.Bacc("TRN2", target_bir_lowering=False, debug=False)

    # host-pre-shuffled layouts: every DMA slice is contiguous per partition
    xP_d = nc.dram_tensor("xP", [128, n_s1, n_din, SC1], bf16, kind="ExternalInput")
    wA_d = nc.dram_tensor("wA", [128, n_din, 3, DLOC], bf16, kind="ExternalInput")
    woP_d = nc.dram_tensor("woP", [128, HPC, dim], bf16, kind="ExternalInput")
    cosT_d = nc.dram_tensor("cosT", [HD, bs], f32, kind="ExternalInput")
    sinT_d = nc.dram_tensor("sinT", [HD, bs], f32, kind="ExternalInput")
    rT_d = nc.dram_tensor("rT", [HD, HD], f32r, kind="ExternalInput")
    ones_d = nc.dram_tensor("ones", [HD, HD], bf16, kind="ExternalInput")
    masks_d = nc.dram_tensor("masks", [128, 4, QCH], bf16, kind="ExternalInput")
    outP_d = nc.dram_tensor(
        "outP", [128, n_og, b * n_qc, 2, QCH], bf16, kind="ExternalOutput"
    )

    with tile.TileContext(nc) as tc:
        with tc.tile_pool(name="persist", bufs=1) as persist:
            # per-512-chunk projection tiles (fine-grained deps; see header)
            qTs = [
                persist.tile([128, HPC, SC1], bf16, name=f"qT{i}")
                for i in range(n_s1)
            ]
            kTs = [
                persist.tile([128, HPC, SC1], bf16, name=f"kT{i}")
                for i in range(n_s1)
            ]
            vSs = [
                persist.tile([128, n_sub, DLOC], bf16, name=f"vS{i}")
                for i in range(n_s1)
            ]
            rTs = persist.tile([HD, HD], f32r)
            ones = persist.tile([128, 128], bf16)
            masks_s = persist.tile([128, 4, QCH], bf16)
            woT_s = persist.tile([128, HPC, dim], bf16)

            # ---------------- phase 1: qkv projections + RoPE ----------------
            with (
                tc.tile_pool(name="p1w", bufs=1) as p1w,
                tc.tile_pool(name="p1x", bufs=2) as p1x,
                tc.tile_pool(name="p1t", bufs=2) as p1t,
                tc.tile_pool(name="ps_qk", bufs=2, space="PSUM") as ps_qk,
                tc.tile_pool(name="ps_rot", bufs=2, space="PSUM") as ps_rot,
                tc.tile_pool(name="ps_v", bufs=2, space="PSUM") as ps_v,
            ):
                wA_s = p1w.tile([128, n_din, 3, DLOC], bf16)
                gw = max(1, n_din // 4)

                # startup order: ones (for warm-up) -> first weight group ->
                # first x chunk in quarters -> remaining weights -> tables.
                # masks/woT (attention-only) ride the gpsimd queue at si==1.
                nc.sync.dma_start(out=ones, in_=ones_d[:])
                xt0 = p1x.tile([128, n_din, SC1], bf16, tag="xt")
                nc.sync.dma_start(out=xt0[:, 0:4, :], in_=xP_d[:, 0, 0:4, :])
                nc.sync.dma_start(out=wA_s[:, 0:gw, :, :], in_=wA_d[:, 0:gw, :, :])
                for qq in range(1, 4):
                    nc.sync.dma_start(
                        out=xt0[:, qq * 4 : (qq + 1) * 4, :],
                        in_=xP_d[:, 0, qq * 4 : (qq + 1) * 4, :],
                    )
                for g0 in range(gw, n_din, gw):
                    nc.sync.dma_start(
                        out=wA_s[:, g0 : g0 + gw, :, :], in_=wA_d[:, g0 : g0 + gw, :, :]
                    )
                nc.sync.dma_start(out=rTs, in_=rT_d[:])
                cost0 = p1x.tile([128, SC1], f32, tag="cost")
                nc.sync.dma_start(out=cost0, in_=cosT_d[:, 0:SC1])
                sint0 = p1x.tile([128, SC1], f32, tag="sint")
                nc.sync.dma_start(out=sint0, in_=sinT_d[:, 0:SC1])

                # PE warm-up: open the HAM clock gate during the DMA wait
                warm = ps_rot.tile([128, SC1], f32, tag="rot")
                for _ in range(N_WARM):
                    nc.tensor.matmul(
                        warm[:, :128], lhsT=ones, rhs=ones, start=True, stop=True
                    )

                def issue_x(si):
                    # all x on the sync queue: the gpsimd DGE would otherwise
                    # compete for HBM bandwidth during the critical startup
                    s0 = si * SC1
                    xt = p1x.tile([128, n_din, SC1], bf16, tag="xt")
                    nh = n_din // 2
                    nc.sync.dma_start(out=xt[:, :nh, :], in_=xP_d[:, si, :nh, :])
                    nc.sync.dma_start(out=xt[:, nh:, :], in_=xP_d[:, si, nh:, :])
                    cost = p1x.tile([128, SC1], f32, tag="cost")
                    nc.sync.dma_start(out=cost, in_=cosT_d[:, s0 : s0 + SC1])
                    sint = p1x.tile([128, SC1], f32, tag="sint")
                    nc.sync.dma_start(out=sint, in_=sinT_d[:, s0 : s0 + SC1])
                    if si == 2:
                        # attention-only tensors, needed ~150us later
                        nc.gpsimd.dma_start(out=masks_s, in_=masks_d[:])
                        nc.gpsimd.dma_start(out=woT_s, in_=woP_d[:])
                    return xt, cost, sint

                # RoPE for a finished half-chunk is emitted one half-chunk
                # later so the rot-matmul never stalls the PE on the scalar
                # engine's PSUM->SBUF copy of its input
                pend = []

                def emit_ropes():
                    # two-pass: the t2 muls (the only PSUM readers) run first
                    # so the rot banks release as early as possible
                    work = []
                    while pend:
                        raw, cs, sn, dst = pend.pop(0)
                        rot = ps_rot.tile([128, SC1], f32, tag="rot")
                        nc.tensor.matmul(
                            rot, lhsT=rTs, rhs=raw, start=True, stop=True
                        )
                        work.append((raw, cs, sn, dst, rot))
                    t2s = []
                    for raw, cs, sn, dst, rot in work:
                        t2 = p1t.tile([128, SC1], f32, tag="t2")
                        nc.vector.tensor_mul(t2, rot, sn)
                        t2s.append(t2)
                    for (raw, cs, sn, dst, rot), t2 in zip(work, t2s):
                        t1 = p1t.tile([128, SC1], f32, tag="t1")
                        nc.vector.tensor_mul(t1, raw.bitcast(f32), cs)
                        nc.vector.tensor_add(dst, t1, t2)

                for si in range(n_s1):
                    xt, cost, sint = (
                        (xt0, cost0, sint0) if si == 0 else issue_x(si)
                    )
                    for h in range(HPC):
                        qacc = ps_qk.tile([128, SC1], f32, tag="qa")
                        kacc = ps_qk.tile([128, SC1], f32, tag="ka")
                        # each h-pass carries half the v sub-chunks; two subs
                        # share this 2KB PSUM bank so the accumulation group
                        # (start clears the whole bank's has_written bits)
                        # opens on the first sub and closes on the last
                        vacc = ps_v.tile([128, 2, DLOC], f32, tag="va")
                        for c in range(n_din):
                            nc.tensor.matmul(
                                qacc,
                                lhsT=wA_s[:, c, 0, h * HD : (h + 1) * HD],
                                rhs=xt[:, c, :],
                                start=(c == 0),
                                stop=(c == n_din - 1),
                            )
                            nc.tensor.matmul(
                                kacc,
                                lhsT=wA_s[:, c, 1, h * HD : (h + 1) * HD],
                                rhs=xt[:, c, :],
                                start=(c == 0),
                                stop=(c == n_din - 1),
                            )
                            for jsub in range(2):
                                sub = 2 * h + jsub
                                nc.tensor.matmul(
                                    vacc[:, jsub, :],
                                    lhsT=xt[:, c, sub * 128 : (sub + 1) * 128],
                                    rhs=wA_s[:, c, 2, :],
                                    start=(c == 0 and jsub == 0),
                                    stop=(c == n_din - 1 and jsub == 1),
                                )
                        emit_ropes()
                        nc.scalar.copy(vSs[si][:, 2 * h : 2 * h + 2, :], vacc)
                        rawq = p1t.tile([128, SC1], f32r, tag=f"rawq{h}")
                        nc.scalar.copy(rawq, qacc)
                        rawk = p1t.tile([128, SC1], f32r, tag=f"rawk{h}")
                        nc.scalar.copy(rawk, kacc)
                        pend.append((rawq, cost, sint, qTs[si][:, h, :]))
                        pend.append((rawk, cost, sint, kTs[si][:, h, :]))
                emit_ropes()

            # ------------- phases 2+3: attention + streamed output projection -------------
            with (
                tc.tile_pool(name="persistB", bufs=1) as persistB,
                tc.tile_pool(name="p2", bufs=7) as p2,
                tc.tile_pool(name="p2l", bufs=3) as p2l,
                tc.tile_pool(name="p2r", bufs=2) as p2r,
                tc.tile_pool(name="p3", bufs=3) as p3,
                # creation order fixes PSUM bank assignment: these banks
                # collide with phase-1 pools whose last readers finish at
                # different times -- put ps_st last so its slots land on the
                # banks the phase-1 tail frees earliest
                tc.tile_pool(name="ps_o", bufs=3, space="PSUM") as ps_o,
                tc.tile_pool(name="ps3", bufs=2, space="PSUM") as ps3,
                tc.tile_pool(name="ps_st", bufs=3, space="PSUM") as ps_st,
            ):
                uT = persistB.tile([128, HPC, bs], bf16)  # attn out, [d, h, b*s]
                jobs = deque()          # deferred pv/ones emissions
                p3q = deque()           # deferred output-projection OG bursts

                # bridge warm-up: keep the PE's HAM clock-gate open while the
                # phase-1 DVE/ACT tail drains (the first attention matmuls
                # wait on PSUM-bank anti-deps from that tail). Allocated from
                # ps_o so it lands on bank 0 = phase-1's qa slot 0, whose
                # last reader (the raw-q copy of the final s-chunk's first
                # head) finishes earliest of all banks.
                wscr = ps_o.tile([128, QCH], f32, tag="o", name="wscr")
                for _ in range(60):
                    nc.tensor.matmul(
                        wscr[:, :128], lhsT=ones, rhs=ones, start=True, stop=True
                    )

                def drain(keep):
                    while len(jobs) > keep:
                        jobs.popleft()()

                def make_job(hctx, kc, nkc, pt, bi, h, q0, first, last):
                    def emit():
                        if "outp" not in hctx:
                            # lrep first: with bufs=3 the next h's outp then
                            # lands on this h's lrep slot, which frees at
                            # recip() -- earlier than outp's uT-divide
                            hctx["lrep"] = ps_o.tile(
                                [128, QCH], f32, tag="o", name="lrep"
                            )
                            hctx["outp"] = ps_o.tile(
                                [128, QCH], f32, tag="o", name="outp"
                            )
                        outp, lrep = hctx["outp"], hctx["lrep"]
                        gk = bi * (s // 128) + kc
                        nc.tensor.matmul(
                            outp,
                            lhsT=vSs[gk // n_sub][
                                :, gk % n_sub, h * HD : (h + 1) * HD
                            ],
                            rhs=pt,
                            start=first,
                            stop=last,
                        )
                        # softmax denominator rides the PE per k-chunk: no
                        # cross-engine latency in the in-order matmul queue
                        nc.tensor.matmul(
                            lrep,
                            lhsT=ones,
                            rhs=pt,
                            start=first,
                            stop=last,
                        )
                        if last:
                            rec = p2r.tile([128, QCH], f32, tag="rec")
                            nc.vector.reciprocal_approx_fast(rec, lrep)
                            nc.vector.tensor_mul(
                                uT[:, h, q0 : q0 + QCH], outp, rec
                            )
                    return emit

                def make_og(q0, qg, og):
                    def emit():
                        o0 = og * 256
                        ot = p3.tile([128, 2, QCH], bf16, tag="ot")
                        for j in range(2):
                            pos = ps3.tile([128, QCH], f32, tag="op")
                            for h in range(HPC):
                                nc.tensor.matmul(
                                    pos,
                                    lhsT=woT_s[
                                        :, h, o0 + j * 128 : o0 + (j + 1) * 128
                                    ],
                                    rhs=uT[:, h, q0 : q0 + QCH],
                                    start=(h == 0),
                                    stop=(h == HPC - 1),
                                )
                            if j == 0 and og % 2 == 0:
                                nc.scalar.copy(ot[:, j, :], pos)
                            else:
                                nc.vector.tensor_copy(ot[:, j, :], pos)
                        nc.sync.dma_start(out=outP_d[:, og, qg, :, :], in_=ot)
                    return emit

                for bi in range(b):
                    for qc in range(n_qc):
                        q0 = bi * s + qc * QCH
                        qg = bi * n_qc + qc
                        nkc = (qc + 1) * QCH // 128
                        for h in range(HPC):
                            hctx = {}
                            # k-chunks DESCENDING: the masked diagonal chunks
                            # (whose mask->exp->pair-sum chains are longest)
                            # run first, when their deferred pv/ones pops are
                            # still far away; the final chunk's exp is
                            # mask-free so the closing recip chain is short
                            for ki in range(nkc):
                                kc = nkc - 1 - ki
                                gk = bi * (s // 128) + kc
                                st = ps_st.tile([128, QCH], f32, tag="st")
                                nc.tensor.matmul(
                                    st,
                                    lhsT=kTs[gk // n_sub][
                                        :,
                                        h,
                                        (gk % n_sub) * 128 : (gk % n_sub + 1) * 128,
                                    ],
                                    rhs=qTs[qg][:, h, :],
                                    start=True,
                                    stop=True,
                                )
                                di = kc - (nkc - 4)
                                if di >= 0:
                                    # additive -1e9 causal mask on raw scores
                                    nc.vector.tensor_add(
                                        st, st, masks_s[:, di, :]
                                    )
                                pt = p2.tile([128, QCH], bf16, tag="pt")
                                nc.scalar.activation(pt, st, EXP, scale=scale)
                                jobs.append(
                                    make_job(
                                        hctx, kc, nkc, pt, bi, h, q0,
                                        first=(ki == 0), last=(ki == nkc - 1),
                                    )
                                )
                                drain(DEFER)
                                if ki >= 3 and p3q:
                                    # one output-projection burst of the
                                    # previous q-chunk per k-chunk step
                                    p3q.popleft()()
                        for og in range(n_og):
                            p3q.append(make_og(q0, qg, og))
                drain(0)
                while p3q:
                    p3q.popleft()()

    nc.compile()
    return nc


def make_in_maps(x, Wq, Wk, Wv, Wo, b=B, s=S, dim=DIM, n_cores=N_CORES):
    import ml_dtypes

    bf16 = ml_dtypes.bfloat16
    bs = b * s
    n_din = dim // 128
    n_s1 = bs // SC1
    # x pre-shuffled so each [128, c, s-chunk] tile DMA is one contiguous
    # per-partition segment: xP[p, si, c, s'] = x[si*SC1+s', c*128+p]
    xP = np.ascontiguousarray(
        x.reshape(bs, dim).reshape(n_s1, SC1, n_din, 128).transpose(3, 0, 2, 1)
    ).astype(bf16)
    cosT1, sinT1 = _rope_tables_T(s, HD)
    cosT = np.ascontiguousarray(np.tile(cosT1, (1, b)))
    sinT = np.ascontiguousarray(np.tile(sinT1, (1, b)))
    rT = _rot_matrix_T(HD)
    ones = np.ones((HD, HD), dtype=bf16)
    masks = np.ascontiguousarray(_causal_masks(QCH).transpose(1, 0, 2)).astype(bf16)
    in_maps = []
    for c in range(n_cores):
        sl = slice(c * DLOC, (c + 1) * DLOC)
        # packed q/k/v weights: wA[p, c, iw, m] = W_iw.T[c*128+p, m]
        wA = np.ascontiguousarray(
            np.stack([Wq[sl].T, Wk[sl].T, Wv[sl].T], axis=0)
            .reshape(3, n_din, 128, DLOC)
            .transpose(2, 1, 0, 3)
        ).astype(bf16)
        woP = np.ascontiguousarray(
            Wo[:, sl].T.reshape(HPC, 128, dim).transpose(1, 0, 2)
        ).astype(bf16)
        in_maps.append(
            {
                "xP": xP,
                "wA": wA,
                "woP": woP,
                "cosT": cosT,
                "sinT": sinT,
                "rT": rT,
                "ones": ones,
                "masks": masks,
            }
        )
    return in_maps


def kernel(x, Wq, Wk, Wv, Wo, _trace=False):
    """Full-input / full-output entry point. Shards over 8 cores internally."""
    if "/opt/trn_rl_repo" not in sys.path:
        sys.path.insert(0, "/opt/trn_rl_repo")
    from concourse.bass_utils import run_bass_kernel_spmd

    x = np.asarray(x, dtype=np.float32)
    Wq, Wk, Wv, Wo = (np.asarray(w, dtype=np.float32) for w in (Wq, Wk, Wv, Wo))

    key = (B, S, DIM)
    if key not in _PROGRAM_CACHE:
        _PROGRAM_CACHE[key] = build_program(B, S, DIM)
    nc = _PROGRAM_CACHE[key]

    in_maps = make_in_maps(x, Wq, Wk, Wv, Wo)
    res = run_bass_kernel_spmd(
        nc, in_maps, core_ids=list(range(N_CORES)), trace=_trace
    )
    kernel.last_results = res

    acc = None
    for c in range(N_CORES):
        # outP[p, og, qg, j, s'] = out[og*256 + j*128 + p, qg*QCH + s']
        o = res.results[c]["outP"].astype(np.float32)
        o = o.transpose(1, 3, 0, 2, 4).reshape(DIM, B * S)
        acc = o if acc is None else acc + o
    return np.ascontiguousarray(acc.T).reshape(B, S, DIM)


# revision 32
# speedup vs baseline: 1.3664x; 1.0002x over previous
"""Multi-head causal self-attention with RoPE, tensor-parallel over heads
across 8 Trainium2 NeuronCores.

Strategy (Megatron-style TP over heads):
  - Each core owns 2 of the 16 heads: rows [c*256,(c+1)*256) of Wq/Wk/Wv
    and the matching columns of Wo.
  - On-core: qT/kT projections in transposed [d, s] layout (natural matmul
    output layout), RoPE via a signed-permutation matmul + elementwise ops,
    v in natural [s, d] layout, causal attention with scores computed
    transposed (S^T = K Q^T, softmax sum via a ones-matmul, no running max
    needed -- scores are O(10) so exp() cannot overflow), then a partial
    output projection against the core's Wo column-slice, streamed per
    512-position q-chunk so output DMA overlaps attention.
  - Host sums the 8 partial outputs (this replaces the TP all-reduce).

Schedule notes (all tuned against perfetto traces):
  - All DRAM tensors use host-pre-shuffled layouts so every DMA moves long
    contiguous per-partition segments (naive rearrange patterns shredded
    weight loads into 512B packets and cost ~30us of startup).
  - Warm-up matmuls on the `ones` tile run during the initial DMA wait so
    the PE's HAM clock-gate opens before real work arrives.
  - q/k/v results live in per-512-chunk tiles, not monolithic tensors, so
    phase 2's first score matmuls do not serialize against the LAST RoPE
    writes (tile-granular dependency tracking); the PE flows from phase 1
    into attention without going idle (and without a HAM re-throttle).
  - Attention is k-chunk-granular: score matmul -> (mask) -> exp, with the
    p@v and denominator ones-matmuls DEFERRED three k-chunks behind via a
    job queue that also drains across h/q-chunk boundaries. This keeps the
    in-order PE queue from ever waiting on the scalar engine's exp.
  - The per-q-chunk output projection is split into 8 one-OG bursts popped
    one per k-chunk of the NEXT q-chunk, so its PSUM->SBUF copies never
    flood the ACT/DVE queues ahead of exp/mask work.
  - dtypes: everything DMA'd or used as a matmul operand is bf16 (PE rate
    is identical to fp32r; DMA/SBUF halve); PSUM accumulation and the
    softmax/RoPE elementwise paths stay fp32.
"""

import sys
from collections import deque

import numpy as np

B, S, DIM = 2, 2048, 2048
NUM_HEADS = 16
HD = 128
N_CORES = 8
HPC = NUM_HEADS // N_CORES  # heads per core
DLOC = HPC * HD             # per-core slice of the model dim
ROPE_BASE = 10000.0
QCH = 512                   # attention q-chunk / phase-3 out-chunk
SC1 = 512                   # phase-1 s-chunk
N_WARM = 40                 # PE warm-up matmuls during startup DMA
DEFER = 4                   # k-chunks of pv/ones deferral behind exp

_PROGRAM_CACHE = {}


def _rope_tables_T(seq_len, head_dim):
    # match reference float32 arithmetic: inv_freq over even indices,
    # emb = cat(freqs, freqs); returned transposed [head_dim, seq_len]
    inv_freq = (
        1.0
        / (np.float32(ROPE_BASE)
           ** (np.arange(0, head_dim, 2, dtype=np.float32) / np.float32(head_dim)))
    ).astype(np.float32)
    t = np.arange(seq_len, dtype=np.float32)
    freqs = np.outer(t, inv_freq).astype(np.float32)      # [S, D/2]
    emb = np.concatenate([freqs, freqs], axis=-1)         # [S, D]
    return (
        np.ascontiguousarray(np.cos(emb).astype(np.float32).T),
        np.ascontiguousarray(np.sin(emb).astype(np.float32).T),
    )


def _rot_matrix_T(head_dim):
    # rotated = cat(-x[1::2], x[::2]) = R @ x; return R.T [D, D]
    d2 = head_dim // 2
    R = np.zeros((head_dim, head_dim), dtype=np.float32)
    for dp in range(d2):
        R[dp, 2 * dp + 1] = -1.0
    for dp in range(d2, head_dim):
        R[dp, 2 * (dp - d2)] = 1.0
    return np.ascontiguousarray(R.T)


def _causal_masks(qch):
    # masks[i][kk, qq] = 0 if 128*i + kk <= qq else -1e9 (additive, applied
    # to raw scores before exp, for the 4 diagonal k-chunks of each q-chunk)
    m = np.zeros((4, 128, qch), dtype=np.float32)
    kk = np.arange(128)[:, None]
    qq = np.arange(qch)[None, :]
    for i in range(4):
        m[i] = np.where(128 * i + kk <= qq, 0.0, -1e9).astype(np.float32)
    return m


def build_program(b=B, s=S, dim=DIM):
    """Builds the per-core SPMD Bass program (identical on every core)."""
    if "/opt/trn_rl_repo" not in sys.path:
        sys.path.insert(0, "/opt/trn_rl_repo")
    import concourse.bacc as bacc
    import concourse.mybir as mybir
    import concourse.tile as tile

    f32 = mybir.dt.float32
    f32r = mybir.dt.float32r
    bf16 = mybir.dt.bfloat16
    EXP = mybir.ActivationFunctionType.Exp

    bs = b * s
    n_din = dim // 128          # contraction chunks for projections
    n_s1 = bs // SC1            # phase-1 s-chunks
    n_qc = s // QCH             # attention q-chunks per batch
    n_sub = SC1 // 128
    n_og = dim // 256           # phase-3 256-row output groups
    scale = float(HD) ** -0.5

    nc = bacc.Bacc("TRN2", target_bir_lowering=False, debug=False)

    # host-pre-shuffled layouts: every DMA slice is contiguous per partition
    xP_d = nc.dram_tensor("xP", [128, n_s1, n_din, SC1], bf16, kind="ExternalInput")
    wA_d = nc.dram_tensor("wA", [128, n_din, 3, DLOC], bf16, kind="ExternalInput")
    woP_d = nc.dram_tensor("woP", [128, HPC, dim], bf16, kind="ExternalInput")
    cosT_d = nc.dram_tensor("cosT", [HD, bs], f32, kind="ExternalInput")
    sinT_d = nc.dram_tensor("sinT", [HD, bs], f32, kind="ExternalInput")
    rT_d = nc.dram_tensor("rT", [HD, HD], f32r, kind="ExternalInput")
    ones_d = nc.dram_tensor("ones", [HD, HD], bf16, kind="ExternalInput")
    masks_d = nc.dram_tensor("masks", [128, 4, QCH], bf16, kind="ExternalInput")
    outP_d = nc.dram_tensor(
        "outP", [128, n_og, b * n_qc, 2, QCH], bf16, kind="ExternalOutput"
    )

    with tile.TileContext(nc) as tc:
        with tc.tile_pool(name="persist", bufs=1) as persist:
            # per-512-chunk projection tiles (fine-grained deps; see header)
            qTs = [
                persist.tile([128, HPC, SC1], bf16, name=f"qT{i}")
                for i in range(n_s1)
            ]
            kTs = [
                persist.tile([128, HPC, SC1], bf16, name=f"kT{i}")
                for i in range(n_s1)
            ]
            vSs = [
                persist.tile([128, n_sub, DLOC], bf16, name=f"vS{i}")
                for i in range(n_s1)
            ]
            rTs = persist.tile([HD, HD], f32r)
            ones = persist.tile([128, 128], bf16)
            masks_s = persist.tile([128, 4, QCH], bf16)
            woT_s = persist.tile([128, HPC, dim], bf16)

            # ---------------- phase 1: qkv projections + RoPE ----------------
            with (
                tc.tile_pool(name="p1w", bufs=1) as p1w,
                tc.tile_pool(name="p1x", bufs=2) as p1x,
                tc.tile_pool(name="p1t", bufs=2) as p1t,
                tc.tile_pool(name="ps_qk", bufs=2, space="PSUM") as ps_qk,
                tc.tile_pool(name="ps_rot", bufs=2, space="PSUM") as ps_rot,
                tc.tile_pool(name="ps_v", bufs=2, space="PSUM") as ps_v,
            ):
                wA_s = p1w.tile([128, n_din, 3, DLOC], bf16)
                gw = max(1, n_din // 4)

                # startup order: ones (for warm-up) -> first weight group ->
                # first x chunk in quarters -> remaining weights -> tables.
                # masks/woT (attention-only) ride the gpsimd queue at si==1.
                nc.sync.dma_start(out=ones, in_=ones_d[:])
                xt0 = p1x.tile([128, n_din, SC1], bf16, tag="xt")
                # first x half rides the (otherwise empty) gpsimd DGE queue,
                # in parallel with the sync queue's weight stream
                nc.gpsimd.dma_start(out=xt0[:, 0:4, :], in_=xP_d[:, 0, 0:4, :])
                nc.gpsimd.dma_start(out=xt0[:, 4:8, :], in_=xP_d[:, 0, 4:8, :])
                nc.sync.dma_start(out=wA_s[:, 0:gw, :, :], in_=wA_d[:, 0:gw, :, :])
                nc.sync.dma_start(out=xt0[:, 8:12, :], in_=xP_d[:, 0, 8:12, :])
                nc.sync.dma_start(out=xt0[:, 12:16, :], in_=xP_d[:, 0, 12:16, :])
                for g0 in range(gw, n_din, gw):
                    nc.sync.dma_start(
                        out=wA_s[:, g0 : g0 + gw, :, :], in_=wA_d[:, g0 : g0 + gw, :, :]
                    )
                nc.sync.dma_start(out=rTs, in_=rT_d[:])
                cost0 = p1x.tile([128, SC1], f32, tag="cost")
                nc.sync.dma_start(out=cost0, in_=cosT_d[:, 0:SC1])
                sint0 = p1x.tile([128, SC1], f32, tag="sint")
                nc.sync.dma_start(out=sint0, in_=sinT_d[:, 0:SC1])

                # PE warm-up: open the HAM clock gate during the DMA wait
                warm = ps_rot.tile([128, SC1], f32, tag="rot")
                for _ in range(N_WARM):
                    nc.tensor.matmul(
                        warm[:, :128], lhsT=ones, rhs=ones, start=True, stop=True
                    )

                def issue_x(si):
                    # all x on the sync queue: the gpsimd DGE would otherwise
                    # compete for HBM bandwidth during the critical startup
                    s0 = si * SC1
                    xt = p1x.tile([128, n_din, SC1], bf16, tag="xt")
                    nh = n_din // 2
                    nc.sync.dma_start(out=xt[:, :nh, :], in_=xP_d[:, si, :nh, :])
                    nc.sync.dma_start(out=xt[:, nh:, :], in_=xP_d[:, si, nh:, :])
                    cost = p1x.tile([128, SC1], f32, tag="cost")
                    nc.sync.dma_start(out=cost, in_=cosT_d[:, s0 : s0 + SC1])
                    sint = p1x.tile([128, SC1], f32, tag="sint")
                    nc.sync.dma_start(out=sint, in_=sinT_d[:, s0 : s0 + SC1])
                    if si == 2:
                        # attention-only tensors, needed ~150us later
                        nc.gpsimd.dma_start(out=masks_s, in_=masks_d[:])
                        nc.gpsimd.dma_start(out=woT_s, in_=woP_d[:])
                    return xt, cost, sint

                # RoPE for a finished half-chunk is emitted one half-chunk
                # later so the rot-matmul never stalls the PE on the scalar
                # engine's PSUM->SBUF copy of its input
                pend = []

                def emit_ropes():
                    # two-pass: the t2 muls (the only PSUM readers) run first
                    # so the rot banks release as early as possible
                    work = []
                    while pend:
                        raw, cs, sn, dst = pend.pop(0)
                        rot = ps_rot.tile([128, SC1], f32, tag="rot")
                        nc.tensor.matmul(
                            rot, lhsT=rTs, rhs=raw, start=True, stop=True
                        )
                        work.append((raw, cs, sn, dst, rot))
                    t2s = []
                    for raw, cs, sn, dst, rot in work:
                        t2 = p1t.tile([128, SC1], f32, tag="t2")
                        nc.vector.tensor_mul(t2, rot, sn)
                        t2s.append(t2)
                    for (raw, cs, sn, dst, rot), t2 in zip(work, t2s):
                        t1 = p1t.tile([128, SC1], f32, tag="t1")
                        nc.vector.tensor_mul(t1, raw.bitcast(f32), cs)
                        nc.vector.tensor_add(dst, t1, t2)

                for si in range(n_s1):
                    xt, cost, sint = (
                        (xt0, cost0, sint0) if si == 0 else issue_x(si)
                    )
                    for h in range(HPC):
                        qacc = ps_qk.tile([128, SC1], f32, tag="qa")
                        kacc = ps_qk.tile([128, SC1], f32, tag="ka")
                        # each h-pass carries half the v sub-chunks; two subs
                        # share this 2KB PSUM bank so the accumulation group
                        # (start clears the whole bank's has_written bits)
                        # opens on the first sub and closes on the last
                        vacc = ps_v.tile([128, 2, DLOC], f32, tag="va")
                        for c in range(n_din):
                            nc.tensor.matmul(
                                qacc,
                                lhsT=wA_s[:, c, 0, h * HD : (h + 1) * HD],
                                rhs=xt[:, c, :],
                                start=(c == 0),
                                stop=(c == n_din - 1),
                            )
                            nc.tensor.matmul(
                                kacc,
                                lhsT=wA_s[:, c, 1, h * HD : (h + 1) * HD],
                                rhs=xt[:, c, :],
                                start=(c == 0),
                                stop=(c == n_din - 1),
                            )
                            for jsub in range(2):
                                sub = 2 * h + jsub
                                nc.tensor.matmul(
                                    vacc[:, jsub, :],
                                    lhsT=xt[:, c, sub * 128 : (sub + 1) * 128],
                                    rhs=wA_s[:, c, 2, :],
                                    start=(c == 0 and jsub == 0),
                                    stop=(c == n_din - 1 and jsub == 1),
                                )
                        emit_ropes()
                        nc.scalar.copy(vSs[si][:, 2 * h : 2 * h + 2, :], vacc)
                        rawq = p1t.tile([128, SC1], f32r, tag=f"rawq{h}")
                        nc.scalar.copy(rawq, qacc)
                        rawk = p1t.tile([128, SC1], f32r, tag=f"rawk{h}")
                        nc.scalar.copy(rawk, kacc)
                        pend.append((rawq, cost, sint, qTs[si][:, h, :]))
                        pend.append((rawk, cost, sint, kTs[si][:, h, :]))
                emit_ropes()

            # ------------- phases 2+3: attention + streamed output projection -------------
            with (
                tc.tile_pool(name="persistB", bufs=1) as persistB,
                tc.tile_pool(name="p2", bufs=7) as p2,
                tc.tile_pool(name="p2l", bufs=3) as p2l,
                tc.tile_pool(name="p2r", bufs=2) as p2r,
                tc.tile_pool(name="p3", bufs=3) as p3,
                # creation order fixes PSUM bank assignment: these banks
                # collide with phase-1 pools whose last readers finish at
                # different times -- put ps_st last so its slots land on the
                # banks the phase-1 tail frees earliest
                tc.tile_pool(name="ps_o", bufs=3, space="PSUM") as ps_o,
                tc.tile_pool(name="ps3", bufs=2, space="PSUM") as ps3,
                tc.tile_pool(name="ps_st", bufs=3, space="PSUM") as ps_st,
            ):
                uT = persistB.tile([128, HPC, bs], bf16)  # attn out, [d, h, b*s]
                jobs = deque()          # deferred pv/ones emissions
                p3q = deque()           # deferred output-projection OG bursts

                # bridge warm-up: keep the PE's HAM clock-gate open while the
                # phase-1 DVE/ACT tail drains (the first attention matmuls
                # wait on PSUM-bank anti-deps from that tail). Allocated from
                # ps_o so it lands on bank 0 = phase-1's qa slot 0, whose
                # last reader (the raw-q copy of the final s-chunk's first
                # head) finishes earliest of all banks.
                wscr = ps_o.tile([128, QCH], f32, tag="o", name="wscr")
                for _ in range(60):
                    nc.tensor.matmul(
                        wscr[:, :128], lhsT=ones, rhs=ones, start=True, stop=True
                    )

                def drain(keep):
                    while len(jobs) > keep:
                        jobs.popleft()()

                def make_job(hctx, kc, nkc, pt, bi, h, q0, first, last):
                    def emit():
                        if "outp" not in hctx:
                            # lrep first: with bufs=3 the next h's outp then
                            # lands on this h's lrep slot, which frees at
                            # recip() -- earlier than outp's uT-divide
                            hctx["lrep"] = ps_o.tile(
                                [128, QCH], f32, tag="o", name="lrep"
                            )
                            hctx["outp"] = ps_o.tile(
                                [128, QCH], f32, tag="o", name="outp"
                            )
                        outp, lrep = hctx["outp"], hctx["lrep"]
                        gk = bi * (s // 128) + kc
                        nc.tensor.matmul(
                            outp,
                            lhsT=vSs[gk // n_sub][
                                :, gk % n_sub, h * HD : (h + 1) * HD
                            ],
                            rhs=pt,
                            start=first,
                            stop=last,
                        )
                        # softmax denominator rides the PE per k-chunk: no
                        # cross-engine latency in the in-order matmul queue
                        nc.tensor.matmul(
                            lrep,
                            lhsT=ones,
                            rhs=pt,
                            start=first,
                            stop=last,
                        )
                        if last:
                            rec = p2r.tile([128, QCH], f32, tag="rec")
                            nc.vector.reciprocal_approx_fast(rec, lrep)
                            nc.vector.tensor_mul(
                                uT[:, h, q0 : q0 + QCH], outp, rec
                            )
                    return emit

                og_pool = [ps3]

                def make_og(q0, qg, og):
                    def emit():
                        o0 = og * 256
                        ot = p3.tile([128, 2, QCH], bf16, tag="ot")
                        for j in range(2):
                            pos = og_pool[0].tile(
                                [128, QCH], f32,
                                tag="op" if og_pool[0] is ps3 else "st",
                                name="pos",
                            )
                            for h in range(HPC):
                                nc.tensor.matmul(
                                    pos,
                                    lhsT=woT_s[
                                        :, h, o0 + j * 128 : o0 + (j + 1) * 128
                                    ],
                                    rhs=uT[:, h, q0 : q0 + QCH],
                                    start=(h == 0),
                                    stop=(h == HPC - 1),
                                )
                            if j == 0 and og % 2 == 0:
                                nc.scalar.copy(ot[:, j, :], pos)
                            else:
                                nc.vector.tensor_copy(ot[:, j, :], pos)
                        nc.sync.dma_start(out=outP_d[:, og, qg, :, :], in_=ot)
                    return emit

                for bi in range(b):
                    for qc in range(n_qc):
                        q0 = bi * s + qc * QCH
                        qg = bi * n_qc + qc
                        nkc = (qc + 1) * QCH // 128
                        for h in range(HPC):
                            hctx = {}
                            # k-chunks DESCENDING: the masked diagonal chunks
                            # (whose mask->exp->pair-sum chains are longest)
                            # run first, when their deferred pv/ones pops are
                            # still far away; the final chunk's exp is
                            # mask-free so the closing recip chain is short
                            for ki in range(nkc):
                                kc = nkc - 1 - ki
                                gk = bi * (s // 128) + kc
                                st = ps_st.tile([128, QCH], f32, tag="st")
                                nc.tensor.matmul(
                                    st,
                                    lhsT=kTs[gk // n_sub][
                                        :,
                                        h,
                                        (gk % n_sub) * 128 : (gk % n_sub + 1) * 128,
                                    ],
                                    rhs=qTs[qg][:, h, :],
                                    start=True,
                                    stop=True,
                                )
                                di = kc - (nkc - 4)
                                if di >= 0:
                                    # additive -1e9 causal mask on raw scores
                                    nc.vector.tensor_add(
                                        st, st, masks_s[:, di, :]
                                    )
                                pt = p2.tile([128, QCH], bf16, tag="pt")
                                nc.scalar.activation(pt, st, EXP, scale=scale)
                                jobs.append(
                                    make_job(
                                        hctx, kc, nkc, pt, bi, h, q0,
                                        first=(ki == 0), last=(ki == nkc - 1),
                                    )
                                )
                                drain(DEFER)
                                if ki >= 3 and p3q:
                                    # one output-projection burst of the
                                    # previous q-chunk per k-chunk step
                                    p3q.popleft()()
                        for og in range(n_og):
                            p3q.append(make_og(q0, qg, og))
                drain(0)
                # attention is done: its st banks are free -- deepen the
                # final output bursts' PSUM pipeline with them
                og_pool[0] = ps_st
                while p3q:
                    p3q.popleft()()

    nc.compile()
    return nc


def make_in_maps(x, Wq, Wk, Wv, Wo, b=B, s=S, dim=DIM, n_cores=N_CORES):
    import ml_dtypes

    bf16 = ml_dtypes.bfloat16
    bs = b * s
    n_din = dim // 128
    n_s1 = bs // SC1
    # x pre-shuffled so each [128, c, s-chunk] tile DMA is one contiguous
    # per-partition segment: xP[p, si, c, s'] = x[si*SC1+s', c*128+p]
    xP = np.ascontiguousarray(
        x.reshape(bs, dim).reshape(n_s1, SC1, n_din, 128).transpose(3, 0, 2, 1)
    ).astype(bf16)
    cosT1, sinT1 = _rope_tables_T(s, HD)
    cosT = np.ascontiguousarray(np.tile(cosT1, (1, b)))
    sinT = np.ascontiguousarray(np.tile(sinT1, (1, b)))
    rT = _rot_matrix_T(HD)
    ones = np.ones((HD, HD), dtype=bf16)
    masks = np.ascontiguousarray(_causal_masks(QCH).transpose(1, 0, 2)).astype(bf16)
    in_maps = []
    for c in range(n_cores):
        sl = slice(c * DLOC, (c + 1) * DLOC)
        # packed q/k/v weights: wA[p, c, iw, m] = W_iw.T[c*128+p, m]
        wA = np.ascontiguousarray(
            np.stack([Wq[sl].T, Wk[sl].T, Wv[sl].T], axis=0)
            .reshape(3, n_din, 128, DLOC)
            .transpose(2, 1, 0, 3)
        ).astype(bf16)
        woP = np.ascontiguousarray(
            Wo[:, sl].T.reshape(HPC, 128, dim).transpose(1, 0, 2)
        ).astype(bf16)
        in_maps.append(
            {
                "xP": xP,
                "wA": wA,
                "woP": woP,
                "cosT": cosT,
                "sinT": sinT,
                "rT": rT,
                "ones": ones,
                "masks": masks,
            }
        )
    return in_maps


def kernel(x, Wq, Wk, Wv, Wo, _trace=False):
    """Full-input / full-output entry point. Shards over 8 cores internally."""
    if "/opt/trn_rl_repo" not in sys.path:
        sys.path.insert(0, "/opt/trn_rl_repo")
    from concourse.bass_utils import run_bass_kernel_spmd

    x = np.asarray(x, dtype=np.float32)
    Wq, Wk, Wv, Wo = (np.asarray(w, dtype=np.float32) for w in (Wq, Wk, Wv, Wo))

    key = (B, S, DIM)
    if key not in _PROGRAM_CACHE:
        _PROGRAM_CACHE[key] = build_program(B, S, DIM)
    nc = _PROGRAM_CACHE[key]

    in_maps = make_in_maps(x, Wq, Wk, Wv, Wo)
    res = run_bass_kernel_spmd(
        nc, in_maps, core_ids=list(range(N_CORES)), trace=_trace
    )
    kernel.last_results = res

    acc = None
    for c in range(N_CORES):
        # outP[p, og, qg, j, s'] = out[og*256 + j*128 + p, qg*QCH + s']
        o = res.results[c]["outP"].astype(np.float32)
        o = o.transpose(1, 3, 0, 2, 4).reshape(DIM, B * S)
        acc = o if acc is None else acc + o
    return np.ascontiguousarray(acc.T).reshape(B, S, DIM)
